# revision 3
# baseline (speedup 1.0000x reference)
"""Trainium2 Bass kernel for NeuralPCG GNN message passing (8 NeuronCores).

Strategy: destination-sharded edges (core k owns all edges whose dest node is
in its 2500-node range), feature-major fp16 matmuls.

Per message-passing step, for each edge e=(r,c):
    pre_h = Wf.T @ h_prev  +  P[r]  +  Q[c]          (PSUM accumulate)
    h     = relu(pre_h + b0')                        (one DVE op)
where Wf = W1(s-1) @ eW0c(s) is host-folded (le never materialized),
P[r] comes from a one-hot sel matmul against the local P table, and
Q[c] comes from an SBUF-source dma_gather out of a packed Q table that
is AllGathered across the 8 cores each step.

The mean aggregation scatters le' = W1.T @ h edge-major via per-tile
matmuls (lhsT=h_tile, rhs=W1) then one-hot sel matmuls accumulating
per-block segment sums in PSUM.
"""
import os
import numpy as np
import ml_dtypes
from contextlib import ExitStack

import concourse.bass as bass
import concourse.tile as tile
from concourse import bacc, mybir
from concourse.bass_utils import run_bass_kernel_spmd

N = 20000
E = 320000
L = 128
S = 3
NCORES = 8
NB = 2500            # nodes per core
BLOCKS = 20          # 128-node blocks per core
NPAD = BLOCKS * 128  # 2560
GT = 6               # tiles per gather group (6*128 = 768 idxs;
                     # dma_gather with num_idxs=1024 hangs the device)

F32 = mybir.dt.float32
F16 = mybir.dt.float16
I16 = mybir.dt.int16
AF = mybir.ActivationFunctionType
OP = mybir.AluOpType

NP16 = np.float16

_CACHE = {}


# ----------------------------------------------------------------------------
# Host-side graph preprocessing (index manipulation + sharding only)
# ----------------------------------------------------------------------------

def _wrap_idxs(idx):
    """[n] int -> [128, n//16] int16 wrapped layout for dma_gather."""
    n = idx.shape[0]
    assert n % 16 == 0
    block = idx.reshape(n // 16, 16).T.astype(np.int16)
    return np.tile(block, (8, 1))


def _prep(x, edge_attr, edge_index):
    row = np.asarray(edge_index[0]).astype(np.int64)
    col = np.asarray(edge_index[1]).astype(np.int64)
    ea = np.asarray(edge_attr).reshape(-1).astype(np.float32)
    xf = np.asarray(x).reshape(-1).astype(np.float32)

    cnt_full = np.bincount(row, minlength=N).astype(np.float32)
    core_of = row // NB

    cores = []
    ebc_max = 0
    for k in range(NCORES):
        eids = np.nonzero(core_of == k)[0]
        order = np.argsort(row[eids], kind="stable")
        eids = eids[order]
        blk = (row[eids] - k * NB) // 128
        bc = np.bincount(blk, minlength=BLOCKS)
        ebc_max = max(ebc_max, int(bc.max()))
        cores.append((eids, blk, bc))

    Tb = max(6, (ebc_max + 127) // 128)
    EB = Tb * 128
    Epad = BLOCKS * EB
    ET = Epad // 128  # number of 128-edge tiles
    NCH = Epad // 512 if Epad % 512 == 0 else -1
    assert Epad % 512 == 0

    def trow(n):
        return (n // NB) * NPAD + (n % NB)

    per_core = []
    for k in range(NCORES):
        eids, blk, bc = cores[k]
        r = row[eids]
        c = col[eids]
        starts = np.zeros(BLOCKS, dtype=np.int64)
        np.cumsum(bc[:-1], out=starts[1:])
        pos_in_blk = np.arange(len(eids)) - starts[blk]
        dst = blk * EB + pos_in_blk

        gq = np.zeros(Epad, dtype=np.int64)
        slot = np.full(Epad, -1, dtype=np.int64)
        ea_s = np.ones(Epad, dtype=np.float32)
        dm = np.zeros(Epad, dtype=np.float32)
        orig = np.full(Epad, -1, dtype=np.int64)

        gq[dst] = trow(c)
        slot[dst] = (r - k * NB) % 128
        ea_s[dst] = ea[eids]
        dm[dst] = (r == c).astype(np.float32)
        orig[dst] = eids

        # one-hot (edge-major, for the scatter) and slot-major (for P bcast)
        sel = np.zeros((Epad, 128), dtype=NP16)
        valid = slot >= 0
        vs = np.nonzero(valid)[0]
        sel[vs, slot[valid]] = 1.0
        selT = np.zeros((128, Epad), dtype=NP16)
        selT[slot[valid], vs] = 1.0

        own = cnt_full[k * NB:(k + 1) * NB]
        tmp = np.zeros(BLOCKS * 128, dtype=np.float32)
        tmp[:NB] = own
        cnt_nm = tmp.reshape(BLOCKS, 128).T.copy()

        x_own = np.zeros(NPAD, dtype=np.float32)
        x_own[:NB] = xf[k * NB:(k + 1) * NB]

        em = lambda a: a.reshape(ET, 128).T.copy()  # edge-slot-major [128, ET]
        per_core.append(dict(
            ea_r=ea_s.reshape(NCH, 512),
            ea_em=em(ea_s),
            dm_em=em(dm).astype(NP16),
            dmc_em=em((1.0 - dm) * (slot >= 0)).astype(NP16),
            gq_idx=_wrap_idxs(gq),
            sel=sel,
            selT=selT,
            x_r=x_own.reshape(NPAD // 512, 512),
            cnt_nm=cnt_nm,
            orig=orig,
        ))
    return per_core, Tb


def _weights_inputs(inp):
    """Build the weight/bias input arrays (shared across cores).

    Host-side weight folding:
      Wf[s]   = W1(s-1) @ eW0c(s)      (le never materialized on device)
      b0p[s]  = eb0(s) + eW0c(s).T @ b1(s-1)
      nb0p[s] = nb0(s) + nW0b(s).T @ eb1(s)   (agg carries no b1 term)
      Wdecf   = eW1(2) @ dec_W0 ;  db0p = dec_b0 + dec_W0.T @ eb1(2)
    """
    g = lambda name: np.asarray(inp[name], dtype=np.float32)
    w = {}
    col = lambda a: a.reshape(128, 1).astype(np.float32)

    w["encn_W0"] = g("encn_W0").reshape(1, L)
    w["encn_b0"] = col(g("encn_b0"))
    w["encn_W1h"] = g("encn_W1").astype(NP16)
    w["encn_b1"] = col(g("encn_b1"))
    w["ence_W0"] = g("ence_W0").reshape(1, L)
    w["ence_b0"] = col(g("ence_b0"))
    eW0, eb0, eW1, eb1 = g("eW0"), g("eb0"), g("eW1"), g("eb1")
    nW0, nb0, nW1, nb1 = g("nW0"), g("nb0"), g("nW1"), g("nb1")
    ence_W1, ence_b1 = g("ence_W1"), g("ence_b1")
    for s in range(S):
        eW0c = eW0[s, 2 * L:, :]                       # [L, L]
        W1prev = ence_W1 if s == 0 else eW1[s - 1]
        b1prev = ence_b1 if s == 0 else eb1[s - 1]
        w[f"Wf_{s}"] = (W1prev @ eW0c).astype(NP16)
        w[f"b0p_{s}"] = col(eb0[s] + eW0c.T @ b1prev)
        w[f"eW0ab_{s}"] = np.concatenate(
            [eW0[s, :L, :], eW0[s, L:2 * L, :]], axis=1).astype(NP16)
        w[f"eW1h_{s}"] = eW1[s].astype(NP16)
    for s in range(S - 1):
        w[f"nW0a_{s}"] = nW0[s, :L, :].astype(NP16)
        w[f"nW0bh_{s}"] = nW0[s, L:, :].astype(NP16)
        w[f"nb0p_{s}"] = col(nb0[s] + nW0[s, L:, :].T @ eb1[s])
        w[f"nW1h_{s}"] = nW1[s].astype(NP16)
        w[f"nb1_{s}"] = col(nb1[s])
    dec_W0, dec_b0 = g("dec_W0"), g("dec_b0")
    w["Wdecf"] = (eW1[2] @ dec_W0).astype(NP16)
    w["db0p"] = col(dec_b0 + dec_W0.T @ eb1[2])
    w["dec_W1h"] = g("dec_W1").reshape(L, 1).astype(NP16)
    w["dec_b1"] = np.full((128, 1), float(np.asarray(inp["dec_b1"]).reshape(-1)[0]),
                          dtype=np.float32)
    w["ident"] = np.eye(128, dtype=np.float32)
    w["identh"] = np.eye(128, dtype=NP16)
    return w


# ----------------------------------------------------------------------------
# Device program
# ----------------------------------------------------------------------------

def _build(nc, Tb, w_shapes):
    kb_no_gather = bool(int(os.environ.get("KB_NO_GATHER", "0")))
    kb_no_cc = bool(int(os.environ.get("KB_NO_CC", "0")))
    EB = Tb * 128
    Epad = BLOCKS * EB
    ET = Epad // 128
    NCH = Epad // 512
    # gather groups (in tiles) per block
    groups = []
    t0 = 0
    while t0 < Tb:
        groups.append((t0, min(GT, Tb - t0)))
        t0 += GT

    din = {}

    def inp(name, shape, dtype):
        din[name] = nc.dram_tensor(name, shape, dtype, kind="ExternalInput")
        return din[name]

    inp("ea_r", [NCH, 512], F32)
    inp("ea_em", [128, ET], F32)
    inp("dm_em", [128, ET], F16)
    inp("dmc_em", [128, ET], F16)
    inp("gq_idx", [128, Epad // 16], I16)
    inp("sel", [Epad, 128], F16)
    inp("selT", [128, Epad], F16)
    inp("x_r", [NPAD // 512, 512], F32)
    inp("cnt_nm", [128, BLOCKS], F32)
    for name, arr_shape, np_dtype in w_shapes:
        inp(name, list(arr_shape), F16 if np_dtype == NP16 else F32)

    out_em = nc.dram_tensor("out_em", [128, ET], F32, kind="ExternalOutput")

    with tile.TileContext(nc) as tc, ExitStack() as ctx:
        P = lambda name, bufs, **kw: ctx.enter_context(
            tc.tile_pool(name=name, bufs=bufs, **kw))
        const = P("const", 1)
        big = P("big", 1)
        dram = P("dram", 1, space="DRAM")
        selp = P("selp", 2)      # edge-major one-hot per block
        seltp = P("seltp", 2)    # slot-major one-hot per block
        gath = P("gath", 3)      # gathered Q tiles
        hring = P("hring", 4)
        letring = P("letring", 3)
        sgring = P("sgring", 2)
        rows = P("rows", 2)
        qownring = P("qownring", 2)
        ps_pre = P("ps_pre", 2, space="PSUM")
        ps_let = P("ps_let", 2, space="PSUM")
        ps_g = P("ps_g", 2, space="PSUM")
        ps_m = P("ps_m", 2, space="PSUM")

        # ---- load constants / weights ----
        W = {}
        for name, arr_shape, np_dtype in w_shapes:
            t = const.tile(list(arr_shape), F16 if np_dtype == NP16 else F32,
                           name=f"w_{name}")
            nc.sync.dma_start(t[:], din[name][:])
            W[name] = t
        gq_idx = const.tile([128, Epad // 16], I16, name="gq_idx_s")
        nc.sync.dma_start(gq_idx[:], din["gq_idx"][:])
        cnt = const.tile([128, BLOCKS], F32, name="cnt_s")
        nc.sync.dma_start(cnt[:], din["cnt_nm"][:])
        ea_em = const.tile([128, ET], F32, name="ea_em_s")
        nc.sync.dma_start(ea_em[:], din["ea_em"][:])
        dm_em = const.tile([128, ET], F16, name="dm_em_s")
        nc.sync.dma_start(dm_em[:], din["dm_em"][:])
        dmc_em = const.tile([128, ET], F16, name="dmc_em_s")
        nc.sync.dma_start(dmc_em[:], din["dmc_em"][:])

        invc = const.tile([128, BLOCKS], F32, name="invc")
        nc.vector.tensor_scalar_max(invc[:], cnt[:], 1.0)
        nc.vector.reciprocal(invc[:], invc[:])

        # ---- persistent big tensors ----
        h = big.tile([128, Epad], F16, name="h")          # edge hidden latent
        ln = big.tile([128, NPAD], F32, name="ln")        # own-node latent
        agg = big.tile([128, NPAD], F16, name="agg")      # aggregated messages
        ptab = big.tile([128, BLOCKS, 128], F16, name="ptab")   # local P table
        qtab = big.tile([128, NCORES * BLOCKS, 128], F16, name="qtab")
        dec_em = big.tile([128, ET], F32, name="dec_em")
        cmb = big.tile([128, ET], F32, name="cmb")

        # DRAM staging for the Q-table AllGather (partition-major layout:
        # [128 parts, BLOCKS ranks, 128 feats] per core).
        qown = [dram.tile([128, BLOCKS * 128], F16, name=f"qown_{s}")
                for s in range(S)]
        qfull = [dram.tile([NCORES, 128, BLOCKS * 128], F16,
                           name=f"qfull_{s}", addr_space="Shared")
                 for s in range(S)]

        def mlp_rows(src_dram, nrows, hidden_W0, b0, W1h, b1, dst):
            """dst[:, 512j:...] = W1h.T @ relu(W0 (x) row_j + b0) + b1."""
            for j in range(nrows):
                r = rows.tile([1, 512], F32, tag="rowin")
                nc.sync.dma_start(r[:], src_dram[j:j + 1, :])
                ps = ps_m.tile([128, 512], F32, tag="m")
                nc.tensor.matmul(ps[:], hidden_W0[:], r[:], start=True, stop=True)
                h0 = hring.tile([128, 512], F16, tag="h")
                nc.scalar.activation(h0[:], ps[:], AF.Relu, bias=b0[:])
                ps2 = ps_pre.tile([128, 512], F32, tag="pre")
                nc.tensor.matmul(ps2[:], W1h[:], h0[:], start=True, stop=True)
                nc.scalar.activation(dst[:, 512 * j:512 * (j + 1)], ps2[:],
                                     AF.Identity, bias=b1[:])

        # ---- encoders ----
        # node encoder: full 2-layer MLP -> ln (fp32)
        mlp_rows(din["x_r"], NPAD // 512, W["encn_W0"], W["encn_b0"],
                 W["encn_W1h"], W["encn_b1"], ln)
        # edge encoder: FIRST layer only -> h ; second layer folds into Wf_0
        for j in range(NCH):
            r = rows.tile([1, 512], F32, tag="rowin")
            nc.sync.dma_start(r[:], din["ea_r"][j:j + 1, :])
            ps = ps_m.tile([128, 512], F32, tag="m")
            nc.tensor.matmul(ps[:], W["ence_W0"][:], r[:], start=True, stop=True)
            nc.scalar.activation(h[:, 512 * j:512 * (j + 1)], ps[:],
                                 AF.Relu, bias=W["ence_b0"][:])

        def pq_pass(s):
            """ptab = ln @ eW0a(s) (local); qown = ln @ eW0b(s) -> AllGather
            -> qtab packed [128, 160 ranks, 128]."""
            for j in range(BLOCKS):
                l16 = hring.tile([128, 128], F16, tag="h")
                nc.vector.tensor_copy(l16[:], ln[:, 128 * j:128 * (j + 1)])
                ps = ps_m.tile([128, 256], F32, tag="m")
                nc.tensor.matmul(ps[:], l16[:], W[f"eW0ab_{s}"][:],
                                 start=True, stop=True)
                nc.scalar.activation(ptab[:, j, :], ps[:, 0:128], AF.Copy)
                qt = qownring.tile([128, 128], F16, tag="qown")
                nc.scalar.activation(qt[:], ps[:, 128:256], AF.Copy)
                nc.sync.dma_start(qown[s][:, 128 * j:128 * (j + 1)], qt[:])
            if kb_no_cc:
                nc.sync.dma_start(qfull[s][0, :, :], qown[s][:])
            else:
                nc.gpsimd.collective_compute(
                    "AllGather", OP.bypass,
                    replica_groups=[list(range(NCORES))],
                    ins=[qown[s].opt()],
                    outs=[qfull[s].opt()],
                )
            # pack into SBUF: qtab[p, j*BLOCKS+rk, :] = qfull[j, p, rk*128:...]
            for j in range(NCORES):
                nc.sync.dma_start(
                    qtab[:, j * BLOCKS:(j + 1) * BLOCKS, :],
                    qfull[s][j].rearrange("p (rk f) -> p rk f", f=128))

        pq_pass(0)

        for s in range(S):
            do_agg = s < S - 1
            Wf = W[f"Wf_{s}"]
            b0p = W[f"b0p_{s}"]
            Wscat = W[f"eW1h_{s}"]
            for b in range(BLOCKS):
                if do_agg:
                    g_ps = ps_g.tile([128, 128], F32, tag="g")
                    sel_t = selp.tile([128, Tb, 128], F16, tag="sel")
                    nc.sync.dma_start(
                        sel_t[:],
                        din["sel"][b * EB:(b + 1) * EB, :].rearrange(
                            "(t p) s -> p t s", p=128))
                selT_t = seltp.tile([128, EB], F16, tag="selT")
                nc.sync.dma_start(selT_t[:], din["selT"][:, b * EB:(b + 1) * EB])
                for (gt0, gnt) in groups:
                    i0 = b * EB + gt0 * 128
                    ni = gnt * 128
                    gq_t = gath.tile([128, 1, GT * 128], F16, tag="gq")
                    if kb_no_gather:
                        nc.vector.memset(gq_t[:], 0.0)
                    else:
                        nc.gpsimd.dma_gather(
                            gq_t[:, :, :ni], qtab[:],
                            gq_idx[:, i0 // 16:(i0 + ni) // 16],
                            num_idxs=ni, num_idxs_reg=ni,
                            elem_size=128, transpose=True,
                            sbuf_tokens_per_rank=128,
                            sbuf_free_dim_per_rank=256,
                            sbuf_byte_offset=0)
                    # chunks of <=512 within the group
                    co = 0
                    while co < ni:
                        cw = min(512, ni - co)
                        goff = i0 + co            # global edge-slot offset
                        lo = gt0 * 128 + co       # offset within block
                        ps = ps_pre.tile([128, 512], F32, tag="pre")
                        nc.tensor.matmul(ps[:, :cw], Wf[:],
                                         h[:, goff:goff + cw],
                                         start=True, stop=False)
                        nc.tensor.matmul(ps[:, :cw], ptab[:, b, :],
                                         selT_t[:, lo:lo + cw],
                                         start=False, stop=False)
                        nc.tensor.matmul(ps[:, :cw], W["identh"][:],
                                         gq_t[:, 0, co:co + cw],
                                         start=False, stop=True)
                        nc.vector.tensor_scalar(h[:, goff:goff + cw],
                                                ps[:, :cw], b0p[:],
                                                0.0, op0=OP.add, op1=OP.max)
                        if do_agg:
                            let_ps = ps_let.tile([128, 512], F32, tag="let")
                            for u in range(cw // 128):
                                nc.tensor.matmul(
                                    let_ps[:, 128 * u:128 * (u + 1)],
                                    h[:, goff + 128 * u:goff + 128 * (u + 1)],
                                    Wscat[:], start=True, stop=True,
                                    skip_group_check=True)
                            let_sb = letring.tile([128, 512], F16, tag="let")
                            nc.scalar.activation(let_sb[:, :cw], let_ps[:, :cw],
                                                 AF.Copy)
                            for u in range(cw // 128):
                                tt = (lo // 128) + u
                                nc.tensor.matmul(
                                    g_ps[:], sel_t[:, tt, :],
                                    let_sb[:, 128 * u:128 * (u + 1)],
                                    start=(tt == 0), stop=(tt == Tb - 1),
                                    skip_group_check=True)
                        co += cw
                if do_agg:
                    sg = sgring.tile([128, 128], F32, tag="sg")
                    nc.vector.tensor_scalar_mul(sg[:], g_ps[:], invc[:, b:b + 1])
                    ps_t = ps_m.tile([128, 128], F32, tag="m")
                    nc.tensor.transpose(ps_t[:], sg[:], W["ident"][:])
                    nc.scalar.activation(agg[:, 128 * b:128 * (b + 1)], ps_t[:],
                                         AF.Copy)

            if do_agg:
                # node MLP (in-place ln update), then next-step P/Q tables
                for j in range(NPAD // 512):
                    o = 512 * j
                    ln16 = hring.tile([128, 512], F16, tag="h")
                    nc.vector.tensor_copy(ln16[:], ln[:, o:o + 512])
                    p_ps = ps_pre.tile([128, 512], F32, tag="pre")
                    nc.tensor.matmul(p_ps[:], W[f"nW0a_{s}"][:], ln16[:],
                                     start=True, stop=False)
                    nc.tensor.matmul(p_ps[:], W[f"nW0bh_{s}"][:],
                                     agg[:, o:o + 512], start=False, stop=True)
                    hn = hring.tile([128, 512], F16, tag="h")
                    nc.scalar.activation(hn[:], p_ps[:], AF.Relu,
                                         bias=W[f"nb0p_{s}"][:])
                    l_ps = ps_let.tile([128, 512], F32, tag="let")
                    nc.tensor.matmul(l_ps[:], W[f"nW1h_{s}"][:], hn[:],
                                     start=True, stop=True)
                    nc.scalar.activation(ln[:, o:o + 512], l_ps[:],
                                         AF.Identity, bias=W[f"nb1_{s}"][:])
                pq_pass(s + 1)

        # ---- decoder (fused: dec layer-0 absorbed le = W1(2).T h + b1) ----
        for ci in range(NCH):
            off = 512 * ci
            ps = ps_pre.tile([128, 512], F32, tag="pre")
            nc.tensor.matmul(ps[:], W["Wdecf"][:], h[:, off:off + 512],
                             start=True, stop=True)
            hd = hring.tile([128, 512], F16, tag="h")
            nc.scalar.activation(hd[:], ps[:], AF.Relu, bias=W["db0p"][:])
            d_ps = ps_m.tile([128, 4], F32, tag="m")
            for u in range(4):
                nc.tensor.matmul(d_ps[:, u:u + 1], hd[:, 128 * u:128 * (u + 1)],
                                 W["dec_W1h"][:], start=True, stop=True)
            nc.vector.tensor_scalar_add(dec_em[:, 4 * ci:4 * ci + 4], d_ps[:],
                                        W["dec_b1"][:])

        # ---- final combine: out = dm*0.5*sqrt(ea) + dmc*dec ----
        nc.scalar.sqrt(cmb[:], ea_em[:])
        nc.vector.scalar_tensor_tensor(cmb[:], dm_em[:], 0.5, cmb[:],
                                       op0=OP.mult, op1=OP.mult)
        nc.vector.tensor_tensor(dec_em[:], dmc_em[:], dec_em[:], op=OP.mult)
        nc.vector.tensor_tensor(cmb[:], cmb[:], dec_em[:], op=OP.add)
        nc.sync.dma_start(out_em[:], cmb[:])

    nc.compile()


# ----------------------------------------------------------------------------
# Entry point
# ----------------------------------------------------------------------------

def _get_program(Tb, w_shapes):
    key = Tb
    if key not in _CACHE:
        import time
        t0 = time.time()
        nc = bacc.Bacc("TRN2", target_bir_lowering=False, debug=False,
                       num_devices=NCORES)
        _build(nc, Tb, w_shapes)
        if os.environ.get("KERNEL_VERBOSE"):
            print(f"[kernel] build+schedule+compile: {time.time()-t0:.1f}s",
                  flush=True)
        _CACHE[key] = nc
    return _CACHE[key]


def kernel(**inputs):
    per_core, Tb = _prep(inputs["x"], inputs["edge_attr"], inputs["edge_index"])
    w = _weights_inputs(inputs)
    w_shapes = [(k, v.shape, v.dtype.type) for k, v in w.items()]
    nc = _get_program(Tb, w_shapes)

    in_maps = []
    for k in range(NCORES):
        m = dict(w)
        pc = per_core[k]
        for key in ("ea_r", "ea_em", "dm_em", "dmc_em", "gq_idx",
                    "sel", "selT", "x_r", "cnt_nm"):
            m[key] = pc[key]
        in_maps.append(m)

    trace = bool(int(os.environ.get("KERNEL_TRACE", "0")))
    import time as _time
    _t0 = _time.time()
    res = run_bass_kernel_spmd(
        nc, in_maps, core_ids=list(range(NCORES)), trace=trace,
        tmpdir=os.environ.get("KERNEL_TRACE_DIR") or None)
    if os.environ.get("KERNEL_VERBOSE"):
        print(f"[kernel] exec phase: {_time.time()-_t0:.1f}s", flush=True)
    if trace:
        print(f"HW exec time: {res.exec_time_ns} ns")
        if res.instructions_and_trace:
            print("trace:", res.instructions_and_trace[1])

    out = np.zeros((E, 1), dtype=np.float32)
    ET = (BLOCKS * Tb * 128) // 128
    for k in range(NCORES):
        o = res.results[k]["out_em"]           # [128, ET]
        flat = o.T.reshape(-1)                 # slot order
        orig = per_core[k]["orig"]
        valid = orig >= 0
        out[orig[valid], 0] = flat[valid]
    return out


# revision 5
# speedup vs baseline: 1.0149x; 1.0149x over previous
"""Trainium2 Bass kernel for NeuralPCG GNN message passing (8 NeuronCores).

Strategy: destination-sharded edges (core k owns all edges whose dest node is
in its 2500-node range), feature-major fp16 matmuls.

Per message-passing step, for each edge e=(r,c):
    pre_h = Wf.T @ h_prev  +  P[r]  +  Q[c]          (PSUM accumulate)
    h     = relu(pre_h + b0')                        (one DVE op)
where Wf = W1(s-1) @ eW0c(s) is host-folded (le never materialized),
P[r] comes from a one-hot sel matmul against the local P table, and
Q[c] comes from an SBUF-source dma_gather out of a packed Q table that
is AllGathered across the 8 cores each step.

The mean aggregation scatters le' = W1.T @ h edge-major via per-tile
matmuls (lhsT=h_tile, rhs=W1) then one-hot sel matmuls accumulating
per-block segment sums in PSUM.
"""
import os
import numpy as np
import ml_dtypes
from contextlib import ExitStack

import concourse.bass as bass
import concourse.tile as tile
from concourse import bacc, mybir
from concourse.bass_utils import run_bass_kernel_spmd

N = 20000
E = 320000
L = 128
S = 3
NCORES = 8
NB = 2500            # nodes per core
BLOCKS = 20          # 128-node blocks per core
NPAD = BLOCKS * 128  # 2560
GT = 6               # tiles per gather group (6*128 = 768 idxs;
                     # dma_gather with num_idxs=1024 hangs the device)

F32 = mybir.dt.float32
F16 = mybir.dt.float16
I16 = mybir.dt.int16
AF = mybir.ActivationFunctionType
OP = mybir.AluOpType

NP16 = np.float16

_CACHE = {}


# ----------------------------------------------------------------------------
# Host-side graph preprocessing (index manipulation + sharding only)
# ----------------------------------------------------------------------------

def _wrap_idxs(idx):
    """[n] int -> [128, n//16] int16 wrapped layout for dma_gather."""
    n = idx.shape[0]
    assert n % 16 == 0
    block = idx.reshape(n // 16, 16).T.astype(np.int16)
    return np.tile(block, (8, 1))


def _prep(x, edge_attr, edge_index):
    row = np.asarray(edge_index[0]).astype(np.int64)
    col = np.asarray(edge_index[1]).astype(np.int64)
    ea = np.asarray(edge_attr).reshape(-1).astype(np.float32)
    xf = np.asarray(x).reshape(-1).astype(np.float32)

    cnt_full = np.bincount(row, minlength=N).astype(np.float32)
    core_of = row // NB

    cores = []
    ebc_max = 0
    for k in range(NCORES):
        eids = np.nonzero(core_of == k)[0]
        order = np.argsort(row[eids], kind="stable")
        eids = eids[order]
        blk = (row[eids] - k * NB) // 128
        bc = np.bincount(blk, minlength=BLOCKS)
        ebc_max = max(ebc_max, int(bc.max()))
        cores.append((eids, blk, bc))

    Tb = max(6, (ebc_max + 127) // 128)
    EB = Tb * 128
    Epad = BLOCKS * EB
    ET = Epad // 128  # number of 128-edge tiles
    NCH = Epad // 512 if Epad % 512 == 0 else -1
    assert Epad % 512 == 0

    def trow(n):
        return (n // NB) * NPAD + (n % NB)

    per_core = []
    for k in range(NCORES):
        eids, blk, bc = cores[k]
        r = row[eids]
        c = col[eids]
        starts = np.zeros(BLOCKS, dtype=np.int64)
        np.cumsum(bc[:-1], out=starts[1:])
        pos_in_blk = np.arange(len(eids)) - starts[blk]
        dst = blk * EB + pos_in_blk

        gq = np.zeros(Epad, dtype=np.int64)
        slot = np.full(Epad, -1, dtype=np.int64)
        ea_s = np.ones(Epad, dtype=np.float32)
        dm = np.zeros(Epad, dtype=np.float32)
        orig = np.full(Epad, -1, dtype=np.int64)

        gq[dst] = trow(c)
        slot[dst] = (r - k * NB) % 128
        ea_s[dst] = ea[eids]
        dm[dst] = (r == c).astype(np.float32)
        orig[dst] = eids

        # one-hot (edge-major, for the scatter) and slot-major (for P bcast)
        sel = np.zeros((Epad, 128), dtype=NP16)
        valid = slot >= 0
        vs = np.nonzero(valid)[0]
        sel[vs, slot[valid]] = 1.0
        selT = np.zeros((128, Epad), dtype=NP16)
        selT[slot[valid], vs] = 1.0

        own = cnt_full[k * NB:(k + 1) * NB]
        tmp = np.zeros(BLOCKS * 128, dtype=np.float32)
        tmp[:NB] = own
        cnt_nm = tmp.reshape(BLOCKS, 128).T.copy()

        x_own = np.zeros(NPAD, dtype=np.float32)
        x_own[:NB] = xf[k * NB:(k + 1) * NB]

        em = lambda a: a.reshape(ET, 128).T.copy()  # edge-slot-major [128, ET]
        per_core.append(dict(
            ea_r=ea_s.reshape(NCH, 512),
            ea_em=em(ea_s),
            dm_em=em(dm).astype(NP16),
            dmc_em=em((1.0 - dm) * (slot >= 0)).astype(NP16),
            gq_idx=_wrap_idxs(gq),
            sel=sel,
            selT=selT,
            x_r=x_own.reshape(NPAD // 512, 512),
            cnt_nm=cnt_nm,
            orig=orig,
        ))
    return per_core, Tb


def _weights_inputs(inp):
    """Build the weight/bias input arrays (shared across cores).

    Host-side weight folding:
      Wf[s]   = W1(s-1) @ eW0c(s)      (le never materialized on device)
      b0p[s]  = eb0(s) + eW0c(s).T @ b1(s-1)
      nb0p[s] = nb0(s) + nW0b(s).T @ eb1(s)   (agg carries no b1 term)
      Wdecf   = eW1(2) @ dec_W0 ;  db0p = dec_b0 + dec_W0.T @ eb1(2)
    """
    g = lambda name: np.asarray(inp[name], dtype=np.float32)
    w = {}
    col = lambda a: a.reshape(128, 1).astype(np.float32)

    w["encn_W0"] = g("encn_W0").reshape(1, L)
    w["encn_b0"] = col(g("encn_b0"))
    w["encn_W1h"] = g("encn_W1").astype(NP16)
    w["encn_b1"] = col(g("encn_b1"))
    w["ence_W0"] = g("ence_W0").reshape(1, L)
    w["ence_b0"] = col(g("ence_b0"))
    eW0, eb0, eW1, eb1 = g("eW0"), g("eb0"), g("eW1"), g("eb1")
    nW0, nb0, nW1, nb1 = g("nW0"), g("nb0"), g("nW1"), g("nb1")
    ence_W1, ence_b1 = g("ence_W1"), g("ence_b1")
    for s in range(S):
        eW0c = eW0[s, 2 * L:, :]                       # [L, L]
        W1prev = ence_W1 if s == 0 else eW1[s - 1]
        b1prev = ence_b1 if s == 0 else eb1[s - 1]
        w[f"Wf_{s}"] = (W1prev @ eW0c).astype(NP16)
        w[f"b0p_{s}"] = col(eb0[s] + eW0c.T @ b1prev)
        w[f"eW0ab_{s}"] = np.concatenate(
            [eW0[s, :L, :], eW0[s, L:2 * L, :]], axis=1).astype(NP16)
        w[f"eW1h_{s}"] = eW1[s].astype(NP16)
    for s in range(S - 1):
        w[f"nW0a_{s}"] = nW0[s, :L, :].astype(NP16)
        w[f"nW0bh_{s}"] = nW0[s, L:, :].astype(NP16)
        w[f"nb0p_{s}"] = col(nb0[s] + nW0[s, L:, :].T @ eb1[s])
        w[f"nW1h_{s}"] = nW1[s].astype(NP16)
        w[f"nb1_{s}"] = col(nb1[s])
    dec_W0, dec_b0 = g("dec_W0"), g("dec_b0")
    w["Wdecf"] = (eW1[2] @ dec_W0).astype(NP16)
    w["db0p"] = col(dec_b0 + dec_W0.T @ eb1[2])
    w["dec_W1h"] = g("dec_W1").reshape(L, 1).astype(NP16)
    w["dec_b1"] = np.full((128, 1), float(np.asarray(inp["dec_b1"]).reshape(-1)[0]),
                          dtype=np.float32)
    w["ident"] = np.eye(128, dtype=np.float32)
    w["identh"] = np.eye(128, dtype=NP16)
    return w


# ----------------------------------------------------------------------------
# Device program
# ----------------------------------------------------------------------------

def _build(nc, Tb, w_shapes):
    kb_no_gather = bool(int(os.environ.get("KB_NO_GATHER", "0")))
    kb_no_cc = bool(int(os.environ.get("KB_NO_CC", "0")))
    kb_sp = bool(int(os.environ.get("KB_SP", "1")))
    kb_nq = int(os.environ.get("KB_NQ", "1"))
    EB = Tb * 128
    Epad = BLOCKS * EB
    ET = Epad // 128
    NCH = Epad // 512
    # gather groups (in tiles) per block
    groups = []
    t0 = 0
    while t0 < Tb:
        groups.append((t0, min(GT, Tb - t0)))
        t0 += GT

    din = {}

    def inp(name, shape, dtype):
        din[name] = nc.dram_tensor(name, shape, dtype, kind="ExternalInput")
        return din[name]

    inp("ea_r", [NCH, 512], F32)
    inp("ea_em", [128, ET], F32)
    inp("dm_em", [128, ET], F16)
    inp("dmc_em", [128, ET], F16)
    inp("gq_idx", [128, Epad // 16], I16)
    inp("sel", [Epad, 128], F16)
    inp("selT", [128, Epad], F16)
    inp("x_r", [NPAD // 512, 512], F32)
    inp("cnt_nm", [128, BLOCKS], F32)
    for name, arr_shape, np_dtype in w_shapes:
        inp(name, list(arr_shape), F16 if np_dtype == NP16 else F32)

    out_em = nc.dram_tensor("out_em", [128, ET], F32, kind="ExternalOutput")

    with tile.TileContext(nc) as tc, ExitStack() as ctx:
        P = lambda name, bufs, **kw: ctx.enter_context(
            tc.tile_pool(name=name, bufs=bufs, **kw))
        const = P("const", 1)
        big = P("big", 1)
        dram = P("dram", 1, space="DRAM")
        selp = P("selp", 2)      # edge-major one-hot per block
        seltp = P("seltp", 2)    # slot-major one-hot per block
        gath = P("gath", 3)      # gathered Q tiles
        hring = P("hring", 4)
        letring = P("letring", 3)
        sgring = P("sgring", 2)
        rows = P("rows", 2)
        qownring = P("qownring", 2)
        ps_pre = P("ps_pre", 2, space="PSUM")
        ps_let = P("ps_let", 2, space="PSUM")
        ps_g = P("ps_g", 2, space="PSUM")
        ps_m = P("ps_m", 2, space="PSUM")

        # ---- load constants / weights ----
        W = {}
        for name, arr_shape, np_dtype in w_shapes:
            t = const.tile(list(arr_shape), F16 if np_dtype == NP16 else F32,
                           name=f"w_{name}")
            nc.sync.dma_start(t[:], din[name][:])
            W[name] = t
        gq_idx = const.tile([128, Epad // 16], I16, name="gq_idx_s")
        nc.sync.dma_start(gq_idx[:], din["gq_idx"][:])
        cnt = const.tile([128, BLOCKS], F32, name="cnt_s")
        nc.sync.dma_start(cnt[:], din["cnt_nm"][:])
        ea_em = const.tile([128, ET], F32, name="ea_em_s")
        nc.sync.dma_start(ea_em[:], din["ea_em"][:])
        dm_em = const.tile([128, ET], F16, name="dm_em_s")
        nc.sync.dma_start(dm_em[:], din["dm_em"][:])
        dmc_em = const.tile([128, ET], F16, name="dmc_em_s")
        nc.sync.dma_start(dmc_em[:], din["dmc_em"][:])

        invc = const.tile([128, BLOCKS], F32, name="invc")
        nc.vector.tensor_scalar_max(invc[:], cnt[:], 1.0)
        nc.vector.reciprocal(invc[:], invc[:])

        # ---- persistent big tensors ----
        h = big.tile([128, Epad], F16, name="h")          # edge hidden latent
        ln = big.tile([128, NPAD], F32, name="ln")        # own-node latent
        agg = big.tile([128, NPAD], F16, name="agg")      # aggregated messages
        ptab = big.tile([128, BLOCKS, 128], F16, name="ptab")   # local P table
        qtab = big.tile([128, NCORES * BLOCKS, 128], F16, name="qtab")
        dec_em = big.tile([128, ET], F32, name="dec_em")
        cmb = big.tile([128, ET], F32, name="cmb")

        # DRAM staging for the Q-table AllGather (partition-major layout:
        # [128 parts, BLOCKS ranks, 128 feats] per core).
        qown = [dram.tile([128, BLOCKS * 128], F16, name=f"qown_{s}")
                for s in range(S)]
        qfull = [dram.tile([NCORES, 128, BLOCKS * 128], F16,
                           name=f"qfull_{s}", addr_space="Shared")
                 for s in range(S)]

        def mlp_rows(src_dram, nrows, hidden_W0, b0, W1h, b1, dst):
            """dst[:, 512j:...] = W1h.T @ relu(W0 (x) row_j + b0) + b1."""
            for j in range(nrows):
                r = rows.tile([1, 512], F32, tag="rowin")
                nc.sync.dma_start(r[:], src_dram[j:j + 1, :])
                ps = ps_m.tile([128, 512], F32, tag="m")
                nc.tensor.matmul(ps[:], hidden_W0[:], r[:], start=True, stop=True)
                h0 = hring.tile([128, 512], F16, tag="h")
                nc.scalar.activation(h0[:], ps[:], AF.Relu, bias=b0[:])
                ps2 = ps_pre.tile([128, 512], F32, tag="pre")
                nc.tensor.matmul(ps2[:], W1h[:], h0[:], start=True, stop=True)
                nc.scalar.activation(dst[:, 512 * j:512 * (j + 1)], ps2[:],
                                     AF.Identity, bias=b1[:])

        # ---- encoders ----
        # node encoder: full 2-layer MLP -> ln (fp32)
        mlp_rows(din["x_r"], NPAD // 512, W["encn_W0"], W["encn_b0"],
                 W["encn_W1h"], W["encn_b1"], ln)
        # edge encoder: FIRST layer only -> h ; second layer folds into Wf_0
        for j in range(NCH):
            r = rows.tile([1, 512], F32, tag="rowin")
            nc.sync.dma_start(r[:], din["ea_r"][j:j + 1, :])
            ps = ps_m.tile([128, 512], F32, tag="m")
            nc.tensor.matmul(ps[:], W["ence_W0"][:], r[:], start=True, stop=True)
            nc.scalar.activation(h[:, 512 * j:512 * (j + 1)], ps[:],
                                 AF.Relu, bias=W["ence_b0"][:])

        def pq_pass(s):
            """ptab = ln @ eW0a(s) (local); qown = ln @ eW0b(s) -> AllGather
            -> qtab packed [128, 160 ranks, 128]."""
            for j in range(BLOCKS):
                l16 = hring.tile([128, 128], F16, tag="h")
                nc.vector.tensor_copy(l16[:], ln[:, 128 * j:128 * (j + 1)])
                ps = ps_m.tile([128, 256], F32, tag="m")
                nc.tensor.matmul(ps[:], l16[:], W[f"eW0ab_{s}"][:],
                                 start=True, stop=True)
                nc.scalar.activation(ptab[:, j, :], ps[:, 0:128], AF.Copy)
                qt = qownring.tile([128, 128], F16, tag="qown")
                nc.scalar.activation(qt[:], ps[:, 128:256], AF.Copy)
                nc.sync.dma_start(qown[s][:, 128 * j:128 * (j + 1)], qt[:])
            if kb_no_cc:
                nc.sync.dma_start(qfull[s][0, :, :], qown[s][:])
            else:
                nc.gpsimd.collective_compute(
                    "AllGather", OP.bypass,
                    replica_groups=[list(range(NCORES))],
                    ins=[qown[s].opt()],
                    outs=[qfull[s].opt()],
                )
            # pack into SBUF: qtab[p, j*BLOCKS+rk, :] = qfull[j, p, rk*128:...]
            for j in range(NCORES):
                nc.sync.dma_start(
                    qtab[:, j * BLOCKS:(j + 1) * BLOCKS, :],
                    qfull[s][j].rearrange("p (rk f) -> p rk f", f=128))

        pq_pass(0)

        for s in range(S):
            do_agg = s < S - 1
            Wf = W[f"Wf_{s}"]
            b0p = W[f"b0p_{s}"]
            Wscat = W[f"eW1h_{s}"]
            for b in range(BLOCKS):
                if do_agg:
                    g_ps = ps_g.tile([128, 128], F32, tag="g")
                    sel_t = selp.tile([128, Tb, 128], F16, tag="sel")
                    nc.sync.dma_start(
                        sel_t[:],
                        din["sel"][b * EB:(b + 1) * EB, :].rearrange(
                            "(t p) s -> p t s", p=128))
                selT_t = seltp.tile([128, EB], F16, tag="selT")
                nc.sync.dma_start(selT_t[:], din["selT"][:, b * EB:(b + 1) * EB])
                for gi, (gt0, gnt) in enumerate(groups):
                    i0 = b * EB + gt0 * 128
                    ni = gnt * 128
                    gq_t = gath.tile([128, 1, GT * 128], F16, tag="gq")
                    if kb_no_gather:
                        nc.vector.memset(gq_t[:], 0.0)
                    else:
                        nc.gpsimd.dma_gather(
                            gq_t[:, :, :ni], qtab[:],
                            gq_idx[:, i0 // 16:(i0 + ni) // 16],
                            num_idxs=ni, num_idxs_reg=ni,
                            elem_size=128, transpose=True,
                            single_packet=kb_sp,
                            queue_num=(b * len(groups) + gi) % kb_nq,
                            sbuf_tokens_per_rank=128,
                            sbuf_free_dim_per_rank=256,
                            sbuf_byte_offset=0)
                    # chunks of <=512 within the group
                    co = 0
                    while co < ni:
                        cw = min(512, ni - co)
                        goff = i0 + co            # global edge-slot offset
                        lo = gt0 * 128 + co       # offset within block
                        ps = ps_pre.tile([128, 512], F32, tag="pre")
                        nc.tensor.matmul(ps[:, :cw], Wf[:],
                                         h[:, goff:goff + cw],
                                         start=True, stop=False)
                        nc.tensor.matmul(ps[:, :cw], ptab[:, b, :],
                                         selT_t[:, lo:lo + cw],
                                         start=False, stop=False)
                        nc.tensor.matmul(ps[:, :cw], W["identh"][:],
                                         gq_t[:, 0, co:co + cw],
                                         start=False, stop=True)
                        nc.vector.tensor_scalar(h[:, goff:goff + cw],
                                                ps[:, :cw], b0p[:],
                                                0.0, op0=OP.add, op1=OP.max)
                        if do_agg:
                            let_ps = ps_let.tile([128, 512], F32, tag="let")
                            for u in range(cw // 128):
                                nc.tensor.matmul(
                                    let_ps[:, 128 * u:128 * (u + 1)],
                                    h[:, goff + 128 * u:goff + 128 * (u + 1)],
                                    Wscat[:], start=True, stop=True,
                                    skip_group_check=True)
                            let_sb = letring.tile([128, 512], F16, tag="let")
                            nc.scalar.activation(let_sb[:, :cw], let_ps[:, :cw],
                                                 AF.Copy)
                            for u in range(cw // 128):
                                tt = (lo // 128) + u
                                nc.tensor.matmul(
                                    g_ps[:], sel_t[:, tt, :],
                                    let_sb[:, 128 * u:128 * (u + 1)],
                                    start=(tt == 0), stop=(tt == Tb - 1),
                                    skip_group_check=True)
                        co += cw
                if do_agg:
                    sg = sgring.tile([128, 128], F32, tag="sg")
                    nc.vector.tensor_scalar_mul(sg[:], g_ps[:], invc[:, b:b + 1])
                    ps_t = ps_m.tile([128, 128], F32, tag="m")
                    nc.tensor.transpose(ps_t[:], sg[:], W["ident"][:])
                    nc.scalar.activation(agg[:, 128 * b:128 * (b + 1)], ps_t[:],
                                         AF.Copy)

            if do_agg:
                # node MLP (in-place ln update), then next-step P/Q tables
                for j in range(NPAD // 512):
                    o = 512 * j
                    ln16 = hring.tile([128, 512], F16, tag="h")
                    nc.vector.tensor_copy(ln16[:], ln[:, o:o + 512])
                    p_ps = ps_pre.tile([128, 512], F32, tag="pre")
                    nc.tensor.matmul(p_ps[:], W[f"nW0a_{s}"][:], ln16[:],
                                     start=True, stop=False)
                    nc.tensor.matmul(p_ps[:], W[f"nW0bh_{s}"][:],
                                     agg[:, o:o + 512], start=False, stop=True)
                    hn = hring.tile([128, 512], F16, tag="h")
                    nc.scalar.activation(hn[:], p_ps[:], AF.Relu,
                                         bias=W[f"nb0p_{s}"][:])
                    l_ps = ps_let.tile([128, 512], F32, tag="let")
                    nc.tensor.matmul(l_ps[:], W[f"nW1h_{s}"][:], hn[:],
                                     start=True, stop=True)
                    nc.scalar.activation(ln[:, o:o + 512], l_ps[:],
                                         AF.Identity, bias=W[f"nb1_{s}"][:])
                pq_pass(s + 1)

        # ---- decoder (fused: dec layer-0 absorbed le = W1(2).T h + b1) ----
        for ci in range(NCH):
            off = 512 * ci
            ps = ps_pre.tile([128, 512], F32, tag="pre")
            nc.tensor.matmul(ps[:], W["Wdecf"][:], h[:, off:off + 512],
                             start=True, stop=True)
            hd = hring.tile([128, 512], F16, tag="h")
            nc.scalar.activation(hd[:], ps[:], AF.Relu, bias=W["db0p"][:])
            d_ps = ps_m.tile([128, 4], F32, tag="m")
            for u in range(4):
                nc.tensor.matmul(d_ps[:, u:u + 1], hd[:, 128 * u:128 * (u + 1)],
                                 W["dec_W1h"][:], start=True, stop=True)
            nc.vector.tensor_scalar_add(dec_em[:, 4 * ci:4 * ci + 4], d_ps[:],
                                        W["dec_b1"][:])

        # ---- final combine: out = dm*0.5*sqrt(ea) + dmc*dec ----
        nc.scalar.sqrt(cmb[:], ea_em[:])
        nc.vector.scalar_tensor_tensor(cmb[:], dm_em[:], 0.5, cmb[:],
                                       op0=OP.mult, op1=OP.mult)
        nc.vector.tensor_tensor(dec_em[:], dmc_em[:], dec_em[:], op=OP.mult)
        nc.vector.tensor_tensor(cmb[:], cmb[:], dec_em[:], op=OP.add)
        nc.sync.dma_start(out_em[:], cmb[:])

    nc.compile()


# ----------------------------------------------------------------------------
# Entry point
# ----------------------------------------------------------------------------

def _get_program(Tb, w_shapes):
    key = Tb
    if key not in _CACHE:
        import time
        t0 = time.time()
        nc = bacc.Bacc("TRN2", target_bir_lowering=False, debug=False,
                       num_devices=NCORES)
        _build(nc, Tb, w_shapes)
        if os.environ.get("KERNEL_VERBOSE"):
            print(f"[kernel] build+schedule+compile: {time.time()-t0:.1f}s",
                  flush=True)
        _CACHE[key] = nc
    return _CACHE[key]


def kernel(**inputs):
    per_core, Tb = _prep(inputs["x"], inputs["edge_attr"], inputs["edge_index"])
    w = _weights_inputs(inputs)
    w_shapes = [(k, v.shape, v.dtype.type) for k, v in w.items()]
    nc = _get_program(Tb, w_shapes)

    in_maps = []
    for k in range(NCORES):
        m = dict(w)
        pc = per_core[k]
        for key in ("ea_r", "ea_em", "dm_em", "dmc_em", "gq_idx",
                    "sel", "selT", "x_r", "cnt_nm"):
            m[key] = pc[key]
        in_maps.append(m)

    trace = bool(int(os.environ.get("KERNEL_TRACE", "0")))
    import time as _time
    _t0 = _time.time()
    res = run_bass_kernel_spmd(
        nc, in_maps, core_ids=list(range(NCORES)), trace=trace,
        tmpdir=os.environ.get("KERNEL_TRACE_DIR") or None)
    if os.environ.get("KERNEL_VERBOSE"):
        print(f"[kernel] exec phase: {_time.time()-_t0:.1f}s", flush=True)
    if trace:
        print(f"HW exec time: {res.exec_time_ns} ns")
        if res.instructions_and_trace:
            print("trace:", res.instructions_and_trace[1])

    out = np.zeros((E, 1), dtype=np.float32)
    ET = (BLOCKS * Tb * 128) // 128
    for k in range(NCORES):
        o = res.results[k]["out_em"]           # [128, ET]
        flat = o.T.reshape(-1)                 # slot order
        orig = per_core[k]["orig"]
        valid = orig >= 0
        out[orig[valid], 0] = flat[valid]
    return out


# revision 22
# speedup vs baseline: 1.0595x; 1.0439x over previous
"""Trainium2 Bass kernel for NeuralPCG GNN message passing (8 NeuronCores).

Strategy: destination-sharded edges (core k owns all edges whose dest node is
in its 2500-node range), feature-major fp16 matmuls.

Per message-passing step, for each edge e=(r,c):
    pre_h = Wf.T @ h_prev  +  P[r]  +  Q[c]          (PSUM accumulate)
    h     = relu(pre_h + b0')                        (one DVE op)
where Wf = W1(s-1) @ eW0c(s) is host-folded (le never materialized),
P[r] comes from a one-hot sel matmul against the local P table, and
Q[c] comes from an SBUF-source dma_gather out of a packed Q table that
is AllGathered across the 8 cores each step.

The mean aggregation scatters le' = W1.T @ h edge-major via per-tile
matmuls (lhsT=h_tile, rhs=W1) then one-hot sel matmuls accumulating
per-block segment sums in PSUM.
"""
import os
import numpy as np
import ml_dtypes
from contextlib import ExitStack

import concourse.bass as bass
import concourse.tile as tile
from concourse import bacc, mybir
from concourse.bass_utils import run_bass_kernel_spmd

N = 20000
E = 320000
L = 128
S = 3
NCORES = 8
NB = 2500            # nodes per core
BLOCKS = 20          # 128-node blocks per core
NPAD = BLOCKS * 128  # 2560
GT = 6               # tiles per gather group (6*128 = 768 idxs;
                     # dma_gather with num_idxs=1024 hangs the device)

F32 = mybir.dt.float32
F16 = mybir.dt.float16
I16 = mybir.dt.int16
AF = mybir.ActivationFunctionType
OP = mybir.AluOpType

NP16 = np.float16

_CACHE = {}


# ----------------------------------------------------------------------------
# Host-side graph preprocessing (index manipulation + sharding only)
# ----------------------------------------------------------------------------

def _wrap_idxs(idx):
    """[n] int -> [128, n//16] int16 wrapped layout for dma_gather."""
    n = idx.shape[0]
    assert n % 16 == 0
    block = idx.reshape(n // 16, 16).T.astype(np.int16)
    return np.tile(block, (8, 1))


def _prep(x, edge_attr, edge_index):
    row = np.asarray(edge_index[0]).astype(np.int64)
    col = np.asarray(edge_index[1]).astype(np.int64)
    ea = np.asarray(edge_attr).reshape(-1).astype(np.float32)
    xf = np.asarray(x).reshape(-1).astype(np.float32)

    cnt_full = np.bincount(row, minlength=N).astype(np.float32)
    core_of = row // NB

    cores = []
    ebc_max = 0
    for k in range(NCORES):
        eids = np.nonzero(core_of == k)[0]
        order = np.argsort(row[eids], kind="stable")
        eids = eids[order]
        blk = (row[eids] - k * NB) // 128
        bc = np.bincount(blk, minlength=BLOCKS)
        ebc_max = max(ebc_max, int(bc.max()))
        cores.append((eids, blk, bc))

    Tb = max(6, (ebc_max + 127) // 128)
    EB = Tb * 128
    Epad = BLOCKS * EB
    ET = Epad // 128  # number of 128-edge tiles
    NCH = Epad // 512 if Epad % 512 == 0 else -1
    assert Epad % 512 == 0

    def trow(n):
        return (n // NB) * NPAD + (n % NB)

    per_core = []
    for k in range(NCORES):
        eids, blk, bc = cores[k]
        r = row[eids]
        c = col[eids]
        starts = np.zeros(BLOCKS, dtype=np.int64)
        np.cumsum(bc[:-1], out=starts[1:])
        pos_in_blk = np.arange(len(eids)) - starts[blk]
        dst = blk * EB + pos_in_blk

        gq = np.zeros(Epad, dtype=np.int64)
        slot = np.full(Epad, -1, dtype=np.int64)
        ea_s = np.ones(Epad, dtype=np.float32)
        dm = np.zeros(Epad, dtype=np.float32)
        orig = np.full(Epad, -1, dtype=np.int64)

        gq[dst] = trow(c)
        slot[dst] = (r - k * NB) % 128
        ea_s[dst] = ea[eids]
        dm[dst] = (r == c).astype(np.float32)
        orig[dst] = eids

        # one-hot (edge-major, for the scatter) and slot-major (for P bcast)
        sel = np.zeros((Epad, 128), dtype=NP16)
        valid = slot >= 0
        vs = np.nonzero(valid)[0]
        sel[vs, slot[valid]] = 1.0
        selT = np.zeros((128, Epad), dtype=NP16)
        selT[slot[valid], vs] = 1.0

        own = cnt_full[k * NB:(k + 1) * NB]
        tmp = np.zeros(BLOCKS * 128, dtype=np.float32)
        tmp[:NB] = own
        cnt_nm = tmp.reshape(BLOCKS, 128).T.copy()

        x_own = np.zeros(NPAD, dtype=np.float32)
        x_own[:NB] = xf[k * NB:(k + 1) * NB]

        em = lambda a: a.reshape(ET, 128).T.copy()  # edge-slot-major [128, ET]
        per_core.append(dict(
            ea_r=ea_s.reshape(NCH, 512),
            ea_em=em(ea_s),
            dm_em=em(dm).astype(NP16),
            dmc_em=em((1.0 - dm) * (slot >= 0)).astype(NP16),
            gq_idx=_wrap_idxs(gq),
            sel=sel,
            selT=selT,
            x_r=x_own.reshape(NPAD // 512, 512),
            cnt_nm=cnt_nm,
            orig=orig,
        ))
    return per_core, Tb


def _weights_inputs(inp):
    """Build the weight/bias input arrays (shared across cores).

    Host-side weight folding:
      Wf[s]   = W1(s-1) @ eW0c(s)      (le never materialized on device)
      b0p[s]  = eb0(s) + eW0c(s).T @ b1(s-1)
      nb0p[s] = nb0(s) + nW0b(s).T @ eb1(s)   (agg carries no b1 term)
      Wdecf   = eW1(2) @ dec_W0 ;  db0p = dec_b0 + dec_W0.T @ eb1(2)
    """
    g = lambda name: np.asarray(inp[name], dtype=np.float32)
    w = {}
    col = lambda a: a.reshape(128, 1).astype(np.float32)

    w["encn_W0"] = g("encn_W0").reshape(1, L)
    w["encn_b0"] = col(g("encn_b0"))
    w["encn_W1h"] = g("encn_W1").astype(NP16)
    w["encn_b1"] = col(g("encn_b1"))
    w["ence_W0"] = g("ence_W0").reshape(1, L)
    w["ence_b0"] = col(g("ence_b0"))
    eW0, eb0, eW1, eb1 = g("eW0"), g("eb0"), g("eW1"), g("eb1")
    nW0, nb0, nW1, nb1 = g("nW0"), g("nb0"), g("nW1"), g("nb1")
    ence_W1, ence_b1 = g("ence_W1"), g("ence_b1")
    for s in range(S):
        eW0c = eW0[s, 2 * L:, :]                       # [L, L]
        W1prev = ence_W1 if s == 0 else eW1[s - 1]
        b1prev = ence_b1 if s == 0 else eb1[s - 1]
        w[f"Wf_{s}"] = (W1prev @ eW0c).astype(NP16)
        w[f"b0p_{s}"] = col(eb0[s] + eW0c.T @ b1prev)
        w[f"eW0ab_{s}"] = np.concatenate(
            [eW0[s, :L, :], eW0[s, L:2 * L, :]], axis=1).astype(NP16)
        w[f"eW1h_{s}"] = eW1[s].astype(NP16)
    for s in range(S - 1):
        w[f"nW0a_{s}"] = nW0[s, :L, :].astype(NP16)
        w[f"nW0bh_{s}"] = nW0[s, L:, :].astype(NP16)
        w[f"nb0p_{s}"] = col(nb0[s] + nW0[s, L:, :].T @ eb1[s])
        w[f"nW1h_{s}"] = nW1[s].astype(NP16)
        w[f"nb1_{s}"] = col(nb1[s])
    dec_W0, dec_b0 = g("dec_W0"), g("dec_b0")
    w["Wdecf"] = (eW1[2] @ dec_W0).astype(NP16)
    w["db0p"] = col(dec_b0 + dec_W0.T @ eb1[2])
    w["dec_W1h"] = g("dec_W1").reshape(L, 1).astype(NP16)
    w["dec_b1"] = np.full((128, 1), float(np.asarray(inp["dec_b1"]).reshape(-1)[0]),
                          dtype=np.float32)
    w["ident"] = np.eye(128, dtype=np.float32)
    w["identh"] = np.eye(128, dtype=NP16)
    return w


# ----------------------------------------------------------------------------
# Device program
# ----------------------------------------------------------------------------

def _build(nc, Tb, w_shapes):
    kb_no_gather = bool(int(os.environ.get("KB_NO_GATHER", "0")))
    kb_no_cc = bool(int(os.environ.get("KB_NO_CC", "0")))
    kb_sp = bool(int(os.environ.get("KB_SP", "1")))
    kb_nq = int(os.environ.get("KB_NQ", "1"))
    kb_hbm = bool(int(os.environ.get("KB_GSRC_HBM", "0")))
    EB = Tb * 128
    Epad = BLOCKS * EB
    ET = Epad // 128
    NCH = Epad // 512
    # gather groups (in tiles) per block
    groups = []
    t0 = 0
    while t0 < Tb:
        groups.append((t0, min(GT, Tb - t0)))
        t0 += GT

    din = {}

    def inp(name, shape, dtype):
        din[name] = nc.dram_tensor(name, shape, dtype, kind="ExternalInput")
        return din[name]

    inp("ea_r", [NCH, 512], F32)
    inp("ea_em", [128, ET], F32)
    inp("dm_em", [128, ET], F16)
    inp("dmc_em", [128, ET], F16)
    inp("gq_idx", [128, Epad // 16], I16)
    inp("sel", [Epad, 128], F16)
    inp("selT", [128, Epad], F16)
    inp("x_r", [NPAD // 512, 512], F32)
    inp("cnt_nm", [128, BLOCKS], F32)
    for name, arr_shape, np_dtype in w_shapes:
        inp(name, list(arr_shape), F16 if np_dtype == NP16 else F32)

    out_em = nc.dram_tensor("out_em", [128, ET], F32, kind="ExternalOutput")

    with tile.TileContext(nc) as tc, ExitStack() as ctx:
        P = lambda name, bufs, **kw: ctx.enter_context(
            tc.tile_pool(name=name, bufs=bufs, **kw))
        const = P("const", 1)
        big = P("big", 1)
        dram = P("dram", 1, space="DRAM")
        selp = P("selp", 2)      # edge-major one-hot per block
        seltp = P("seltp", 2)    # slot-major one-hot per block
        gath = P("gath", 3)      # gathered Q tiles
        hring = P("hring", 4)
        letring = P("letring", 3)
        sgring = P("sgring", 2)
        rows = P("rows", 4)
        qownring = P("qownring", 2)
        ps_pre = P("ps_pre", 2, space="PSUM")
        ps_let = P("ps_let", 2, space="PSUM")
        ps_g = P("ps_g", 2, space="PSUM")
        ps_m = P("ps_m", 2, space="PSUM")

        # ---- load constants / weights ----
        W = {}
        for name, arr_shape, np_dtype in w_shapes:
            t = const.tile(list(arr_shape), F16 if np_dtype == NP16 else F32,
                           name=f"w_{name}")
            nc.sync.dma_start(t[:], din[name][:])
            W[name] = t
        gq_idx = const.tile([128, Epad // 16], I16, name="gq_idx_s")
        nc.sync.dma_start(gq_idx[:], din["gq_idx"][:])
        cnt = const.tile([128, BLOCKS], F32, name="cnt_s")
        nc.sync.dma_start(cnt[:], din["cnt_nm"][:])
        ea_em = const.tile([128, ET], F32, name="ea_em_s")
        nc.sync.dma_start(ea_em[:], din["ea_em"][:])
        dm_em = const.tile([128, ET], F16, name="dm_em_s")
        nc.sync.dma_start(dm_em[:], din["dm_em"][:])
        dmc_em = const.tile([128, ET], F16, name="dmc_em_s")
        nc.sync.dma_start(dmc_em[:], din["dmc_em"][:])

        invc = const.tile([128, BLOCKS], F32, name="invc")
        nc.vector.tensor_scalar_max(invc[:], cnt[:], 1.0)
        nc.vector.reciprocal(invc[:], invc[:])

        # ---- persistent big tensors ----
        h = big.tile([128, Epad], F16, name="h")          # edge hidden latent
        ln = big.tile([128, NPAD], F32, name="ln")        # own-node latent
        agg = big.tile([128, NPAD], F16, name="agg")      # aggregated messages
        ptab = big.tile([128, BLOCKS, 128], F16, name="ptab")   # local P table
        if not kb_hbm:
            qtab = big.tile([128, NCORES * BLOCKS, 128], F16, name="qtab")
        dec_em = big.tile([128, ET], F32, name="dec_em")
        cmb = big.tile([128, ET], F32, name="cmb")

        # DRAM staging for the Q-table AllGather.
        # sbuf-gather mode: partition-major ([128 parts, BLOCKS ranks, 128]).
        # hbm-gather mode: row-major ([NPAD rows, 128]).
        if kb_hbm:
            qown = [dram.tile([NPAD, 128], F16, name=f"qown_{s}")
                    for s in range(S)]
            qfull = [dram.tile([NCORES * NPAD, 128], F16,
                               name=f"qfull_{s}", addr_space="Shared")
                     for s in range(S)]
        else:
            qown = [dram.tile([128, BLOCKS * 128], F16, name=f"qown_{s}")
                    for s in range(S)]
            qfull = [dram.tile([NCORES, 128, BLOCKS * 128], F16,
                               name=f"qfull_{s}", addr_space="Shared")
                     for s in range(S)]

        def mlp_rows(src_dram, nrows, hidden_W0, b0, W1h, b1, dst):
            """dst[:, 512j:...] = W1h.T @ relu(W0 (x) row_j + b0) + b1."""
            for j in range(nrows):
                r = rows.tile([1, 512], F32, tag="rowin")
                nc.sync.dma_start(r[:], src_dram[j:j + 1, :])
                ps = ps_m.tile([128, 512], F32, tag="m")
                nc.tensor.matmul(ps[:], hidden_W0[:], r[:], start=True, stop=True)
                h0 = hring.tile([128, 512], F16, tag="h")
                nc.scalar.activation(h0[:], ps[:], AF.Relu, bias=b0[:])
                ps2 = ps_pre.tile([128, 512], F32, tag="pre")
                nc.tensor.matmul(ps2[:], W1h[:], h0[:], start=True, stop=True)
                nc.scalar.activation(dst[:, 512 * j:512 * (j + 1)], ps2[:],
                                     AF.Identity, bias=b1[:])

        # ---- encoders ----
        # node encoder: full 2-layer MLP -> ln (fp32)
        mlp_rows(din["x_r"], NPAD // 512, W["encn_W0"], W["encn_b0"],
                 W["encn_W1h"], W["encn_b1"], ln)
        # edge encoder: FIRST layer only -> h ; second layer folds into Wf_0.
        for j in range(NCH):
            r = rows.tile([1, 512], F32, tag="rowin")
            nc.sync.dma_start(r[:], din["ea_r"][j:j + 1, :])
            ps = ps_m.tile([128, 512], F32, tag="m")
            nc.tensor.matmul(ps[:], W["ence_W0"][:], r[:], start=True, stop=True)
            nc.scalar.activation(h[:, 512 * j:512 * (j + 1)], ps[:],
                                 AF.Relu, bias=W["ence_b0"][:])

        def pq_pass(s):
            """ptab = ln @ eW0a(s) (local); qown = ln @ eW0b(s) -> AllGather
            -> gather table (SBUF-packed or HBM row-major)."""
            for j in range(BLOCKS):
                l16 = hring.tile([128, 128], F16, tag="h")
                nc.vector.tensor_copy(l16[:], ln[:, 128 * j:128 * (j + 1)])
                ps = ps_m.tile([128, 256], F32, tag="m")
                nc.tensor.matmul(ps[:], l16[:], W[f"eW0ab_{s}"][:],
                                 start=True, stop=True)
                nc.scalar.activation(ptab[:, j, :], ps[:, 0:128], AF.Copy)
                qt = qownring.tile([128, 128], F16, tag="qown")
                nc.scalar.activation(qt[:], ps[:, 128:256], AF.Copy)
                if kb_hbm:
                    nc.sync.dma_start(qown[s][128 * j:128 * (j + 1), :], qt[:])
                else:
                    nc.sync.dma_start(qown[s][:, 128 * j:128 * (j + 1)], qt[:])
            if kb_no_cc:
                if kb_hbm:
                    nc.sync.dma_start(qfull[s][0:NPAD, :], qown[s][:])
                else:
                    nc.sync.dma_start(qfull[s][0, :, :], qown[s][:])
            else:
                nc.gpsimd.collective_compute(
                    "AllGather", OP.bypass,
                    replica_groups=[list(range(NCORES))],
                    ins=[qown[s].opt()],
                    outs=[qfull[s].opt()],
                )
            if not kb_hbm:
                # pack into SBUF: qtab[p, j*BLOCKS+rk, :] = qfull[j,p,rk*128:]
                for j in range(NCORES):
                    nc.sync.dma_start(
                        qtab[:, j * BLOCKS:(j + 1) * BLOCKS, :],
                        qfull[s][j].rearrange("p (rk f) -> p rk f", f=128))

        pq_pass(0)

        for s in range(S):
            do_agg = s < S - 1
            Wf = W[f"Wf_{s}"]
            b0p = W[f"b0p_{s}"]
            Wscat = W[f"eW1h_{s}"]
            for b in range(BLOCKS):
                if do_agg:
                    g_ps = ps_g.tile([128, 128], F32, tag="g")
                    sel_t = selp.tile([128, Tb, 128], F16, tag="sel")
                    nc.sync.dma_start(
                        sel_t[:],
                        din["sel"][b * EB:(b + 1) * EB, :].rearrange(
                            "(t p) s -> p t s", p=128))
                selT_t = seltp.tile([128, EB], F16, tag="selT")
                nc.sync.dma_start(selT_t[:], din["selT"][:, b * EB:(b + 1) * EB])
                for gi, (gt0, gnt) in enumerate(groups):
                    i0 = b * EB + gt0 * 128
                    ni = gnt * 128
                    gq_t = gath.tile([128, 1, GT * 128], F16, tag="gq")
                    if kb_no_gather:
                        nc.vector.memset(gq_t[:], 0.0)
                    elif kb_hbm:
                        nc.gpsimd.dma_gather(
                            gq_t[:, :, :ni], qfull[s][:, :],
                            gq_idx[:, i0 // 16:(i0 + ni) // 16],
                            num_idxs=ni, num_idxs_reg=ni,
                            elem_size=128, elem_step=128, transpose=True,
                            single_packet=kb_sp,
                            queue_num=(b * len(groups) + gi) % kb_nq)
                    else:
                        nc.gpsimd.dma_gather(
                            gq_t[:, :, :ni], qtab[:],
                            gq_idx[:, i0 // 16:(i0 + ni) // 16],
                            num_idxs=ni, num_idxs_reg=ni,
                            elem_size=128, transpose=True,
                            single_packet=kb_sp,
                            queue_num=(b * len(groups) + gi) % kb_nq,
                            sbuf_tokens_per_rank=128,
                            sbuf_free_dim_per_rank=256,
                            sbuf_byte_offset=0)
                    # chunks of <=512 within the group
                    co = 0
                    while co < ni:
                        cw = min(512, ni - co)
                        goff = i0 + co            # global edge-slot offset
                        lo = gt0 * 128 + co       # offset within block
                        ps = ps_pre.tile([128, 512], F32, tag="pre")
                        nc.tensor.matmul(ps[:, :cw], Wf[:],
                                         h[:, goff:goff + cw],
                                         start=True, stop=False)
                        nc.tensor.matmul(ps[:, :cw], ptab[:, b, :],
                                         selT_t[:, lo:lo + cw],
                                         start=False, stop=False)
                        nc.tensor.matmul(ps[:, :cw], W["identh"][:],
                                         gq_t[:, 0, co:co + cw],
                                         start=False, stop=True)
                        nc.vector.tensor_scalar(h[:, goff:goff + cw],
                                                ps[:, :cw], b0p[:],
                                                0.0, op0=OP.add, op1=OP.max)
                        if s == S - 1:
                            # decoder fused into the last step's chunk loop
                            # (dec layer-0 absorbed le = W1(2).T h + b1)
                            dps = ps_let.tile([128, 512], F32, tag="let")
                            nc.tensor.matmul(dps[:, :cw], W["Wdecf"][:],
                                             h[:, goff:goff + cw],
                                             start=True, stop=True)
                            hd = hring.tile([128, 512], F16, tag="h")
                            nc.scalar.activation(hd[:, :cw], dps[:, :cw],
                                                 AF.Relu, bias=W["db0p"][:])
                            d_ps = ps_m.tile([128, 4], F32, tag="m")
                            for u in range(cw // 128):
                                nc.tensor.matmul(
                                    d_ps[:, u:u + 1],
                                    hd[:, 128 * u:128 * (u + 1)],
                                    W["dec_W1h"][:], start=True, stop=True,
                                    skip_group_check=True)
                            nc.vector.tensor_scalar_add(
                                dec_em[:, goff // 128:goff // 128 + cw // 128],
                                d_ps[:, :cw // 128], W["dec_b1"][:])
                        if do_agg:
                            let_ps = ps_let.tile([128, 512], F32, tag="let")
                            for u in range(cw // 128):
                                nc.tensor.matmul(
                                    let_ps[:, 128 * u:128 * (u + 1)],
                                    h[:, goff + 128 * u:goff + 128 * (u + 1)],
                                    Wscat[:], start=True, stop=True,
                                    skip_group_check=True)
                            let_sb = letring.tile([128, 512], F16, tag="let")
                            nc.scalar.activation(let_sb[:, :cw], let_ps[:, :cw],
                                                 AF.Copy)
                            for u in range(cw // 128):
                                tt = (lo // 128) + u
                                nc.tensor.matmul(
                                    g_ps[:], sel_t[:, tt, :],
                                    let_sb[:, 128 * u:128 * (u + 1)],
                                    start=(tt == 0), stop=(tt == Tb - 1),
                                    skip_group_check=True)
                        co += cw
                if do_agg:
                    sg = sgring.tile([128, 128], F32, tag="sg")
                    nc.vector.tensor_scalar_mul(sg[:], g_ps[:], invc[:, b:b + 1])
                    ps_t = ps_m.tile([128, 128], F32, tag="m")
                    nc.tensor.transpose(ps_t[:], sg[:], W["ident"][:])
                    nc.scalar.activation(agg[:, 128 * b:128 * (b + 1)], ps_t[:],
                                         AF.Copy)

            if do_agg:
                # node MLP (in-place ln update), then next-step P/Q tables
                for j in range(NPAD // 512):
                    o = 512 * j
                    ln16 = hring.tile([128, 512], F16, tag="h")
                    nc.vector.tensor_copy(ln16[:], ln[:, o:o + 512])
                    p_ps = ps_pre.tile([128, 512], F32, tag="pre")
                    nc.tensor.matmul(p_ps[:], W[f"nW0a_{s}"][:], ln16[:],
                                     start=True, stop=False)
                    nc.tensor.matmul(p_ps[:], W[f"nW0bh_{s}"][:],
                                     agg[:, o:o + 512], start=False, stop=True)
                    hn = hring.tile([128, 512], F16, tag="h")
                    nc.scalar.activation(hn[:], p_ps[:], AF.Relu,
                                         bias=W[f"nb0p_{s}"][:])
                    l_ps = ps_let.tile([128, 512], F32, tag="let")
                    nc.tensor.matmul(l_ps[:], W[f"nW1h_{s}"][:], hn[:],
                                     start=True, stop=True)
                    nc.scalar.activation(ln[:, o:o + 512], l_ps[:],
                                         AF.Identity, bias=W[f"nb1_{s}"][:])
                pq_pass(s + 1)

        # ---- final combine: out = dm*0.5*sqrt(ea) + dmc*dec ----
        nc.scalar.sqrt(cmb[:], ea_em[:])
        nc.vector.scalar_tensor_tensor(cmb[:], dm_em[:], 0.5, cmb[:],
                                       op0=OP.mult, op1=OP.mult)
        nc.vector.tensor_tensor(dec_em[:], dmc_em[:], dec_em[:], op=OP.mult)
        nc.vector.tensor_tensor(cmb[:], cmb[:], dec_em[:], op=OP.add)
        nc.sync.dma_start(out_em[:], cmb[:])

    nc.compile()


# ----------------------------------------------------------------------------
# Entry point
# ----------------------------------------------------------------------------

def _get_program(Tb, w_shapes):
    key = (Tb, os.environ.get("KB_GSRC_HBM"), os.environ.get("KB_SP"),
           os.environ.get("KB_NQ"))
    if key not in _CACHE:
        import time
        t0 = time.time()
        nc = bacc.Bacc("TRN2", target_bir_lowering=False, debug=False,
                       num_devices=NCORES)
        _build(nc, Tb, w_shapes)
        if os.environ.get("KERNEL_VERBOSE"):
            print(f"[kernel] build+schedule+compile: {time.time()-t0:.1f}s",
                  flush=True)
        _CACHE[key] = nc
    return _CACHE[key]


def kernel(**inputs):
    per_core, Tb = _prep(inputs["x"], inputs["edge_attr"], inputs["edge_index"])
    w = _weights_inputs(inputs)
    w_shapes = [(k, v.shape, v.dtype.type) for k, v in w.items()]
    nc = _get_program(Tb, w_shapes)

    in_maps = []
    for k in range(NCORES):
        m = dict(w)
        pc = per_core[k]
        for key in ("ea_r", "ea_em", "dm_em", "dmc_em", "gq_idx",
                    "sel", "selT", "x_r", "cnt_nm"):
            m[key] = pc[key]
        in_maps.append(m)

    trace = bool(int(os.environ.get("KERNEL_TRACE", "0")))
    import time as _time
    _t0 = _time.time()
    res = run_bass_kernel_spmd(
        nc, in_maps, core_ids=list(range(NCORES)), trace=trace,
        tmpdir=os.environ.get("KERNEL_TRACE_DIR") or None)
    if os.environ.get("KERNEL_VERBOSE"):
        print(f"[kernel] exec phase: {_time.time()-_t0:.1f}s", flush=True)
    if trace:
        print(f"HW exec time: {res.exec_time_ns} ns")
        if res.instructions_and_trace:
            print("trace:", res.instructions_and_trace[1])

    out = np.zeros((E, 1), dtype=np.float32)
    ET = (BLOCKS * Tb * 128) // 128
    for k in range(NCORES):
        o = res.results[k]["out_em"]           # [128, ET]
        flat = o.T.reshape(-1)                 # slot order
        orig = per_core[k]["orig"]
        valid = orig >= 0
        out[orig[valid], 0] = flat[valid]
    return out


# revision 70
# speedup vs baseline: 1.3691x; 1.2923x over previous
"""Trainium2 Bass kernel for NeuralPCG GNN message passing (8 NeuronCores).

Strategy: destination-sharded edges (core k owns all edges whose dest node is
in its 2500-node range), feature-major fp16 matmuls.

Per message-passing step, for each edge e=(r,c):
    pre_h = Wf.T @ h_prev  +  P[r]  +  Q[c]          (PSUM accumulate)
    h     = relu(pre_h + b0')                        (one DVE op)
where Wf = W1(s-1) @ eW0c(s) is host-folded (le never materialized),
P[r] comes from a one-hot sel matmul against the local P table, and
Q[c] comes from an SBUF-source dma_gather out of a packed Q table that
is AllGathered across the 8 cores each step.

The mean aggregation scatters le' = W1.T @ h edge-major via per-tile
matmuls (lhsT=h_tile, rhs=W1) then one-hot sel matmuls accumulating
per-block segment sums in PSUM.
"""
import os
import numpy as np
import ml_dtypes
from contextlib import ExitStack

import concourse.bass as bass
import concourse.tile as tile
from concourse import bacc, mybir
from concourse.bass_utils import run_bass_kernel_spmd

N = 20000
E = 320000
L = 128
S = 3
NCORES = 8
NB = 2500            # nodes per core
BLOCKS = 20          # 128-node blocks per core
NPAD = BLOCKS * 128  # 2560
GT = int(os.environ.get("KB_GT", "6"))  # tiles per gather group
HS1 = int(os.environ.get("KB_HS1", "18"))  # blocks in AllGather half A

F32 = mybir.dt.float32
F16 = mybir.dt.float16
I16 = mybir.dt.int16
AF = mybir.ActivationFunctionType
OP = mybir.AluOpType

NP16 = np.float16

_CACHE = {}


# ----------------------------------------------------------------------------
# Host-side graph preprocessing (index manipulation + sharding only)
# ----------------------------------------------------------------------------

def _wrap_idxs(idx):
    """[n] int -> [128, n//16] int16 wrapped layout for dma_gather."""
    n = idx.shape[0]
    assert n % 16 == 0
    block = idx.reshape(n // 16, 16).T.astype(np.int16)
    return np.tile(block, (8, 1))


def _prep(x, edge_attr, edge_index):
    row = np.asarray(edge_index[0]).astype(np.int64)
    col = np.asarray(edge_index[1]).astype(np.int64)
    ea = np.asarray(edge_attr).reshape(-1).astype(np.float32)
    xf = np.asarray(x).reshape(-1).astype(np.float32)

    cnt_full = np.bincount(row, minlength=N).astype(np.float32)
    core_of = row // NB

    # Bin-pack each core's nodes into its 20 slot-blocks so per-block edge
    # counts are balanced (greedy LPT): the padded tile count Tb is set by
    # the WORST block, and consecutive-id blocks leave ~13% padding.
    posg = np.empty(N, dtype=np.int64)   # node -> within-core slot position
    for k in range(NCORES):
        deg = cnt_full[k * NB:(k + 1) * NB]
        order = np.argsort(-deg, kind="stable")
        bl = np.zeros(BLOCKS)
        bn = np.zeros(BLOCKS, dtype=np.int64)
        pos = np.empty(NB, dtype=np.int64)
        for i in order:
            cand = np.nonzero(bn < 128)[0]
            b = cand[np.argmin(bl[cand])]
            pos[i] = b * 128 + bn[b]
            bn[b] += 1
            bl[b] += deg[i]
        posg[k * NB:(k + 1) * NB] = pos
    blk_of = posg // 128
    slot_of = posg % 128

    cores = []
    ebc_max = 0
    a_ok = True
    for k in range(NCORES):
        eids = np.nonzero(core_of == k)[0]
        blk_e = blk_of[row[eids]]
        # within each block put half-A-referencing edges (col in any core's
        # blocks [0, HS1)) first, so each block's first gather group only
        # needs the big (early) AllGather half
        isA = blk_of[col[eids]] < HS1
        order = np.lexsort((~isA, blk_e))
        eids = eids[order]
        blk = blk_of[row[eids]]
        bc = np.bincount(blk, minlength=BLOCKS)
        bcA = np.bincount(blk[blk_of[col[eids]] < HS1], minlength=BLOCKS)
        if np.any(bcA < np.minimum(bc, GT * 128)):
            a_ok = False
        ebc_max = max(ebc_max, int(bc.max()))
        cores.append((eids, blk, bc))

    Tb = max(6, (ebc_max + 127) // 128)
    EB = Tb * 128
    Epad = BLOCKS * EB
    ET = Epad // 128  # number of 128-edge tiles
    NCH = Epad // 512 if Epad % 512 == 0 else -1
    assert Epad % 512 == 0

    # Gather-table row numbering. Ranks are laid out so each AllGather half
    # is a contiguous slice of qtab: first all cores' blocks [0, HS1), then
    # all cores' blocks [HS1, BLOCKS).
    def trow(n):
        j = n // NB
        b, sl = blk_of[n], slot_of[n]
        g = np.where(b < HS1, j * HS1 + b,
                     NCORES * HS1 + j * (BLOCKS - HS1) + (b - HS1))
        return g * 128 + sl

    per_core = []
    for k in range(NCORES):
        eids, blk, bc = cores[k]
        r = row[eids]
        c = col[eids]
        starts = np.zeros(BLOCKS, dtype=np.int64)
        np.cumsum(bc[:-1], out=starts[1:])
        pos_in_blk = np.arange(len(eids)) - starts[blk]
        dst = blk * EB + pos_in_blk

        gq = np.zeros(Epad, dtype=np.int64)
        slot = np.full(Epad, -1, dtype=np.int64)
        ea_s = np.ones(Epad, dtype=np.float32)
        dm = np.zeros(Epad, dtype=np.float32)
        orig = np.full(Epad, -1, dtype=np.int64)

        gq[dst] = trow(c)
        slot[dst] = slot_of[r]
        ea_s[dst] = ea[eids]
        dm[dst] = (r == c).astype(np.float32)
        orig[dst] = eids

        # one-hot (edge-major, for the scatter) and slot-major (for P bcast)
        sel = np.zeros((Epad, 128), dtype=NP16)
        valid = slot >= 0
        vs = np.nonzero(valid)[0]
        sel[vs, slot[valid]] = 1.0
        selT = np.zeros((128, Epad), dtype=NP16)
        selT[slot[valid], vs] = 1.0

        pos = posg[k * NB:(k + 1) * NB]
        own = cnt_full[k * NB:(k + 1) * NB]
        tmp = np.zeros(BLOCKS * 128, dtype=np.float32)
        tmp[pos] = own
        cnt_nm = tmp.reshape(BLOCKS, 128).T.copy()

        x_own = np.zeros(NPAD, dtype=np.float32)
        x_own[pos] = xf[k * NB:(k + 1) * NB]

        em = lambda a: a.reshape(ET, 128).T.copy()  # edge-slot-major [128, ET]
        per_core.append(dict(
            ea_r=ea_s.reshape(NCH, 512),
            ea_em=em(ea_s),
            dm_em=em(dm).astype(NP16),
            dmc_em=em((1.0 - dm) * (slot >= 0)).astype(NP16),
            gq_idx=_wrap_idxs(gq),
            sel=sel,
            selT=selT,
            x_r=x_own.reshape(NPAD // 512, 512).astype(NP16),
            cnt_nm=cnt_nm,
            orig=orig,
        ))
    return per_core, Tb, a_ok


def _weights_inputs(inp):
    """Build the weight/bias input arrays (shared across cores).

    Host-side weight folding:
      Wf[s]   = W1(s-1) @ eW0c(s)      (le never materialized on device)
      b0p[s]  = eb0(s) + eW0c(s).T @ b1(s-1)
      nb0p[s] = nb0(s) + nW0b(s).T @ eb1(s)   (agg carries no b1 term)
      Wdecf   = eW1(2) @ dec_W0 ;  db0p = dec_b0 + dec_W0.T @ eb1(2)
    """
    g = lambda name: np.asarray(inp[name], dtype=np.float32)
    w = {}
    col = lambda a: a.reshape(128, 1).astype(np.float32)

    w["encn_W0h"] = g("encn_W0").reshape(1, L).astype(NP16)
    w["encn_b0"] = col(g("encn_b0"))
    w["encn_W1h"] = g("encn_W1").astype(NP16)
    w["encn_b1"] = col(g("encn_b1"))
    w["ence_W0"] = g("ence_W0").reshape(1, L)
    w["ence_b0"] = col(g("ence_b0"))
    eW0, eb0, eW1, eb1 = g("eW0"), g("eb0"), g("eW1"), g("eb1")
    nW0, nb0, nW1, nb1 = g("nW0"), g("nb0"), g("nW1"), g("nb1")
    ence_W1, ence_b1 = g("ence_W1"), g("ence_b1")
    for s in range(S):
        eW0c = eW0[s, 2 * L:, :]                       # [L, L]
        W1prev = ence_W1 if s == 0 else eW1[s - 1]
        b1prev = ence_b1 if s == 0 else eb1[s - 1]
        w[f"Wf_{s}"] = (W1prev @ eW0c).astype(NP16)
        w[f"b0p_{s}"] = col(eb0[s] + eW0c.T @ b1prev)
        w[f"eW0ab_{s}"] = np.concatenate(
            [eW0[s, :L, :], eW0[s, L:2 * L, :]], axis=1).astype(NP16)
        w[f"eW1h_{s}"] = eW1[s].astype(NP16)
    for s in range(S - 1):
        w[f"nW0a_{s}"] = nW0[s, :L, :].astype(NP16)
        w[f"nW0bh_{s}"] = nW0[s, L:, :].astype(NP16)
        w[f"nb0p_{s}"] = col(nb0[s] + nW0[s, L:, :].T @ eb1[s])
        w[f"nW1h_{s}"] = nW1[s].astype(NP16)
        w[f"nb1_{s}"] = col(nb1[s])
    dec_W0, dec_b0 = g("dec_W0"), g("dec_b0")
    w["Wdecf"] = (eW1[2] @ dec_W0).astype(NP16)
    w["db0p"] = col(dec_b0 + dec_W0.T @ eb1[2])
    w["dec_W1h"] = g("dec_W1").reshape(L, 1).astype(NP16)
    w["dec_b1"] = np.full((128, 1), float(np.asarray(inp["dec_b1"]).reshape(-1)[0]),
                          dtype=np.float32)
    w["ident"] = np.eye(128, dtype=np.float32)
    return w


# ----------------------------------------------------------------------------
# Device program
# ----------------------------------------------------------------------------

def _build(nc, Tb, w_shapes, a_ok):
    kb_no_gather = bool(int(os.environ.get("KB_NO_GATHER", "0")))
    kb_no_cc = bool(int(os.environ.get("KB_NO_CC", "0")))
    kb_sp = bool(int(os.environ.get("KB_SP", "1")))
    kb_hbm = bool(int(os.environ.get("KB_GSRC_HBM", "0")))
    EB = Tb * 128
    Epad = BLOCKS * EB
    ET = Epad // 128
    NCH = Epad // 512
    # gather groups (in tiles) per block
    groups = []
    t0 = 0
    while t0 < Tb:
        groups.append((t0, min(GT, Tb - t0)))
        t0 += GT

    din = {}

    def inp(name, shape, dtype):
        din[name] = nc.dram_tensor(name, shape, dtype, kind="ExternalInput")
        return din[name]

    inp("ea_r", [NCH, 512], F32)
    inp("ea_em", [128, ET], F32)
    inp("dm_em", [128, ET], F16)
    inp("dmc_em", [128, ET], F16)
    inp("gq_idx", [128, Epad // 16], I16)
    inp("sel", [Epad, 128], F16)
    inp("selT", [128, Epad], F16)
    inp("x_r", [NPAD // 512, 512], F16)
    inp("cnt_nm", [128, BLOCKS], F32)
    for name, arr_shape, np_dtype in w_shapes:
        inp(name, list(arr_shape), F16 if np_dtype == NP16 else F32)

    out_em = nc.dram_tensor("out_em", [128, ET], F32, kind="ExternalOutput")

    with tile.TileContext(nc) as tc, ExitStack() as ctx:
        P = lambda name, bufs, **kw: ctx.enter_context(
            tc.tile_pool(name=name, bufs=bufs, **kw))
        const = P("const", 1)
        big = P("big", 1)
        dram = P("dram", 1, space="DRAM")
        selp = P("selp", 2)      # edge-major one-hot per block
        seltp = P("seltp", 2)    # slot-major one-hot per block
        gath = P("gath", 3)      # gathered Q tiles
        hring = P("hring", 4)
        letring = P("letring", 3)
        sgring = P("sgring", 2)
        rows16 = P("rows16", 2)
        ps_pre = P("ps_pre", 2, space="PSUM")
        ps_let = P("ps_let", 2, space="PSUM")
        ps_g = P("ps_g", 2, space="PSUM")
        ps_m = P("ps_m", 2, space="PSUM")

        # ---- load constants / weights ----
        # node-encoder input rows first: they head the Sync DMA queue so the
        # node-enc -> pq -> AllGather(0) critical path starts immediately
        xrows = const.tile([1, NPAD], F16, name="xrows")
        for j in range(NPAD // 512):
            nc.sync.dma_start(xrows[:, 512 * j:512 * (j + 1)],
                              din["x_r"][j:j + 1, :])
        W = {}
        for name, arr_shape, np_dtype in w_shapes:
            t = const.tile(list(arr_shape), F16 if np_dtype == NP16 else F32,
                           name=f"w_{name}")
            nc.sync.dma_start(t[:], din[name][:])
            W[name] = t
        cnt = const.tile([128, BLOCKS], F32, name="cnt_s")
        nc.sync.dma_start(cnt[:], din["cnt_nm"][:])

        invc = const.tile([128, BLOCKS], F32, name="invc")
        nc.vector.tensor_scalar_max(invc[:], cnt[:], 1.0)
        nc.vector.reciprocal(invc[:], invc[:])

        # ---- persistent big tensors ----
        h = big.tile([128, Epad], F16, name="h")          # edge hidden latent
        ln = big.tile([128, NPAD], F16, name="ln")        # own-node latent
        agg = big.tile([128, NPAD], F16, name="agg")      # aggregated messages
        pqtab = big.tile([128, BLOCKS, 256], F16, name="pqtab")  # local P|Q
        qtab = big.tile([128, NCORES * BLOCKS, 128], F16, name="qtab")
        dec_em = big.tile([128, ET], F32, name="dec_em")

        # DRAM staging for the split Q-table AllGathers (partition-major:
        # [128 parts, nb ranks, 128 feats] per core per part). Uneven 16/4
        # split so the step-end tail collective is small; both sizes are
        # multiples of UNIT=4 so the qtab pack stays a single strided DMA.
        UNIT = 4
        HSPLIT = [(0, HS1), (HS1, BLOCKS)]
        # step 0 ships the whole table in one collective (it is the first cc,
        # so it also absorbs the one-time comm-init barrier); steps 1-2 use
        # the HS1 split so the AllGathers fire mid-step.
        qown = [None] + [
            [dram.tile([128, (b1 - b0) * 128], F16, name=f"qown_{s}_{hf}")
             for hf, (b0, b1) in enumerate(HSPLIT)] for s in range(1, S)]
        qfull = [None] + [
            [dram.tile([NCORES, 128, (b1 - b0) * 128], F16,
                       name=f"qfull_{s}_{hf}", addr_space="Shared")
             for hf, (b0, b1) in enumerate(HSPLIT)] for s in range(1, S)]
        qown0 = dram.tile([128, BLOCKS * 128], F16, name="qown0")
        qfull0 = dram.tile([NCORES, 128, BLOCKS * 128], F16,
                           name="qfull0", addr_space="Shared")

        def half_of(b):
            for hf, (b0, b1) in enumerate(HSPLIT):
                if b0 <= b < b1:
                    return hf, b - b0
            raise AssertionError

        def mlp_rows(src_row, nrows, hidden_W0, b0, W1h, b1, dst):
            """dst[:, 512j:...] = W1h.T @ relu(W0 (x) row_j + b0) + b1."""
            for j in range(nrows):
                ps = ps_m.tile([128, 512], F32, tag="m")
                nc.tensor.matmul(ps[:], hidden_W0[:],
                                 src_row[:, 512 * j:512 * (j + 1)],
                                 start=True, stop=True)
                h0 = hring.tile([128, 512], F16, tag="h")
                nc.scalar.activation(h0[:], ps[:], AF.Relu, bias=b0[:])
                ps2 = ps_pre.tile([128, 512], F32, tag="pre")
                nc.tensor.matmul(ps2[:], W1h[:], h0[:], start=True, stop=True)
                nc.scalar.activation(dst[:, 512 * j:512 * (j + 1)], ps2[:],
                                     AF.Identity, bias=b1[:])

        def pq_block(s, b):
            """pqtab[:, b] = ln_b @ [eW0a(s)|eW0b(s)]; Q half DMAs to qown."""
            ps = ps_m.tile([128, 256], F32, tag="m")
            nc.tensor.matmul(ps[:], ln[:, 128 * b:128 * (b + 1)],
                             W[f"eW0ab_{s}"][:], start=True, stop=True)
            nc.scalar.activation(pqtab[:, b, :], ps[:], AF.Copy)
            if s == 0:
                nc.sync.dma_start(qown0[:, 128 * b:128 * (b + 1)],
                                  pqtab[:, b, 128:256])
            else:
                hf, rb = half_of(b)
                nc.sync.dma_start(qown[s][hf][:, 128 * rb:128 * (rb + 1)],
                                  pqtab[:, b, 128:256])

        def _cc(in_tile, out_tile):
            if kb_no_cc:
                nc.sync.dma_start(out_tile[0, :, :], in_tile[:])
            else:
                nc.gpsimd.collective_compute(
                    "AllGather", OP.bypass,
                    replica_groups=[list(range(NCORES))],
                    ins=[in_tile.opt()],
                    outs=[out_tile.opt()],
                )

        def qshare(s, hf):
            """AllGather one half of the Q table (qtab load happens later)."""
            _cc(qown[s][hf], qfull[s][hf])

        def qtab_load(s, hf):
            """Pack one qfull half into qtab. Issued on the GpSimd engine so
            its FIFO orders these writes after every step-(s-1) gather (Tile
            does not track dma_gather's read of qtab). Ranks are numbered so
            each half is a contiguous qtab slice -> one strided DMA."""
            b0, b1 = HSPLIT[hf]
            nb = b1 - b0
            off = NCORES * sum(e - a for a, e in HSPLIT[:hf])
            src = (qfull0[:, :, 128 * b0:128 * b1] if s == 0
                   else qfull[s][hf][:])
            nc.gpsimd.dma_start(
                qtab[:, off:off + NCORES * nb, :].rearrange(
                    "p (j rk) f -> p j (rk f)", j=NCORES),
                src.rearrange("j p x -> p j x"))

        # ---- encoders / initial tables ----
        # node encoder first so pq_pass(0) + AllGather overlap the edge encoder
        mlp_rows(xrows, NPAD // 512, W["encn_W0h"], W["encn_b0"],
                 W["encn_W1h"], W["encn_b1"], ln)
        for b in range(BLOCKS):
            pq_block(0, b)
        if kb_no_cc:
            nc.sync.dma_start(qfull0[0, :, :], qown0[:])
        else:
            nc.gpsimd.collective_compute(
                "AllGather", OP.bypass,
                replica_groups=[list(range(NCORES))],
                ins=[qown0.opt()],
                outs=[qfull0.opt()],
            )
        # bulk constants, traced after the step-0 collective so they don't
        # delay the node encoder -> pq -> AllGather critical path
        # gq_idx is consumed by dma_gather, whose input reads Tile does not
        # track; issue its load on the GpSimd engine so the SWDGE ring
        # orders it ahead of every gather.
        gq_idx = const.tile([128, Epad // 16], I16, name="gq_idx_s")
        nc.gpsimd.dma_start(gq_idx[:], din["gq_idx"][:])
        ea_em = const.tile([128, ET], F32, name="ea_em_s")
        nc.sync.dma_start(ea_em[:], din["ea_em"][:])
        dm_em = const.tile([128, ET], F16, name="dm_em_s")
        nc.sync.dma_start(dm_em[:], din["dm_em"][:])
        dmc_em = const.tile([128, ET], F16, name="dmc_em_s")
        nc.sync.dma_start(dmc_em[:], din["dmc_em"][:])
        # edge encoder: FIRST layer only -> h ; second layer folds into Wf_0.
        for j in range(NCH):
            r = rows16.tile([1, 512], F16, tag="rowin16")
            nc.sync.dma_start(r[:], din["ea_r"][j:j + 1, :])
            ps = ps_m.tile([128, 512], F32, tag="m")
            nc.tensor.matmul(ps[:], W["ence_W0h"][:], r[:], start=True, stop=True)
            nc.scalar.activation(h[:, 512 * j:512 * (j + 1)], ps[:],
                                 AF.Relu, bias=W["ence_b0"][:])

        for s in range(S):
            do_agg = s < S - 1
            Wf = W[f"Wf_{s}"]
            b0p = W[f"b0p_{s}"]
            Wscat = W[f"eW1h_{s}"]

            gq_store = {}

            def emit_gather(b, gi):
                gt0, gnt = groups[gi]
                i0 = b * EB + gt0 * 128
                ni = gnt * 128
                gq_t = gath.tile([128, 1, GT * 128], F16, tag="gq")
                if kb_no_gather:
                    nc.vector.memset(gq_t[:], 0.0)
                else:
                    nc.gpsimd.dma_gather(
                        gq_t[:, :, :ni], qtab[:],
                        gq_idx[:, i0 // 16:(i0 + ni) // 16],
                        num_idxs=ni, num_idxs_reg=ni,
                        elem_size=128, transpose=True,
                        single_packet=kb_sp,
                        sbuf_tokens_per_rank=128,
                        sbuf_free_dim_per_rank=256,
                        sbuf_byte_offset=0)
                gq_store[(b, gi)] = gq_t

            qtab_load(s, 0)
            if a_ok:
                # each block's first group only references half-A rows, so
                # two of those gathers can run before the half-B table load
                # (covering the tail AllGather + load latency)
                emit_gather(0, 0)
                emit_gather(1, 0)
                emit_gather(2, 0)
            qtab_load(s, 1)
            for b in range(BLOCKS):
                if do_agg:
                    g_ps = ps_g.tile([128, 128], F32, tag="g")
                    sel_t = selp.tile([128, Tb, 128], F16, tag="sel")
                    nc.sync.dma_start(
                        sel_t[:],
                        din["sel"][b * EB:(b + 1) * EB, :].rearrange(
                            "(t p) s -> p t s", p=128))
                selT_t = seltp.tile([128, EB], F16, tag="selT")
                nc.sync.dma_start(selT_t[:], din["selT"][:, b * EB:(b + 1) * EB])
                for gi, (gt0, gnt) in enumerate(groups):
                    i0 = b * EB + gt0 * 128
                    ni = gnt * 128
                    if (b, gi) in gq_store:
                        gq_t = gq_store.pop((b, gi))
                    else:
                        emit_gather(b, gi)
                        gq_t = gq_store.pop((b, gi))
                    # chunks of <=512 within the group
                    co = 0
                    while co < ni:
                        cw = min(512, ni - co)
                        goff = i0 + co            # global edge-slot offset
                        lo = gt0 * 128 + co       # offset within block
                        ps = ps_pre.tile([128, 512], F32, tag="pre")
                        nc.tensor.matmul(ps[:, :cw], Wf[:],
                                         h[:, goff:goff + cw],
                                         start=True, stop=False)
                        nc.tensor.matmul(ps[:, :cw], pqtab[:, b, 0:128],
                                         selT_t[:, lo:lo + cw],
                                         start=False, stop=True)
                        pre16 = hring.tile([128, 512], F16, tag="h")
                        nc.vector.tensor_tensor(pre16[:, :cw], ps[:, :cw],
                                                gq_t[:, 0, co:co + cw],
                                                op=OP.add)
                        nc.vector.tensor_scalar(h[:, goff:goff + cw],
                                                pre16[:, :cw], b0p[:],
                                                0.0, op0=OP.add, op1=OP.max)
                        if s == S - 1:
                            # decoder fused into the last step's chunk loop
                            # (dec layer-0 absorbed le = W1(2).T h + b1)
                            dps = ps_let.tile([128, 512], F32, tag="let")
                            nc.tensor.matmul(dps[:, :cw], W["Wdecf"][:],
                                             h[:, goff:goff + cw],
                                             start=True, stop=True)
                            hd = hring.tile([128, 512], F16, tag="h")
                            nc.scalar.activation(hd[:, :cw], dps[:, :cw],
                                                 AF.Relu, bias=W["db0p"][:])
                            d_ps = ps_m.tile([128, 4], F32, tag="m")
                            for u in range(cw // 128):
                                nc.tensor.matmul(
                                    d_ps[:, u:u + 1],
                                    hd[:, 128 * u:128 * (u + 1)],
                                    W["dec_W1h"][:], start=True, stop=True,
                                    skip_group_check=True)
                            nc.vector.tensor_scalar_add(
                                dec_em[:, goff // 128:goff // 128 + cw // 128],
                                d_ps[:, :cw // 128], W["dec_b1"][:])
                        if do_agg:
                            let_ps = ps_let.tile([128, 512], F32, tag="let")
                            for u in range(cw // 128):
                                nc.tensor.matmul(
                                    let_ps[:, 128 * u:128 * (u + 1)],
                                    h[:, goff + 128 * u:goff + 128 * (u + 1)],
                                    Wscat[:], start=True, stop=True,
                                    skip_group_check=True)
                            let_sb = letring.tile([128, 512], F16, tag="let")
                            nc.scalar.activation(let_sb[:, :cw], let_ps[:, :cw],
                                                 AF.Copy)
                            for u in range(cw // 128):
                                tt = (lo // 128) + u
                                nc.tensor.matmul(
                                    g_ps[:], sel_t[:, tt, :],
                                    let_sb[:, 128 * u:128 * (u + 1)],
                                    start=(tt == 0), stop=(tt == Tb - 1),
                                    skip_group_check=True)
                        co += cw
                if do_agg:
                    # finish this block's aggregation, then immediately run
                    # its node MLP + next-step P/Q so the AllGather halves
                    # launch mid-step instead of serializing at the step end.
                    sg = sgring.tile([128, 128], F32, tag="sg")
                    nc.vector.tensor_scalar_mul(sg[:], g_ps[:], invc[:, b:b + 1])
                    ps_t = ps_m.tile([128, 128], F32, tag="m")
                    nc.tensor.transpose(ps_t[:], sg[:], W["ident"][:])
                    nc.scalar.activation(agg[:, 128 * b:128 * (b + 1)], ps_t[:],
                                         AF.Copy)
                    o = 128 * b
                    p_ps = ps_m.tile([128, 128], F32, tag="m")
                    nc.tensor.matmul(p_ps[:], W[f"nW0a_{s}"][:],
                                     ln[:, o:o + 128], start=True, stop=False)
                    nc.tensor.matmul(p_ps[:], W[f"nW0bh_{s}"][:],
                                     agg[:, o:o + 128], start=False, stop=True)
                    hn = hring.tile([128, 128], F16, tag="h")
                    nc.scalar.activation(hn[:], p_ps[:], AF.Relu,
                                         bias=W[f"nb0p_{s}"][:])
                    l_ps = ps_m.tile([128, 128], F32, tag="m")
                    nc.tensor.matmul(l_ps[:], W[f"nW1h_{s}"][:], hn[:],
                                     start=True, stop=True)
                    nc.scalar.activation(ln[:, o:o + 128], l_ps[:],
                                         AF.Identity, bias=W[f"nb1_{s}"][:])
                    pq_block(s + 1, b)
                    for hf, (b0, b1) in enumerate(HSPLIT):
                        if b == b1 - 1:
                            qshare(s + 1, hf)

        # ---- final combine (in ea_em): out = dm*0.5*sqrt(ea) + dmc*dec ----
        nc.scalar.sqrt(ea_em[:], ea_em[:])
        nc.vector.scalar_tensor_tensor(ea_em[:], dm_em[:], 0.5, ea_em[:],
                                       op0=OP.mult, op1=OP.mult)
        nc.vector.tensor_tensor(dec_em[:], dmc_em[:], dec_em[:], op=OP.mult)
        nc.vector.tensor_tensor(ea_em[:], ea_em[:], dec_em[:], op=OP.add)
        nc.sync.dma_start(out_em[:], ea_em[:])

    nc.compile()


# ----------------------------------------------------------------------------
# Entry point
# ----------------------------------------------------------------------------

def _get_program(Tb, w_shapes):
    key = (Tb, os.environ.get("KB_GSRC_HBM"), os.environ.get("KB_SP"),
           os.environ.get("KB_NQ"))
    if key not in _CACHE:
        import time
        t0 = time.time()
        nc = bacc.Bacc("TRN2", target_bir_lowering=False, debug=False,
                       num_devices=NCORES)
        _build(nc, Tb, w_shapes, a_ok)
        if os.environ.get("KERNEL_VERBOSE"):
            print(f"[kernel] build+schedule+compile: {time.time()-t0:.1f}s",
                  flush=True)
        _CACHE[key] = nc
    return _CACHE[key]


def kernel(**inputs):
    per_core, Tb, a_ok = _prep(inputs["x"], inputs["edge_attr"], inputs["edge_index"])
    w = _weights_inputs(inputs)
    w_shapes = [(k, v.shape, v.dtype.type) for k, v in w.items()]
    nc = _get_program(Tb, w_shapes, a_ok)

    in_maps = []
    for k in range(NCORES):
        m = dict(w)
        pc = per_core[k]
        for key in ("ea_r", "ea_em", "dm_em", "dmc_em", "gq_idx",
                    "sel", "selT", "x_r", "cnt_nm"):
            m[key] = pc[key]
        in_maps.append(m)

    trace = bool(int(os.environ.get("KERNEL_TRACE", "0")))
    import time as _time
    _t0 = _time.time()
    res = run_bass_kernel_spmd(
        nc, in_maps, core_ids=list(range(NCORES)), trace=trace,
        tmpdir=os.environ.get("KERNEL_TRACE_DIR") or None)
    if os.environ.get("KERNEL_VERBOSE"):
        print(f"[kernel] exec phase: {_time.time()-_t0:.1f}s", flush=True)
    if trace:
        print(f"HW exec time: {res.exec_time_ns} ns")
        if res.instructions_and_trace:
            print("trace:", res.instructions_and_trace[1])

    out = np.zeros((E, 1), dtype=np.float32)
    ET = (BLOCKS * Tb * 128) // 128
    for k in range(NCORES):
        o = res.results[k]["out_em"]           # [128, ET]
        flat = o.T.reshape(-1)                 # slot order
        orig = per_core[k]["orig"]
        valid = orig >= 0
        out[orig[valid], 0] = flat[valid]
    return out


# revision 71
# speedup vs baseline: 1.3715x; 1.0017x over previous
"""Trainium2 Bass kernel for NeuralPCG GNN message passing (8 NeuronCores).

Strategy: destination-sharded edges (core k owns all edges whose dest node is
in its 2500-node range), feature-major fp16 matmuls.

Per message-passing step, for each edge e=(r,c):
    pre_h = Wf.T @ h_prev  +  P[r]  +  Q[c]          (PSUM accumulate)
    h     = relu(pre_h + b0')                        (one DVE op)
where Wf = W1(s-1) @ eW0c(s) is host-folded (le never materialized),
P[r] comes from a one-hot sel matmul against the local P table, and
Q[c] comes from an SBUF-source dma_gather out of a packed Q table that
is AllGathered across the 8 cores each step.

The mean aggregation scatters le' = W1.T @ h edge-major via per-tile
matmuls (lhsT=h_tile, rhs=W1) then one-hot sel matmuls accumulating
per-block segment sums in PSUM.
"""
import os
import numpy as np
import ml_dtypes
from contextlib import ExitStack

import concourse.bass as bass
import concourse.tile as tile
from concourse import bacc, mybir
from concourse.bass_utils import run_bass_kernel_spmd

N = 20000
E = 320000
L = 128
S = 3
NCORES = 8
NB = 2500            # nodes per core
BLOCKS = 20          # 128-node blocks per core
NPAD = BLOCKS * 128  # 2560
GT = int(os.environ.get("KB_GT", "6"))  # tiles per gather group
HS1 = int(os.environ.get("KB_HS1", "18"))  # blocks in AllGather half A

F32 = mybir.dt.float32
F16 = mybir.dt.float16
I16 = mybir.dt.int16
AF = mybir.ActivationFunctionType
OP = mybir.AluOpType

NP16 = np.float16

_CACHE = {}


# ----------------------------------------------------------------------------
# Host-side graph preprocessing (index manipulation + sharding only)
# ----------------------------------------------------------------------------

def _wrap_idxs(idx):
    """[n] int -> [128, n//16] int16 wrapped layout for dma_gather."""
    n = idx.shape[0]
    assert n % 16 == 0
    block = idx.reshape(n // 16, 16).T.astype(np.int16)
    return np.tile(block, (8, 1))


def _prep(x, edge_attr, edge_index):
    row = np.asarray(edge_index[0]).astype(np.int64)
    col = np.asarray(edge_index[1]).astype(np.int64)
    ea = np.asarray(edge_attr).reshape(-1).astype(np.float32)
    xf = np.asarray(x).reshape(-1).astype(np.float32)

    cnt_full = np.bincount(row, minlength=N).astype(np.float32)
    core_of = row // NB

    # Bin-pack each core's nodes into its 20 slot-blocks so per-block edge
    # counts are balanced (greedy LPT): the padded tile count Tb is set by
    # the WORST block, and consecutive-id blocks leave ~13% padding.
    posg = np.empty(N, dtype=np.int64)   # node -> within-core slot position
    for k in range(NCORES):
        deg = cnt_full[k * NB:(k + 1) * NB]
        order = np.argsort(-deg, kind="stable")
        bl = np.zeros(BLOCKS)
        bn = np.zeros(BLOCKS, dtype=np.int64)
        pos = np.empty(NB, dtype=np.int64)
        for i in order:
            cand = np.nonzero(bn < 128)[0]
            b = cand[np.argmin(bl[cand])]
            pos[i] = b * 128 + bn[b]
            bn[b] += 1
            bl[b] += deg[i]
        posg[k * NB:(k + 1) * NB] = pos
    blk_of = posg // 128
    slot_of = posg % 128

    cores = []
    ebc_max = 0
    a_ok = True
    for k in range(NCORES):
        eids = np.nonzero(core_of == k)[0]
        blk_e = blk_of[row[eids]]
        # within each block put half-A-referencing edges (col in any core's
        # blocks [0, HS1)) first, so each block's first gather group only
        # needs the big (early) AllGather half
        isA = blk_of[col[eids]] < HS1
        order = np.lexsort((~isA, blk_e))
        eids = eids[order]
        blk = blk_of[row[eids]]
        bc = np.bincount(blk, minlength=BLOCKS)
        bcA = np.bincount(blk[blk_of[col[eids]] < HS1], minlength=BLOCKS)
        if np.any(bcA < np.minimum(bc, GT * 128)):
            a_ok = False
        ebc_max = max(ebc_max, int(bc.max()))
        cores.append((eids, blk, bc))

    Tb = max(6, (ebc_max + 127) // 128)
    EB = Tb * 128
    Epad = BLOCKS * EB
    ET = Epad // 128  # number of 128-edge tiles
    NCH = Epad // 512 if Epad % 512 == 0 else -1
    assert Epad % 512 == 0

    # Gather-table row numbering. Ranks are laid out so each AllGather half
    # is a contiguous slice of qtab: first all cores' blocks [0, HS1), then
    # all cores' blocks [HS1, BLOCKS).
    def trow(n):
        j = n // NB
        b, sl = blk_of[n], slot_of[n]
        g = np.where(b < HS1, j * HS1 + b,
                     NCORES * HS1 + j * (BLOCKS - HS1) + (b - HS1))
        return g * 128 + sl

    per_core = []
    for k in range(NCORES):
        eids, blk, bc = cores[k]
        r = row[eids]
        c = col[eids]
        starts = np.zeros(BLOCKS, dtype=np.int64)
        np.cumsum(bc[:-1], out=starts[1:])
        pos_in_blk = np.arange(len(eids)) - starts[blk]
        dst = blk * EB + pos_in_blk

        gq = np.zeros(Epad, dtype=np.int64)
        slot = np.full(Epad, -1, dtype=np.int64)
        ea_s = np.ones(Epad, dtype=np.float32)
        dm = np.zeros(Epad, dtype=np.float32)
        orig = np.full(Epad, -1, dtype=np.int64)

        gq[dst] = trow(c)
        slot[dst] = slot_of[r]
        ea_s[dst] = ea[eids]
        dm[dst] = (r == c).astype(np.float32)
        orig[dst] = eids

        # one-hot (edge-major, for the scatter) and slot-major (for P bcast)
        sel = np.zeros((Epad, 128), dtype=NP16)
        valid = slot >= 0
        vs = np.nonzero(valid)[0]
        sel[vs, slot[valid]] = 1.0
        selT = np.zeros((128, Epad), dtype=NP16)
        selT[slot[valid], vs] = 1.0

        pos = posg[k * NB:(k + 1) * NB]
        own = cnt_full[k * NB:(k + 1) * NB]
        tmp = np.zeros(BLOCKS * 128, dtype=np.float32)
        tmp[pos] = own
        cnt_nm = tmp.reshape(BLOCKS, 128).T.copy()

        x_own = np.zeros(NPAD, dtype=np.float32)
        x_own[pos] = xf[k * NB:(k + 1) * NB]

        em = lambda a: a.reshape(ET, 128).T.copy()  # edge-slot-major [128, ET]
        per_core.append(dict(
            ea_r=ea_s.reshape(NCH, 512),
            ea_em=em(ea_s),
            dm_em=em(dm).astype(NP16),
            dmc_em=em((1.0 - dm) * (slot >= 0)).astype(NP16),
            gq_idx=_wrap_idxs(gq),
            sel=sel,
            selT=selT,
            x_r=x_own.reshape(NPAD // 512, 512).astype(NP16),
            cnt_nm=cnt_nm,
            orig=orig,
        ))
    return per_core, Tb, a_ok


def _weights_inputs(inp):
    """Build the weight/bias input arrays (shared across cores).

    Host-side weight folding:
      Wf[s]   = W1(s-1) @ eW0c(s)      (le never materialized on device)
      b0p[s]  = eb0(s) + eW0c(s).T @ b1(s-1)
      nb0p[s] = nb0(s) + nW0b(s).T @ eb1(s)   (agg carries no b1 term)
      Wdecf   = eW1(2) @ dec_W0 ;  db0p = dec_b0 + dec_W0.T @ eb1(2)
    """
    g = lambda name: np.asarray(inp[name], dtype=np.float32)
    w = {}
    col = lambda a: a.reshape(128, 1).astype(np.float32)

    w["encn_W0h"] = g("encn_W0").reshape(1, L).astype(NP16)
    w["encn_b0"] = col(g("encn_b0"))
    w["encn_W1h"] = g("encn_W1").astype(NP16)
    w["encn_b1"] = col(g("encn_b1"))
    w["ence_W0"] = g("ence_W0").reshape(1, L)
    w["ence_b0"] = col(g("ence_b0"))
    eW0, eb0, eW1, eb1 = g("eW0"), g("eb0"), g("eW1"), g("eb1")
    nW0, nb0, nW1, nb1 = g("nW0"), g("nb0"), g("nW1"), g("nb1")
    ence_W1, ence_b1 = g("ence_W1"), g("ence_b1")
    for s in range(S):
        eW0c = eW0[s, 2 * L:, :]                       # [L, L]
        W1prev = ence_W1 if s == 0 else eW1[s - 1]
        b1prev = ence_b1 if s == 0 else eb1[s - 1]
        w[f"Wf_{s}"] = (W1prev @ eW0c).astype(NP16)
        w[f"b0p_{s}"] = col(eb0[s] + eW0c.T @ b1prev)
        w[f"eW0ab_{s}"] = np.concatenate(
            [eW0[s, :L, :], eW0[s, L:2 * L, :]], axis=1).astype(NP16)
        w[f"eW1h_{s}"] = eW1[s].astype(NP16)
    for s in range(S - 1):
        w[f"nW0a_{s}"] = nW0[s, :L, :].astype(NP16)
        w[f"nW0bh_{s}"] = nW0[s, L:, :].astype(NP16)
        w[f"nb0p_{s}"] = col(nb0[s] + nW0[s, L:, :].T @ eb1[s])
        w[f"nW1h_{s}"] = nW1[s].astype(NP16)
        w[f"nb1_{s}"] = col(nb1[s])
    dec_W0, dec_b0 = g("dec_W0"), g("dec_b0")
    w["Wdecf"] = (eW1[2] @ dec_W0).astype(NP16)
    w["db0p"] = col(dec_b0 + dec_W0.T @ eb1[2])
    w["dec_W1h"] = g("dec_W1").reshape(L, 1).astype(NP16)
    w["dec_b1"] = np.full((128, 1), float(np.asarray(inp["dec_b1"]).reshape(-1)[0]),
                          dtype=np.float32)
    w["ident"] = np.eye(128, dtype=np.float32)
    return w


# ----------------------------------------------------------------------------
# Device program
# ----------------------------------------------------------------------------

def _build(nc, Tb, w_shapes, a_ok):
    kb_no_gather = bool(int(os.environ.get("KB_NO_GATHER", "0")))
    kb_no_cc = bool(int(os.environ.get("KB_NO_CC", "0")))
    kb_sp = bool(int(os.environ.get("KB_SP", "1")))
    kb_hbm = bool(int(os.environ.get("KB_GSRC_HBM", "0")))
    EB = Tb * 128
    Epad = BLOCKS * EB
    ET = Epad // 128
    NCH = Epad // 512
    # gather groups (in tiles) per block
    groups = []
    t0 = 0
    while t0 < Tb:
        groups.append((t0, min(GT, Tb - t0)))
        t0 += GT

    din = {}

    def inp(name, shape, dtype):
        din[name] = nc.dram_tensor(name, shape, dtype, kind="ExternalInput")
        return din[name]

    inp("ea_r", [NCH, 512], F32)
    inp("ea_em", [128, ET], F32)
    inp("dm_em", [128, ET], F16)
    inp("dmc_em", [128, ET], F16)
    inp("gq_idx", [128, Epad // 16], I16)
    inp("sel", [Epad, 128], F16)
    inp("selT", [128, Epad], F16)
    inp("x_r", [NPAD // 512, 512], F16)
    inp("cnt_nm", [128, BLOCKS], F32)
    for name, arr_shape, np_dtype in w_shapes:
        inp(name, list(arr_shape), F16 if np_dtype == NP16 else F32)

    out_em = nc.dram_tensor("out_em", [128, ET], F32, kind="ExternalOutput")

    with tile.TileContext(nc) as tc, ExitStack() as ctx:
        P = lambda name, bufs, **kw: ctx.enter_context(
            tc.tile_pool(name=name, bufs=bufs, **kw))
        const = P("const", 1)
        big = P("big", 1)
        dram = P("dram", 1, space="DRAM")
        selp = P("selp", 3)      # edge-major one-hot per block
        seltp = P("seltp", 3)    # slot-major one-hot per block
        gath = P("gath", 3)      # gathered Q tiles
        hring = P("hring", 4)
        letring = P("letring", 3)
        sgring = P("sgring", 2)
        rows16 = P("rows16", 2)
        ps_pre = P("ps_pre", 2, space="PSUM")
        ps_let = P("ps_let", 2, space="PSUM")
        ps_g = P("ps_g", 2, space="PSUM")
        ps_m = P("ps_m", 2, space="PSUM")

        # ---- load constants / weights ----
        # node-encoder input rows first: they head the Sync DMA queue so the
        # node-enc -> pq -> AllGather(0) critical path starts immediately
        xrows = const.tile([1, NPAD], F16, name="xrows")
        for j in range(NPAD // 512):
            nc.sync.dma_start(xrows[:, 512 * j:512 * (j + 1)],
                              din["x_r"][j:j + 1, :])
        W = {}
        for name, arr_shape, np_dtype in w_shapes:
            t = const.tile(list(arr_shape), F16 if np_dtype == NP16 else F32,
                           name=f"w_{name}")
            nc.sync.dma_start(t[:], din[name][:])
            W[name] = t
        cnt = const.tile([128, BLOCKS], F32, name="cnt_s")
        nc.sync.dma_start(cnt[:], din["cnt_nm"][:])

        invc = const.tile([128, BLOCKS], F32, name="invc")
        nc.vector.tensor_scalar_max(invc[:], cnt[:], 1.0)
        nc.vector.reciprocal(invc[:], invc[:])

        # ---- persistent big tensors ----
        h = big.tile([128, Epad], F16, name="h")          # edge hidden latent
        ln = big.tile([128, NPAD], F16, name="ln")        # own-node latent
        agg = big.tile([128, NPAD], F16, name="agg")      # aggregated messages
        pqtab = big.tile([128, BLOCKS, 256], F16, name="pqtab")  # local P|Q
        qtab = big.tile([128, NCORES * BLOCKS, 128], F16, name="qtab")
        dec_em = big.tile([128, ET], F32, name="dec_em")

        # DRAM staging for the split Q-table AllGathers (partition-major:
        # [128 parts, nb ranks, 128 feats] per core per part). Uneven 16/4
        # split so the step-end tail collective is small; both sizes are
        # multiples of UNIT=4 so the qtab pack stays a single strided DMA.
        UNIT = 4
        HSPLIT = [(0, HS1), (HS1, BLOCKS)]
        # step 0 ships the whole table in one collective (it is the first cc,
        # so it also absorbs the one-time comm-init barrier); steps 1-2 use
        # the HS1 split so the AllGathers fire mid-step.
        qown = [None] + [
            [dram.tile([128, (b1 - b0) * 128], F16, name=f"qown_{s}_{hf}")
             for hf, (b0, b1) in enumerate(HSPLIT)] for s in range(1, S)]
        qfull = [None] + [
            [dram.tile([NCORES, 128, (b1 - b0) * 128], F16,
                       name=f"qfull_{s}_{hf}", addr_space="Shared")
             for hf, (b0, b1) in enumerate(HSPLIT)] for s in range(1, S)]
        qown0 = dram.tile([128, BLOCKS * 128], F16, name="qown0")
        qfull0 = dram.tile([NCORES, 128, BLOCKS * 128], F16,
                           name="qfull0", addr_space="Shared")

        def half_of(b):
            for hf, (b0, b1) in enumerate(HSPLIT):
                if b0 <= b < b1:
                    return hf, b - b0
            raise AssertionError

        def mlp_rows(src_row, nrows, hidden_W0, b0, W1h, b1, dst):
            """dst[:, 512j:...] = W1h.T @ relu(W0 (x) row_j + b0) + b1."""
            for j in range(nrows):
                ps = ps_m.tile([128, 512], F32, tag="m")
                nc.tensor.matmul(ps[:], hidden_W0[:],
                                 src_row[:, 512 * j:512 * (j + 1)],
                                 start=True, stop=True)
                h0 = hring.tile([128, 512], F16, tag="h")
                nc.scalar.activation(h0[:], ps[:], AF.Relu, bias=b0[:])
                ps2 = ps_pre.tile([128, 512], F32, tag="pre")
                nc.tensor.matmul(ps2[:], W1h[:], h0[:], start=True, stop=True)
                nc.scalar.activation(dst[:, 512 * j:512 * (j + 1)], ps2[:],
                                     AF.Identity, bias=b1[:])

        def pq_block(s, b):
            """pqtab[:, b] = ln_b @ [eW0a(s)|eW0b(s)]; Q half DMAs to qown."""
            ps = ps_m.tile([128, 256], F32, tag="m")
            nc.tensor.matmul(ps[:], ln[:, 128 * b:128 * (b + 1)],
                             W[f"eW0ab_{s}"][:], start=True, stop=True)
            nc.scalar.activation(pqtab[:, b, :], ps[:], AF.Copy)
            if s == 0:
                nc.sync.dma_start(qown0[:, 128 * b:128 * (b + 1)],
                                  pqtab[:, b, 128:256])
            else:
                hf, rb = half_of(b)
                nc.sync.dma_start(qown[s][hf][:, 128 * rb:128 * (rb + 1)],
                                  pqtab[:, b, 128:256])

        def _cc(in_tile, out_tile):
            if kb_no_cc:
                nc.sync.dma_start(out_tile[0, :, :], in_tile[:])
            else:
                nc.gpsimd.collective_compute(
                    "AllGather", OP.bypass,
                    replica_groups=[list(range(NCORES))],
                    ins=[in_tile.opt()],
                    outs=[out_tile.opt()],
                )

        def qshare(s, hf):
            """AllGather one half of the Q table (qtab load happens later)."""
            _cc(qown[s][hf], qfull[s][hf])

        def qtab_load(s, hf):
            """Pack one qfull half into qtab. Issued on the GpSimd engine so
            its FIFO orders these writes after every step-(s-1) gather (Tile
            does not track dma_gather's read of qtab). Ranks are numbered so
            each half is a contiguous qtab slice -> one strided DMA."""
            b0, b1 = HSPLIT[hf]
            nb = b1 - b0
            off = NCORES * sum(e - a for a, e in HSPLIT[:hf])
            src = (qfull0[:, :, 128 * b0:128 * b1] if s == 0
                   else qfull[s][hf][:])
            nc.gpsimd.dma_start(
                qtab[:, off:off + NCORES * nb, :].rearrange(
                    "p (j rk) f -> p j (rk f)", j=NCORES),
                src.rearrange("j p x -> p j x"))

        # ---- encoders / initial tables ----
        # node encoder first so pq_pass(0) + AllGather overlap the edge encoder
        mlp_rows(xrows, NPAD // 512, W["encn_W0h"], W["encn_b0"],
                 W["encn_W1h"], W["encn_b1"], ln)
        for b in range(BLOCKS):
            pq_block(0, b)
        if kb_no_cc:
            nc.sync.dma_start(qfull0[0, :, :], qown0[:])
        else:
            nc.gpsimd.collective_compute(
                "AllGather", OP.bypass,
                replica_groups=[list(range(NCORES))],
                ins=[qown0.opt()],
                outs=[qfull0.opt()],
            )
        # bulk constants, traced after the step-0 collective so they don't
        # delay the node encoder -> pq -> AllGather critical path
        # gq_idx is consumed by dma_gather, whose input reads Tile does not
        # track; issue its load on the GpSimd engine so the SWDGE ring
        # orders it ahead of every gather.
        gq_idx = const.tile([128, Epad // 16], I16, name="gq_idx_s")
        nc.gpsimd.dma_start(gq_idx[:], din["gq_idx"][:])
        ea_em = const.tile([128, ET], F32, name="ea_em_s")
        nc.sync.dma_start(ea_em[:], din["ea_em"][:])
        dm_em = const.tile([128, ET], F16, name="dm_em_s")
        nc.sync.dma_start(dm_em[:], din["dm_em"][:])
        dmc_em = const.tile([128, ET], F16, name="dmc_em_s")
        nc.sync.dma_start(dmc_em[:], din["dmc_em"][:])
        # edge encoder: FIRST layer only -> h ; second layer folds into Wf_0.
        for j in range(NCH):
            r = rows16.tile([1, 512], F16, tag="rowin16")
            nc.sync.dma_start(r[:], din["ea_r"][j:j + 1, :])
            ps = ps_m.tile([128, 512], F32, tag="m")
            nc.tensor.matmul(ps[:], W["ence_W0h"][:], r[:], start=True, stop=True)
            nc.scalar.activation(h[:, 512 * j:512 * (j + 1)], ps[:],
                                 AF.Relu, bias=W["ence_b0"][:])

        for s in range(S):
            do_agg = s < S - 1
            Wf = W[f"Wf_{s}"]
            b0p = W[f"b0p_{s}"]
            Wscat = W[f"eW1h_{s}"]

            gq_store = {}

            def emit_gather(b, gi):
                gt0, gnt = groups[gi]
                i0 = b * EB + gt0 * 128
                ni = gnt * 128
                gq_t = gath.tile([128, 1, GT * 128], F16, tag="gq")
                if kb_no_gather:
                    nc.vector.memset(gq_t[:], 0.0)
                else:
                    nc.gpsimd.dma_gather(
                        gq_t[:, :, :ni], qtab[:],
                        gq_idx[:, i0 // 16:(i0 + ni) // 16],
                        num_idxs=ni, num_idxs_reg=ni,
                        elem_size=128, transpose=True,
                        single_packet=kb_sp,
                        sbuf_tokens_per_rank=128,
                        sbuf_free_dim_per_rank=256,
                        sbuf_byte_offset=0)
                gq_store[(b, gi)] = gq_t

            qtab_load(s, 0)
            if a_ok:
                # each block's first group only references half-A rows, so
                # two of those gathers can run before the half-B table load
                # (covering the tail AllGather + load latency)
                emit_gather(0, 0)
                emit_gather(1, 0)
                emit_gather(2, 0)
            qtab_load(s, 1)
            for b in range(BLOCKS):
                if do_agg:
                    g_ps = ps_g.tile([128, 128], F32, tag="g")
                    sel_t = selp.tile([128, Tb, 128], F16, tag="sel")
                    nc.sync.dma_start(
                        sel_t[:],
                        din["sel"][b * EB:(b + 1) * EB, :].rearrange(
                            "(t p) s -> p t s", p=128))
                selT_t = seltp.tile([128, EB], F16, tag="selT")
                nc.sync.dma_start(selT_t[:], din["selT"][:, b * EB:(b + 1) * EB])
                for gi, (gt0, gnt) in enumerate(groups):
                    i0 = b * EB + gt0 * 128
                    ni = gnt * 128
                    if (b, gi) in gq_store:
                        gq_t = gq_store.pop((b, gi))
                    else:
                        emit_gather(b, gi)
                        gq_t = gq_store.pop((b, gi))
                    # chunks of <=512 within the group
                    co = 0
                    while co < ni:
                        cw = min(512, ni - co)
                        goff = i0 + co            # global edge-slot offset
                        lo = gt0 * 128 + co       # offset within block
                        ps = ps_pre.tile([128, 512], F32, tag="pre")
                        nc.tensor.matmul(ps[:, :cw], Wf[:],
                                         h[:, goff:goff + cw],
                                         start=True, stop=False)
                        nc.tensor.matmul(ps[:, :cw], pqtab[:, b, 0:128],
                                         selT_t[:, lo:lo + cw],
                                         start=False, stop=True)
                        pre16 = hring.tile([128, 512], F16, tag="h")
                        nc.vector.tensor_tensor(pre16[:, :cw], ps[:, :cw],
                                                gq_t[:, 0, co:co + cw],
                                                op=OP.add)
                        nc.vector.tensor_scalar(h[:, goff:goff + cw],
                                                pre16[:, :cw], b0p[:],
                                                0.0, op0=OP.add, op1=OP.max)
                        if s == S - 1:
                            # decoder fused into the last step's chunk loop
                            # (dec layer-0 absorbed le = W1(2).T h + b1)
                            dps = ps_let.tile([128, 512], F32, tag="let")
                            nc.tensor.matmul(dps[:, :cw], W["Wdecf"][:],
                                             h[:, goff:goff + cw],
                                             start=True, stop=True)
                            hd = hring.tile([128, 512], F16, tag="h")
                            nc.scalar.activation(hd[:, :cw], dps[:, :cw],
                                                 AF.Relu, bias=W["db0p"][:])
                            d_ps = ps_m.tile([128, 4], F32, tag="m")
                            for u in range(cw // 128):
                                nc.tensor.matmul(
                                    d_ps[:, u:u + 1],
                                    hd[:, 128 * u:128 * (u + 1)],
                                    W["dec_W1h"][:], start=True, stop=True,
                                    skip_group_check=True)
                            nc.vector.tensor_scalar_add(
                                dec_em[:, goff // 128:goff // 128 + cw // 128],
                                d_ps[:, :cw // 128], W["dec_b1"][:])
                        if do_agg:
                            let_ps = ps_let.tile([128, 512], F32, tag="let")
                            for u in range(cw // 128):
                                nc.tensor.matmul(
                                    let_ps[:, 128 * u:128 * (u + 1)],
                                    h[:, goff + 128 * u:goff + 128 * (u + 1)],
                                    Wscat[:], start=True, stop=True,
                                    skip_group_check=True)
                            let_sb = letring.tile([128, 512], F16, tag="let")
                            nc.scalar.activation(let_sb[:, :cw], let_ps[:, :cw],
                                                 AF.Copy)
                            for u in range(cw // 128):
                                tt = (lo // 128) + u
                                nc.tensor.matmul(
                                    g_ps[:], sel_t[:, tt, :],
                                    let_sb[:, 128 * u:128 * (u + 1)],
                                    start=(tt == 0), stop=(tt == Tb - 1),
                                    skip_group_check=True)
                        co += cw
                if do_agg:
                    # finish this block's aggregation, then immediately run
                    # its node MLP + next-step P/Q so the AllGather halves
                    # launch mid-step instead of serializing at the step end.
                    sg = sgring.tile([128, 128], F32, tag="sg")
                    nc.vector.tensor_scalar_mul(sg[:], g_ps[:], invc[:, b:b + 1])
                    ps_t = ps_m.tile([128, 128], F32, tag="m")
                    nc.tensor.transpose(ps_t[:], sg[:], W["ident"][:])
                    nc.scalar.activation(agg[:, 128 * b:128 * (b + 1)], ps_t[:],
                                         AF.Copy)
                    o = 128 * b
                    p_ps = ps_m.tile([128, 128], F32, tag="m")
                    nc.tensor.matmul(p_ps[:], W[f"nW0a_{s}"][:],
                                     ln[:, o:o + 128], start=True, stop=False)
                    nc.tensor.matmul(p_ps[:], W[f"nW0bh_{s}"][:],
                                     agg[:, o:o + 128], start=False, stop=True)
                    hn = hring.tile([128, 128], F16, tag="h")
                    nc.scalar.activation(hn[:], p_ps[:], AF.Relu,
                                         bias=W[f"nb0p_{s}"][:])
                    l_ps = ps_m.tile([128, 128], F32, tag="m")
                    nc.tensor.matmul(l_ps[:], W[f"nW1h_{s}"][:], hn[:],
                                     start=True, stop=True)
                    nc.scalar.activation(ln[:, o:o + 128], l_ps[:],
                                         AF.Identity, bias=W[f"nb1_{s}"][:])
                    pq_block(s + 1, b)
                    for hf, (b0, b1) in enumerate(HSPLIT):
                        if b == b1 - 1:
                            qshare(s + 1, hf)

        # ---- final combine (in ea_em): out = dm*0.5*sqrt(ea) + dmc*dec ----
        nc.scalar.sqrt(ea_em[:], ea_em[:])
        nc.vector.scalar_tensor_tensor(ea_em[:], dm_em[:], 0.5, ea_em[:],
                                       op0=OP.mult, op1=OP.mult)
        nc.vector.tensor_tensor(dec_em[:], dmc_em[:], dec_em[:], op=OP.mult)
        nc.vector.tensor_tensor(ea_em[:], ea_em[:], dec_em[:], op=OP.add)
        nc.sync.dma_start(out_em[:], ea_em[:])

    nc.compile()


# ----------------------------------------------------------------------------
# Entry point
# ----------------------------------------------------------------------------

def _get_program(Tb, w_shapes):
    key = (Tb, os.environ.get("KB_GSRC_HBM"), os.environ.get("KB_SP"),
           os.environ.get("KB_NQ"))
    if key not in _CACHE:
        import time
        t0 = time.time()
        nc = bacc.Bacc("TRN2", target_bir_lowering=False, debug=False,
                       num_devices=NCORES)
        _build(nc, Tb, w_shapes, a_ok)
        if os.environ.get("KERNEL_VERBOSE"):
            print(f"[kernel] build+schedule+compile: {time.time()-t0:.1f}s",
                  flush=True)
        _CACHE[key] = nc
    return _CACHE[key]


def kernel(**inputs):
    per_core, Tb, a_ok = _prep(inputs["x"], inputs["edge_attr"], inputs["edge_index"])
    w = _weights_inputs(inputs)
    w_shapes = [(k, v.shape, v.dtype.type) for k, v in w.items()]
    nc = _get_program(Tb, w_shapes, a_ok)

    in_maps = []
    for k in range(NCORES):
        m = dict(w)
        pc = per_core[k]
        for key in ("ea_r", "ea_em", "dm_em", "dmc_em", "gq_idx",
                    "sel", "selT", "x_r", "cnt_nm"):
            m[key] = pc[key]
        in_maps.append(m)

    trace = bool(int(os.environ.get("KERNEL_TRACE", "0")))
    import time as _time
    _t0 = _time.time()
    res = run_bass_kernel_spmd(
        nc, in_maps, core_ids=list(range(NCORES)), trace=trace,
        tmpdir=os.environ.get("KERNEL_TRACE_DIR") or None)
    if os.environ.get("KERNEL_VERBOSE"):
        print(f"[kernel] exec phase: {_time.time()-_t0:.1f}s", flush=True)
    if trace:
        print(f"HW exec time: {res.exec_time_ns} ns")
        if res.instructions_and_trace:
            print("trace:", res.instructions_and_trace[1])

    out = np.zeros((E, 1), dtype=np.float32)
    ET = (BLOCKS * Tb * 128) // 128
    for k in range(NCORES):
        o = res.results[k]["out_em"]           # [128, ET]
        flat = o.T.reshape(-1)                 # slot order
        orig = per_core[k]["orig"]
        valid = orig >= 0
        out[orig[valid], 0] = flat[valid]
    return out


# revision 72
# speedup vs baseline: 1.4007x; 1.0213x over previous
"""Trainium2 Bass kernel for NeuralPCG GNN message passing (8 NeuronCores).

Strategy: destination-sharded edges (core k owns all edges whose dest node is
in its 2500-node range), feature-major fp16 matmuls.

Per message-passing step, for each edge e=(r,c):
    pre_h = Wf.T @ h_prev  +  P[r]  +  Q[c]          (PSUM accumulate)
    h     = relu(pre_h + b0')                        (one DVE op)
where Wf = W1(s-1) @ eW0c(s) is host-folded (le never materialized),
P[r] comes from a one-hot sel matmul against the local P table, and
Q[c] comes from an SBUF-source dma_gather out of a packed Q table that
is AllGathered across the 8 cores each step.

The mean aggregation scatters le' = W1.T @ h edge-major via per-tile
matmuls (lhsT=h_tile, rhs=W1) then one-hot sel matmuls accumulating
per-block segment sums in PSUM.
"""
import os
import numpy as np
import ml_dtypes
from contextlib import ExitStack

import concourse.bass as bass
import concourse.tile as tile
from concourse import bacc, mybir
from concourse.bass_utils import run_bass_kernel_spmd

N = 20000
E = 320000
L = 128
S = 3
NCORES = 8
NB = 2500            # nodes per core
BLOCKS = 20          # 128-node blocks per core
NPAD = BLOCKS * 128  # 2560
GT = int(os.environ.get("KB_GT", "6"))  # tiles per gather group
HS1 = int(os.environ.get("KB_HS1", "18"))  # blocks in AllGather half A

F32 = mybir.dt.float32
F16 = mybir.dt.float16
I16 = mybir.dt.int16
AF = mybir.ActivationFunctionType
OP = mybir.AluOpType

NP16 = np.float16

_CACHE = {}


# ----------------------------------------------------------------------------
# Host-side graph preprocessing (index manipulation + sharding only)
# ----------------------------------------------------------------------------

def _wrap_idxs(idx):
    """[n] int -> [128, n//16] int16 wrapped layout for dma_gather."""
    n = idx.shape[0]
    assert n % 16 == 0
    block = idx.reshape(n // 16, 16).T.astype(np.int16)
    return np.tile(block, (8, 1))


def _prep(x, edge_attr, edge_index):
    row = np.asarray(edge_index[0]).astype(np.int64)
    col = np.asarray(edge_index[1]).astype(np.int64)
    ea = np.asarray(edge_attr).reshape(-1).astype(np.float32)
    xf = np.asarray(x).reshape(-1).astype(np.float32)

    cnt_full = np.bincount(row, minlength=N).astype(np.float32)
    core_of = row // NB

    # Bin-pack each core's nodes into its 20 slot-blocks so per-block edge
    # counts are balanced (greedy LPT): the padded tile count Tb is set by
    # the WORST block, and consecutive-id blocks leave ~13% padding.
    posg = np.empty(N, dtype=np.int64)   # node -> within-core slot position
    for k in range(NCORES):
        deg = cnt_full[k * NB:(k + 1) * NB]
        order = np.argsort(-deg, kind="stable")
        bl = np.zeros(BLOCKS)
        bn = np.zeros(BLOCKS, dtype=np.int64)
        pos = np.empty(NB, dtype=np.int64)
        for i in order:
            cand = np.nonzero(bn < 128)[0]
            b = cand[np.argmin(bl[cand])]
            pos[i] = b * 128 + bn[b]
            bn[b] += 1
            bl[b] += deg[i] - 1  # self-loop edges are not gathered
        posg[k * NB:(k + 1) * NB] = pos
    blk_of = posg // 128
    slot_of = posg % 128

    # one self-loop edge per node goes to a dedicated per-block tile (its
    # Q[c]=Q[r] comes from the local table, no gather); everything else is
    # the gathered "regular" region of TbR tiles per block.
    first_self = np.zeros(E, dtype=bool)
    diag = row == col
    seen = np.zeros(N, dtype=bool)
    for e in np.nonzero(diag)[0]:
        n = row[e]
        if not seen[n]:
            seen[n] = True
            first_self[e] = True

    cores = []
    ebc_max = 0
    a_ok = True
    for k in range(NCORES):
        keids = np.nonzero(core_of == k)[0]
        selfe = keids[first_self[keids]]
        eids = keids[~first_self[keids]]
        blk_e = blk_of[row[eids]]
        # within each block put half-A-referencing edges (col in any core's
        # blocks [0, HS1)) first, so each block's first gather group only
        # needs the big (early) AllGather half
        isA = blk_of[col[eids]] < HS1
        order = np.lexsort((~isA, blk_e))
        eids = eids[order]
        blk = blk_of[row[eids]]
        bc = np.bincount(blk, minlength=BLOCKS)
        bcA = np.bincount(blk[blk_of[col[eids]] < HS1], minlength=BLOCKS)
        if np.any(bcA < np.minimum(bc, GT * 128)):
            a_ok = False
        ebc_max = max(ebc_max, int(bc.max()))
        cores.append((eids, blk, bc, selfe))

    Tb = max(6, (ebc_max + 127) // 128)   # regular (gathered) tiles per block
    EB = (Tb + 1) * 128                   # + 1 self-loop tile per block
    Epad = BLOCKS * EB
    ET = Epad // 128  # number of 128-edge tiles
    NCH = Epad // 512 if Epad % 512 == 0 else -1
    assert Epad % 512 == 0

    # Gather-table row numbering. Ranks are laid out so each AllGather half
    # is a contiguous slice of qtab: first all cores' blocks [0, HS1), then
    # all cores' blocks [HS1, BLOCKS).
    def trow(n):
        j = n // NB
        b, sl = blk_of[n], slot_of[n]
        g = np.where(b < HS1, j * HS1 + b,
                     NCORES * HS1 + j * (BLOCKS - HS1) + (b - HS1))
        return g * 128 + sl

    per_core = []
    for k in range(NCORES):
        eids, blk, bc, selfe = cores[k]
        starts = np.zeros(BLOCKS, dtype=np.int64)
        np.cumsum(bc[:-1], out=starts[1:])
        pos_in_blk = np.arange(len(eids)) - starts[blk]
        dst_reg = blk * EB + pos_in_blk
        # self-loop edges: tile Tb of their block, position = slot
        dst_self = blk_of[row[selfe]] * EB + Tb * 128 + slot_of[row[selfe]]
        eids = np.concatenate([eids, selfe])
        dst = np.concatenate([dst_reg, dst_self])
        r = row[eids]
        c = col[eids]

        gq = np.zeros(Epad, dtype=np.int64)
        slot = np.full(Epad, -1, dtype=np.int64)
        ea_s = np.ones(Epad, dtype=np.float32)
        dm = np.zeros(Epad, dtype=np.float32)
        orig = np.full(Epad, -1, dtype=np.int64)

        gq[dst] = trow(c)
        slot[dst] = slot_of[r]
        ea_s[dst] = ea[eids]
        dm[dst] = (r == c).astype(np.float32)
        orig[dst] = eids

        # one-hot (edge-major, for the scatter) and slot-major (for P bcast)
        sel = np.zeros((Epad, 128), dtype=NP16)
        valid = slot >= 0
        vs = np.nonzero(valid)[0]
        sel[vs, slot[valid]] = 1.0
        selT = np.zeros((128, Epad), dtype=NP16)
        selT[slot[valid], vs] = 1.0

        pos = posg[k * NB:(k + 1) * NB]
        own = cnt_full[k * NB:(k + 1) * NB]
        tmp = np.zeros(BLOCKS * 128, dtype=np.float32)
        tmp[pos] = own
        cnt_nm = tmp.reshape(BLOCKS, 128).T.copy()

        x_own = np.zeros(NPAD, dtype=np.float32)
        x_own[pos] = xf[k * NB:(k + 1) * NB]

        em = lambda a: a.reshape(ET, 128).T.copy()  # edge-slot-major [128, ET]
        per_core.append(dict(
            ea_r=ea_s.reshape(NCH, 512),
            ea_em=em(ea_s),
            dm_em=em(dm).astype(NP16),
            dmc_em=em((1.0 - dm) * (slot >= 0)).astype(NP16),
            gq_idx=_wrap_idxs(gq),
            sel=sel,
            selT=selT,
            x_r=x_own.reshape(NPAD // 512, 512).astype(NP16),
            cnt_nm=cnt_nm,
            orig=orig,
        ))
    return per_core, Tb, a_ok


def _weights_inputs(inp):
    """Build the weight/bias input arrays (shared across cores).

    Host-side weight folding:
      Wf[s]   = W1(s-1) @ eW0c(s)      (le never materialized on device)
      b0p[s]  = eb0(s) + eW0c(s).T @ b1(s-1)
      nb0p[s] = nb0(s) + nW0b(s).T @ eb1(s)   (agg carries no b1 term)
      Wdecf   = eW1(2) @ dec_W0 ;  db0p = dec_b0 + dec_W0.T @ eb1(2)
    """
    g = lambda name: np.asarray(inp[name], dtype=np.float32)
    w = {}
    col = lambda a: a.reshape(128, 1).astype(np.float32)

    w["encn_W0h"] = g("encn_W0").reshape(1, L).astype(NP16)
    w["encn_b0"] = col(g("encn_b0"))
    w["encn_W1h"] = g("encn_W1").astype(NP16)
    w["encn_b1"] = col(g("encn_b1"))
    w["ence_W0"] = g("ence_W0").reshape(1, L)
    w["ence_b0"] = col(g("ence_b0"))
    eW0, eb0, eW1, eb1 = g("eW0"), g("eb0"), g("eW1"), g("eb1")
    nW0, nb0, nW1, nb1 = g("nW0"), g("nb0"), g("nW1"), g("nb1")
    ence_W1, ence_b1 = g("ence_W1"), g("ence_b1")
    for s in range(S):
        eW0c = eW0[s, 2 * L:, :]                       # [L, L]
        W1prev = ence_W1 if s == 0 else eW1[s - 1]
        b1prev = ence_b1 if s == 0 else eb1[s - 1]
        w[f"Wf_{s}"] = (W1prev @ eW0c).astype(NP16)
        w[f"b0p_{s}"] = col(eb0[s] + eW0c.T @ b1prev)
        w[f"eW0ab_{s}"] = np.concatenate(
            [eW0[s, :L, :], eW0[s, L:2 * L, :]], axis=1).astype(NP16)
        w[f"eW1h_{s}"] = eW1[s].astype(NP16)
    for s in range(S - 1):
        w[f"nW0a_{s}"] = nW0[s, :L, :].astype(NP16)
        w[f"nW0bh_{s}"] = nW0[s, L:, :].astype(NP16)
        w[f"nb0p_{s}"] = col(nb0[s] + nW0[s, L:, :].T @ eb1[s])
        w[f"nW1h_{s}"] = nW1[s].astype(NP16)
        w[f"nb1_{s}"] = col(nb1[s])
    dec_W0, dec_b0 = g("dec_W0"), g("dec_b0")
    w["Wdecf"] = (eW1[2] @ dec_W0).astype(NP16)
    w["db0p"] = col(dec_b0 + dec_W0.T @ eb1[2])
    w["dec_W1h"] = g("dec_W1").reshape(L, 1).astype(NP16)
    w["dec_b1"] = np.full((128, 1), float(np.asarray(inp["dec_b1"]).reshape(-1)[0]),
                          dtype=np.float32)
    w["ident"] = np.eye(128, dtype=np.float32)
    return w


# ----------------------------------------------------------------------------
# Device program
# ----------------------------------------------------------------------------

def _build(nc, Tb, w_shapes, a_ok):
    kb_no_gather = bool(int(os.environ.get("KB_NO_GATHER", "0")))
    kb_no_cc = bool(int(os.environ.get("KB_NO_CC", "0")))
    kb_sp = bool(int(os.environ.get("KB_SP", "1")))
    kb_hbm = bool(int(os.environ.get("KB_GSRC_HBM", "0")))
    EB = Tb * 128
    Epad = BLOCKS * EB
    ET = Epad // 128
    NCH = Epad // 512
    # gather groups (in tiles) per block
    groups = []
    t0 = 0
    while t0 < Tb:
        groups.append((t0, min(GT, Tb - t0)))
        t0 += GT

    din = {}

    def inp(name, shape, dtype):
        din[name] = nc.dram_tensor(name, shape, dtype, kind="ExternalInput")
        return din[name]

    inp("ea_r", [NCH, 512], F32)
    inp("ea_em", [128, ET], F32)
    inp("dm_em", [128, ET], F16)
    inp("dmc_em", [128, ET], F16)
    inp("gq_idx", [128, Epad // 16], I16)
    inp("sel", [Epad, 128], F16)
    inp("selT", [128, Epad], F16)
    inp("x_r", [NPAD // 512, 512], F16)
    inp("cnt_nm", [128, BLOCKS], F32)
    for name, arr_shape, np_dtype in w_shapes:
        inp(name, list(arr_shape), F16 if np_dtype == NP16 else F32)

    out_em = nc.dram_tensor("out_em", [128, ET], F32, kind="ExternalOutput")

    with tile.TileContext(nc) as tc, ExitStack() as ctx:
        P = lambda name, bufs, **kw: ctx.enter_context(
            tc.tile_pool(name=name, bufs=bufs, **kw))
        const = P("const", 1)
        big = P("big", 1)
        dram = P("dram", 1, space="DRAM")
        selp = P("selp", 3)      # edge-major one-hot per block
        seltp = P("seltp", 3)    # slot-major one-hot per block
        gath = P("gath", 3)      # gathered Q tiles
        hring = P("hring", 4)
        letring = P("letring", 3)
        sgring = P("sgring", 2)
        rows16 = P("rows16", 2)
        ps_pre = P("ps_pre", 2, space="PSUM")
        ps_let = P("ps_let", 2, space="PSUM")
        ps_g = P("ps_g", 2, space="PSUM")
        ps_m = P("ps_m", 2, space="PSUM")

        # ---- load constants / weights ----
        # node-encoder input rows first: they head the Sync DMA queue so the
        # node-enc -> pq -> AllGather(0) critical path starts immediately
        xrows = const.tile([1, NPAD], F16, name="xrows")
        for j in range(NPAD // 512):
            nc.sync.dma_start(xrows[:, 512 * j:512 * (j + 1)],
                              din["x_r"][j:j + 1, :])
        W = {}
        for name, arr_shape, np_dtype in w_shapes:
            t = const.tile(list(arr_shape), F16 if np_dtype == NP16 else F32,
                           name=f"w_{name}")
            nc.sync.dma_start(t[:], din[name][:])
            W[name] = t
        cnt = const.tile([128, BLOCKS], F32, name="cnt_s")
        nc.sync.dma_start(cnt[:], din["cnt_nm"][:])

        invc = const.tile([128, BLOCKS], F32, name="invc")
        nc.vector.tensor_scalar_max(invc[:], cnt[:], 1.0)
        nc.vector.reciprocal(invc[:], invc[:])

        # ---- persistent big tensors ----
        h = big.tile([128, Epad], F16, name="h")          # edge hidden latent
        ln = big.tile([128, NPAD], F16, name="ln")        # own-node latent
        agg = big.tile([128, NPAD], F16, name="agg")      # aggregated messages
        pqtab = big.tile([128, BLOCKS, 256], F16, name="pqtab")  # local P|Q
        qtab = big.tile([128, NCORES * BLOCKS, 128], F16, name="qtab")
        dec_em = big.tile([128, ET], F32, name="dec_em")

        # DRAM staging for the split Q-table AllGathers (partition-major:
        # [128 parts, nb ranks, 128 feats] per core per part). Uneven 16/4
        # split so the step-end tail collective is small; both sizes are
        # multiples of UNIT=4 so the qtab pack stays a single strided DMA.
        UNIT = 4
        HSPLIT = [(0, HS1), (HS1, BLOCKS)]
        # step 0 ships the whole table in one collective (it is the first cc,
        # so it also absorbs the one-time comm-init barrier); steps 1-2 use
        # the HS1 split so the AllGathers fire mid-step.
        qown = [None] + [
            [dram.tile([128, (b1 - b0) * 128], F16, name=f"qown_{s}_{hf}")
             for hf, (b0, b1) in enumerate(HSPLIT)] for s in range(1, S)]
        qfull = [None] + [
            [dram.tile([NCORES, 128, (b1 - b0) * 128], F16,
                       name=f"qfull_{s}_{hf}", addr_space="Shared")
             for hf, (b0, b1) in enumerate(HSPLIT)] for s in range(1, S)]
        qown0 = dram.tile([128, BLOCKS * 128], F16, name="qown0")
        qfull0 = dram.tile([NCORES, 128, BLOCKS * 128], F16,
                           name="qfull0", addr_space="Shared")

        def half_of(b):
            for hf, (b0, b1) in enumerate(HSPLIT):
                if b0 <= b < b1:
                    return hf, b - b0
            raise AssertionError

        def mlp_rows(src_row, nrows, hidden_W0, b0, W1h, b1, dst):
            """dst[:, 512j:...] = W1h.T @ relu(W0 (x) row_j + b0) + b1."""
            for j in range(nrows):
                ps = ps_m.tile([128, 512], F32, tag="m")
                nc.tensor.matmul(ps[:], hidden_W0[:],
                                 src_row[:, 512 * j:512 * (j + 1)],
                                 start=True, stop=True)
                h0 = hring.tile([128, 512], F16, tag="h")
                nc.scalar.activation(h0[:], ps[:], AF.Relu, bias=b0[:])
                ps2 = ps_pre.tile([128, 512], F32, tag="pre")
                nc.tensor.matmul(ps2[:], W1h[:], h0[:], start=True, stop=True)
                nc.scalar.activation(dst[:, 512 * j:512 * (j + 1)], ps2[:],
                                     AF.Identity, bias=b1[:])

        def pq_block(s, b):
            """pqtab[:, b] = ln_b @ [eW0a(s)|eW0b(s)]; Q half DMAs to qown."""
            ps = ps_m.tile([128, 256], F32, tag="m")
            nc.tensor.matmul(ps[:], ln[:, 128 * b:128 * (b + 1)],
                             W[f"eW0ab_{s}"][:], start=True, stop=True)
            nc.scalar.activation(pqtab[:, b, :], ps[:], AF.Copy)
            if s == 0:
                nc.sync.dma_start(qown0[:, 128 * b:128 * (b + 1)],
                                  pqtab[:, b, 128:256])
            else:
                hf, rb = half_of(b)
                nc.sync.dma_start(qown[s][hf][:, 128 * rb:128 * (rb + 1)],
                                  pqtab[:, b, 128:256])

        def _cc(in_tile, out_tile):
            if kb_no_cc:
                nc.sync.dma_start(out_tile[0, :, :], in_tile[:])
            else:
                nc.gpsimd.collective_compute(
                    "AllGather", OP.bypass,
                    replica_groups=[list(range(NCORES))],
                    ins=[in_tile.opt()],
                    outs=[out_tile.opt()],
                )

        def qshare(s, hf):
            """AllGather one half of the Q table (qtab load happens later)."""
            _cc(qown[s][hf], qfull[s][hf])

        def qtab_load(s, hf):
            """Pack one qfull half into qtab. Issued on the GpSimd engine so
            its FIFO orders these writes after every step-(s-1) gather (Tile
            does not track dma_gather's read of qtab). Ranks are numbered so
            each half is a contiguous qtab slice -> one strided DMA."""
            b0, b1 = HSPLIT[hf]
            nb = b1 - b0
            off = NCORES * sum(e - a for a, e in HSPLIT[:hf])
            src = (qfull0[:, :, 128 * b0:128 * b1] if s == 0
                   else qfull[s][hf][:])
            nc.gpsimd.dma_start(
                qtab[:, off:off + NCORES * nb, :].rearrange(
                    "p (j rk) f -> p j (rk f)", j=NCORES),
                src.rearrange("j p x -> p j x"))

        # ---- encoders / initial tables ----
        # node encoder first so pq_pass(0) + AllGather overlap the edge encoder
        mlp_rows(xrows, NPAD // 512, W["encn_W0h"], W["encn_b0"],
                 W["encn_W1h"], W["encn_b1"], ln)
        for b in range(BLOCKS):
            pq_block(0, b)
        if kb_no_cc:
            nc.sync.dma_start(qfull0[0, :, :], qown0[:])
        else:
            nc.gpsimd.collective_compute(
                "AllGather", OP.bypass,
                replica_groups=[list(range(NCORES))],
                ins=[qown0.opt()],
                outs=[qfull0.opt()],
            )
        # bulk constants, traced after the step-0 collective so they don't
        # delay the node encoder -> pq -> AllGather critical path
        # gq_idx is consumed by dma_gather, whose input reads Tile does not
        # track; issue its load on the GpSimd engine so the SWDGE ring
        # orders it ahead of every gather.
        gq_idx = const.tile([128, Epad // 16], I16, name="gq_idx_s")
        nc.gpsimd.dma_start(gq_idx[:], din["gq_idx"][:])
        ea_em = const.tile([128, ET], F32, name="ea_em_s")
        nc.sync.dma_start(ea_em[:], din["ea_em"][:])
        dm_em = const.tile([128, ET], F16, name="dm_em_s")
        nc.sync.dma_start(dm_em[:], din["dm_em"][:])
        dmc_em = const.tile([128, ET], F16, name="dmc_em_s")
        nc.sync.dma_start(dmc_em[:], din["dmc_em"][:])
        # edge encoder: FIRST layer only -> h ; second layer folds into Wf_0.
        for j in range(NCH):
            r = rows16.tile([1, 512], F16, tag="rowin16")
            nc.sync.dma_start(r[:], din["ea_r"][j:j + 1, :])
            ps = ps_m.tile([128, 512], F32, tag="m")
            nc.tensor.matmul(ps[:], W["ence_W0h"][:], r[:], start=True, stop=True)
            nc.scalar.activation(h[:, 512 * j:512 * (j + 1)], ps[:],
                                 AF.Relu, bias=W["ence_b0"][:])

        for s in range(S):
            do_agg = s < S - 1
            Wf = W[f"Wf_{s}"]
            b0p = W[f"b0p_{s}"]
            Wscat = W[f"eW1h_{s}"]

            gq_store = {}

            def emit_gather(b, gi):
                gt0, gnt = groups[gi]
                i0 = b * EB + gt0 * 128
                ni = gnt * 128
                gq_t = gath.tile([128, 1, GT * 128], F16, tag="gq")
                if kb_no_gather:
                    nc.vector.memset(gq_t[:], 0.0)
                else:
                    nc.gpsimd.dma_gather(
                        gq_t[:, :, :ni], qtab[:],
                        gq_idx[:, i0 // 16:(i0 + ni) // 16],
                        num_idxs=ni, num_idxs_reg=ni,
                        elem_size=128, transpose=True,
                        single_packet=kb_sp,
                        sbuf_tokens_per_rank=128,
                        sbuf_free_dim_per_rank=256,
                        sbuf_byte_offset=0)
                gq_store[(b, gi)] = gq_t

            qtab_load(s, 0)
            if a_ok:
                # each block's first group only references half-A rows, so
                # two of those gathers can run before the half-B table load
                # (covering the tail AllGather + load latency)
                emit_gather(0, 0)
                emit_gather(1, 0)
                emit_gather(2, 0)
            qtab_load(s, 1)
            for b in range(BLOCKS):
                if do_agg:
                    g_ps = ps_g.tile([128, 128], F32, tag="g")
                    sel_t = selp.tile([128, Tb, 128], F16, tag="sel")
                    nc.sync.dma_start(
                        sel_t[:],
                        din["sel"][b * EB:(b + 1) * EB, :].rearrange(
                            "(t p) s -> p t s", p=128))
                selT_t = seltp.tile([128, EB], F16, tag="selT")
                nc.sync.dma_start(selT_t[:], din["selT"][:, b * EB:(b + 1) * EB])
                for gi, (gt0, gnt) in enumerate(groups):
                    i0 = b * EB + gt0 * 128
                    ni = gnt * 128
                    if (b, gi) in gq_store:
                        gq_t = gq_store.pop((b, gi))
                    else:
                        emit_gather(b, gi)
                        gq_t = gq_store.pop((b, gi))
                    # chunks of <=512 within the group
                    co = 0
                    while co < ni:
                        cw = min(512, ni - co)
                        goff = i0 + co            # global edge-slot offset
                        lo = gt0 * 128 + co       # offset within block
                        ps = ps_pre.tile([128, 512], F32, tag="pre")
                        nc.tensor.matmul(ps[:, :cw], Wf[:],
                                         h[:, goff:goff + cw],
                                         start=True, stop=False)
                        nc.tensor.matmul(ps[:, :cw], pqtab[:, b, 0:128],
                                         selT_t[:, lo:lo + cw],
                                         start=False, stop=True)
                        pre16 = hring.tile([128, 512], F16, tag="h")
                        nc.vector.tensor_tensor(pre16[:, :cw], ps[:, :cw],
                                                gq_t[:, 0, co:co + cw],
                                                op=OP.add)
                        nc.vector.tensor_scalar(h[:, goff:goff + cw],
                                                pre16[:, :cw], b0p[:],
                                                0.0, op0=OP.add, op1=OP.max)
                        if s == S - 1:
                            # decoder fused into the last step's chunk loop
                            # (dec layer-0 absorbed le = W1(2).T h + b1)
                            dps = ps_let.tile([128, 512], F32, tag="let")
                            nc.tensor.matmul(dps[:, :cw], W["Wdecf"][:],
                                             h[:, goff:goff + cw],
                                             start=True, stop=True)
                            hd = hring.tile([128, 512], F16, tag="h")
                            nc.scalar.activation(hd[:, :cw], dps[:, :cw],
                                                 AF.Relu, bias=W["db0p"][:])
                            d_ps = ps_m.tile([128, 4], F32, tag="m")
                            for u in range(cw // 128):
                                nc.tensor.matmul(
                                    d_ps[:, u:u + 1],
                                    hd[:, 128 * u:128 * (u + 1)],
                                    W["dec_W1h"][:], start=True, stop=True,
                                    skip_group_check=True)
                            nc.vector.tensor_scalar_add(
                                dec_em[:, goff // 128:goff // 128 + cw // 128],
                                d_ps[:, :cw // 128], W["dec_b1"][:])
                        if do_agg:
                            let_ps = ps_let.tile([128, 512], F32, tag="let")
                            for u in range(cw // 128):
                                nc.tensor.matmul(
                                    let_ps[:, 128 * u:128 * (u + 1)],
                                    h[:, goff + 128 * u:goff + 128 * (u + 1)],
                                    Wscat[:], start=True, stop=True,
                                    skip_group_check=True)
                            let_sb = letring.tile([128, 512], F16, tag="let")
                            nc.scalar.activation(let_sb[:, :cw], let_ps[:, :cw],
                                                 AF.Copy)
                            for u in range(cw // 128):
                                tt = (lo // 128) + u
                                nc.tensor.matmul(
                                    g_ps[:], sel_t[:, tt, :],
                                    let_sb[:, 128 * u:128 * (u + 1)],
                                    start=(tt == 0), stop=(tt == Tb - 1),
                                    skip_group_check=True)
                        co += cw
                if do_agg:
                    # finish this block's aggregation, then immediately run
                    # its node MLP + next-step P/Q so the AllGather halves
                    # launch mid-step instead of serializing at the step end.
                    sg = sgring.tile([128, 128], F32, tag="sg")
                    nc.vector.tensor_scalar_mul(sg[:], g_ps[:], invc[:, b:b + 1])
                    ps_t = ps_m.tile([128, 128], F32, tag="m")
                    nc.tensor.transpose(ps_t[:], sg[:], W["ident"][:])
                    nc.scalar.activation(agg[:, 128 * b:128 * (b + 1)], ps_t[:],
                                         AF.Copy)
                    o = 128 * b
                    p_ps = ps_m.tile([128, 128], F32, tag="m")
                    nc.tensor.matmul(p_ps[:], W[f"nW0a_{s}"][:],
                                     ln[:, o:o + 128], start=True, stop=False)
                    nc.tensor.matmul(p_ps[:], W[f"nW0bh_{s}"][:],
                                     agg[:, o:o + 128], start=False, stop=True)
                    hn = hring.tile([128, 128], F16, tag="h")
                    nc.scalar.activation(hn[:], p_ps[:], AF.Relu,
                                         bias=W[f"nb0p_{s}"][:])
                    l_ps = ps_m.tile([128, 128], F32, tag="m")
                    nc.tensor.matmul(l_ps[:], W[f"nW1h_{s}"][:], hn[:],
                                     start=True, stop=True)
                    nc.scalar.activation(ln[:, o:o + 128], l_ps[:],
                                         AF.Identity, bias=W[f"nb1_{s}"][:])
                    pq_block(s + 1, b)
                    for hf, (b0, b1) in enumerate(HSPLIT):
                        if b == b1 - 1:
                            qshare(s + 1, hf)

        # ---- final combine (in ea_em): out = dm*0.5*sqrt(ea) + dmc*dec ----
        nc.scalar.sqrt(ea_em[:], ea_em[:])
        nc.vector.scalar_tensor_tensor(ea_em[:], dm_em[:], 0.5, ea_em[:],
                                       op0=OP.mult, op1=OP.mult)
        nc.vector.tensor_tensor(dec_em[:], dmc_em[:], dec_em[:], op=OP.mult)
        nc.vector.tensor_tensor(ea_em[:], ea_em[:], dec_em[:], op=OP.add)
        nc.sync.dma_start(out_em[:], ea_em[:])

    nc.compile()


# ----------------------------------------------------------------------------
# Entry point
# ----------------------------------------------------------------------------

def _get_program(Tb, w_shapes):
    key = (Tb, os.environ.get("KB_GSRC_HBM"), os.environ.get("KB_SP"),
           os.environ.get("KB_NQ"))
    if key not in _CACHE:
        import time
        t0 = time.time()
        nc = bacc.Bacc("TRN2", target_bir_lowering=False, debug=False,
                       num_devices=NCORES)
        _build(nc, Tb, w_shapes, a_ok)
        if os.environ.get("KERNEL_VERBOSE"):
            print(f"[kernel] build+schedule+compile: {time.time()-t0:.1f}s",
                  flush=True)
        _CACHE[key] = nc
    return _CACHE[key]


def kernel(**inputs):
    per_core, Tb, a_ok = _prep(inputs["x"], inputs["edge_attr"], inputs["edge_index"])
    w = _weights_inputs(inputs)
    w_shapes = [(k, v.shape, v.dtype.type) for k, v in w.items()]
    nc = _get_program(Tb, w_shapes, a_ok)

    in_maps = []
    for k in range(NCORES):
        m = dict(w)
        pc = per_core[k]
        for key in ("ea_r", "ea_em", "dm_em", "dmc_em", "gq_idx",
                    "sel", "selT", "x_r", "cnt_nm"):
            m[key] = pc[key]
        in_maps.append(m)

    trace = bool(int(os.environ.get("KERNEL_TRACE", "0")))
    import time as _time
    _t0 = _time.time()
    res = run_bass_kernel_spmd(
        nc, in_maps, core_ids=list(range(NCORES)), trace=trace,
        tmpdir=os.environ.get("KERNEL_TRACE_DIR") or None)
    if os.environ.get("KERNEL_VERBOSE"):
        print(f"[kernel] exec phase: {_time.time()-_t0:.1f}s", flush=True)
    if trace:
        print(f"HW exec time: {res.exec_time_ns} ns")
        if res.instructions_and_trace:
            print("trace:", res.instructions_and_trace[1])

    out = np.zeros((E, 1), dtype=np.float32)
    ET = (BLOCKS * Tb * 128) // 128
    for k in range(NCORES):
        o = res.results[k]["out_em"]           # [128, ET]
        flat = o.T.reshape(-1)                 # slot order
        orig = per_core[k]["orig"]
        valid = orig >= 0
        out[orig[valid], 0] = flat[valid]
    return out


# revision 73
# speedup vs baseline: 1.4213x; 1.0147x over previous
"""Trainium2 Bass kernel for NeuralPCG GNN message passing (8 NeuronCores).

Strategy: destination-sharded edges (core k owns all edges whose dest node is
in its 2500-node range), feature-major fp16 matmuls.

Per message-passing step, for each edge e=(r,c):
    pre_h = Wf.T @ h_prev  +  P[r]  +  Q[c]          (PSUM accumulate)
    h     = relu(pre_h + b0')                        (one DVE op)
where Wf = W1(s-1) @ eW0c(s) is host-folded (le never materialized),
P[r] comes from a one-hot sel matmul against the local P table, and
Q[c] comes from an SBUF-source dma_gather out of a packed Q table that
is AllGathered across the 8 cores each step.

The mean aggregation scatters le' = W1.T @ h edge-major via per-tile
matmuls (lhsT=h_tile, rhs=W1) then one-hot sel matmuls accumulating
per-block segment sums in PSUM.
"""
import os
import numpy as np
import ml_dtypes
from contextlib import ExitStack

import concourse.bass as bass
import concourse.tile as tile
from concourse import bacc, mybir
from concourse.bass_utils import run_bass_kernel_spmd

N = 20000
E = 320000
L = 128
S = 3
NCORES = 8
NB = 2500            # nodes per core
BLOCKS = 20          # 128-node blocks per core
NPAD = BLOCKS * 128  # 2560
GT = int(os.environ.get("KB_GT", "6"))  # tiles per gather group
HS1 = int(os.environ.get("KB_HS1", "18"))  # blocks in AllGather half A

F32 = mybir.dt.float32
F16 = mybir.dt.float16
I16 = mybir.dt.int16
AF = mybir.ActivationFunctionType
OP = mybir.AluOpType

NP16 = np.float16

_CACHE = {}


# ----------------------------------------------------------------------------
# Host-side graph preprocessing (index manipulation + sharding only)
# ----------------------------------------------------------------------------

def _wrap_idxs(idx):
    """[n] int -> [128, n//16] int16 wrapped layout for dma_gather."""
    n = idx.shape[0]
    assert n % 16 == 0
    block = idx.reshape(n // 16, 16).T.astype(np.int16)
    return np.tile(block, (8, 1))


def _prep(x, edge_attr, edge_index):
    row = np.asarray(edge_index[0]).astype(np.int64)
    col = np.asarray(edge_index[1]).astype(np.int64)
    ea = np.asarray(edge_attr).reshape(-1).astype(np.float32)
    xf = np.asarray(x).reshape(-1).astype(np.float32)

    cnt_full = np.bincount(row, minlength=N).astype(np.float32)
    core_of = row // NB

    # Bin-pack each core's nodes into its 20 slot-blocks so per-block edge
    # counts are balanced (greedy LPT): the padded tile count Tb is set by
    # the WORST block, and consecutive-id blocks leave ~13% padding.
    posg = np.empty(N, dtype=np.int64)   # node -> within-core slot position
    for k in range(NCORES):
        deg = cnt_full[k * NB:(k + 1) * NB]
        order = np.argsort(-deg, kind="stable")
        bl = np.zeros(BLOCKS)
        bn = np.zeros(BLOCKS, dtype=np.int64)
        pos = np.empty(NB, dtype=np.int64)
        for i in order:
            cand = np.nonzero(bn < 128)[0]
            b = cand[np.argmin(bl[cand])]
            pos[i] = b * 128 + bn[b]
            bn[b] += 1
            bl[b] += deg[i] - 1  # self-loop edges are not gathered
        posg[k * NB:(k + 1) * NB] = pos
    blk_of = posg // 128
    slot_of = posg % 128

    # one self-loop edge per node goes to a dedicated per-block tile (its
    # Q[c]=Q[r] comes from the local table, no gather); everything else is
    # the gathered "regular" region of TbR tiles per block.
    first_self = np.zeros(E, dtype=bool)
    diag = row == col
    seen = np.zeros(N, dtype=bool)
    for e in np.nonzero(diag)[0]:
        n = row[e]
        if not seen[n]:
            seen[n] = True
            first_self[e] = True

    cores = []
    ebc_max = 0
    a_ok = True
    for k in range(NCORES):
        keids = np.nonzero(core_of == k)[0]
        selfe = keids[first_self[keids]]
        eids = keids[~first_self[keids]]
        blk_e = blk_of[row[eids]]
        # within each block put half-A-referencing edges (col in any core's
        # blocks [0, HS1)) first, so each block's first gather group only
        # needs the big (early) AllGather half
        isA = blk_of[col[eids]] < HS1
        order = np.lexsort((~isA, blk_e))
        eids = eids[order]
        blk = blk_of[row[eids]]
        bc = np.bincount(blk, minlength=BLOCKS)
        bcA = np.bincount(blk[blk_of[col[eids]] < HS1], minlength=BLOCKS)
        if np.any(bcA < np.minimum(bc, GT * 128)):
            a_ok = False
        ebc_max = max(ebc_max, int(bc.max()))
        cores.append((eids, blk, bc, selfe))

    Tb = max(6, (ebc_max + 127) // 128)   # regular (gathered) tiles per block
    EB = (Tb + 1) * 128                   # + 1 self-loop tile per block
    Epad = BLOCKS * EB
    ET = Epad // 128  # number of 128-edge tiles
    NCH = Epad // 512 if Epad % 512 == 0 else -1
    assert Epad % 512 == 0

    # Gather-table row numbering. Ranks are laid out so each AllGather half
    # is a contiguous slice of qtab: first all cores' blocks [0, HS1), then
    # all cores' blocks [HS1, BLOCKS).
    def trow(n):
        j = n // NB
        b, sl = blk_of[n], slot_of[n]
        g = np.where(b < HS1, j * HS1 + b,
                     NCORES * HS1 + j * (BLOCKS - HS1) + (b - HS1))
        return g * 128 + sl

    per_core = []
    for k in range(NCORES):
        eids, blk, bc, selfe = cores[k]
        starts = np.zeros(BLOCKS, dtype=np.int64)
        np.cumsum(bc[:-1], out=starts[1:])
        pos_in_blk = np.arange(len(eids)) - starts[blk]
        dst_reg = blk * EB + pos_in_blk
        # self-loop edges: tile Tb of their block, position = slot
        dst_self = blk_of[row[selfe]] * EB + Tb * 128 + slot_of[row[selfe]]
        eids = np.concatenate([eids, selfe])
        dst = np.concatenate([dst_reg, dst_self])
        r = row[eids]
        c = col[eids]

        gq = np.zeros(Epad, dtype=np.int64)
        slot = np.full(Epad, -1, dtype=np.int64)
        ea_s = np.ones(Epad, dtype=np.float32)
        dm = np.zeros(Epad, dtype=np.float32)
        orig = np.full(Epad, -1, dtype=np.int64)

        gq[dst] = trow(c)
        slot[dst] = slot_of[r]
        ea_s[dst] = ea[eids]
        dm[dst] = (r == c).astype(np.float32)
        orig[dst] = eids

        # one-hot (edge-major, for the scatter) and slot-major (for P bcast)
        sel = np.zeros((Epad, 128), dtype=NP16)
        valid = slot >= 0
        vs = np.nonzero(valid)[0]
        sel[vs, slot[valid]] = 1.0
        selT = np.zeros((128, Epad), dtype=NP16)
        selT[slot[valid], vs] = 1.0

        pos = posg[k * NB:(k + 1) * NB]
        own = cnt_full[k * NB:(k + 1) * NB]
        tmp = np.zeros(BLOCKS * 128, dtype=np.float32)
        tmp[pos] = own
        cnt_nm = tmp.reshape(BLOCKS, 128).T.copy()

        x_own = np.zeros(NPAD, dtype=np.float32)
        x_own[pos] = xf[k * NB:(k + 1) * NB]

        em = lambda a: a.reshape(ET, 128).T.copy()  # edge-slot-major [128, ET]
        per_core.append(dict(
            ea_r=ea_s.reshape(NCH, 512),
            ea_em=em(ea_s),
            dm_em=em(dm).astype(NP16),
            dmc_em=em((1.0 - dm) * (slot >= 0)).astype(NP16),
            gq_idx=_wrap_idxs(gq),
            sel=sel,
            selT=selT,
            x_r=x_own.reshape(NPAD // 512, 512).astype(NP16),
            cnt_nm=cnt_nm,
            orig=orig,
        ))
    return per_core, Tb, a_ok


def _weights_inputs(inp):
    """Build the weight/bias input arrays (shared across cores).

    Host-side weight folding:
      Wf[s]   = W1(s-1) @ eW0c(s)      (le never materialized on device)
      b0p[s]  = eb0(s) + eW0c(s).T @ b1(s-1)
      nb0p[s] = nb0(s) + nW0b(s).T @ eb1(s)   (agg carries no b1 term)
      Wdecf   = eW1(2) @ dec_W0 ;  db0p = dec_b0 + dec_W0.T @ eb1(2)
    """
    g = lambda name: np.asarray(inp[name], dtype=np.float32)
    w = {}
    col = lambda a: a.reshape(128, 1).astype(np.float32)

    w["encn_W0h"] = g("encn_W0").reshape(1, L).astype(NP16)
    w["encn_b0"] = col(g("encn_b0"))
    w["encn_W1h"] = g("encn_W1").astype(NP16)
    w["encn_b1"] = col(g("encn_b1"))
    w["ence_W0"] = g("ence_W0").reshape(1, L)
    w["ence_b0"] = col(g("ence_b0"))
    eW0, eb0, eW1, eb1 = g("eW0"), g("eb0"), g("eW1"), g("eb1")
    nW0, nb0, nW1, nb1 = g("nW0"), g("nb0"), g("nW1"), g("nb1")
    ence_W1, ence_b1 = g("ence_W1"), g("ence_b1")
    for s in range(S):
        eW0c = eW0[s, 2 * L:, :]                       # [L, L]
        W1prev = ence_W1 if s == 0 else eW1[s - 1]
        b1prev = ence_b1 if s == 0 else eb1[s - 1]
        w[f"Wf_{s}"] = (W1prev @ eW0c).astype(NP16)
        w[f"b0p_{s}"] = col(eb0[s] + eW0c.T @ b1prev)
        w[f"eW0ab_{s}"] = np.concatenate(
            [eW0[s, :L, :], eW0[s, L:2 * L, :]], axis=1).astype(NP16)
        w[f"eW1h_{s}"] = eW1[s].astype(NP16)
    for s in range(S - 1):
        w[f"nW0a_{s}"] = nW0[s, :L, :].astype(NP16)
        w[f"nW0bh_{s}"] = nW0[s, L:, :].astype(NP16)
        w[f"nb0p_{s}"] = col(nb0[s] + nW0[s, L:, :].T @ eb1[s])
        w[f"nW1h_{s}"] = nW1[s].astype(NP16)
        w[f"nb1_{s}"] = col(nb1[s])
    dec_W0, dec_b0 = g("dec_W0"), g("dec_b0")
    w["Wdecf"] = (eW1[2] @ dec_W0).astype(NP16)
    w["db0p"] = col(dec_b0 + dec_W0.T @ eb1[2])
    w["dec_W1h"] = g("dec_W1").reshape(L, 1).astype(NP16)
    w["dec_b1"] = np.full((128, 1), float(np.asarray(inp["dec_b1"]).reshape(-1)[0]),
                          dtype=np.float32)
    w["ident"] = np.eye(128, dtype=np.float32)
    return w


# ----------------------------------------------------------------------------
# Device program
# ----------------------------------------------------------------------------

def _build(nc, Tb, w_shapes, a_ok):
    kb_no_gather = bool(int(os.environ.get("KB_NO_GATHER", "0")))
    kb_no_cc = bool(int(os.environ.get("KB_NO_CC", "0")))
    kb_sp = bool(int(os.environ.get("KB_SP", "1")))
    kb_hbm = bool(int(os.environ.get("KB_GSRC_HBM", "0")))
    EB = Tb * 128
    Epad = BLOCKS * EB
    ET = Epad // 128
    NCH = Epad // 512
    # gather groups (in tiles) per block
    groups = []
    t0 = 0
    while t0 < Tb:
        groups.append((t0, min(GT, Tb - t0)))
        t0 += GT

    din = {}

    def inp(name, shape, dtype):
        din[name] = nc.dram_tensor(name, shape, dtype, kind="ExternalInput")
        return din[name]

    inp("ea_r", [NCH, 512], F32)
    inp("ea_em", [128, ET], F32)
    inp("dm_em", [128, ET], F16)
    inp("dmc_em", [128, ET], F16)
    inp("gq_idx", [128, Epad // 16], I16)
    inp("sel", [Epad, 128], F16)
    inp("selT", [128, Epad], F16)
    inp("x_r", [NPAD // 512, 512], F16)
    inp("cnt_nm", [128, BLOCKS], F32)
    for name, arr_shape, np_dtype in w_shapes:
        inp(name, list(arr_shape), F16 if np_dtype == NP16 else F32)

    out_em = nc.dram_tensor("out_em", [128, ET], F32, kind="ExternalOutput")

    with tile.TileContext(nc) as tc, ExitStack() as ctx:
        P = lambda name, bufs, **kw: ctx.enter_context(
            tc.tile_pool(name=name, bufs=bufs, **kw))
        const = P("const", 1)
        big = P("big", 1)
        dram = P("dram", 1, space="DRAM")
        selp = P("selp", 3)      # edge-major one-hot per block
        seltp = P("seltp", 3)    # slot-major one-hot per block
        gath = P("gath", 3)      # gathered Q tiles
        hring = P("hring", 4)
        letring = P("letring", 3)
        sgring = P("sgring", 2)
        rows16 = P("rows16", 2)
        ps_pre = P("ps_pre", 2, space="PSUM")
        ps_let = P("ps_let", 2, space="PSUM")
        ps_g = P("ps_g", 2, space="PSUM")
        ps_m = P("ps_m", 2, space="PSUM")

        # ---- load constants / weights ----
        # node-encoder input rows first: they head the Sync DMA queue so the
        # node-enc -> pq -> AllGather(0) critical path starts immediately
        xrows = const.tile([1, NPAD], F16, name="xrows")
        for j in range(NPAD // 512):
            nc.sync.dma_start(xrows[:, 512 * j:512 * (j + 1)],
                              din["x_r"][j:j + 1, :])
        W = {}
        for name, arr_shape, np_dtype in w_shapes:
            t = const.tile(list(arr_shape), F16 if np_dtype == NP16 else F32,
                           name=f"w_{name}")
            nc.sync.dma_start(t[:], din[name][:])
            W[name] = t
        cnt = const.tile([128, BLOCKS], F32, name="cnt_s")
        nc.sync.dma_start(cnt[:], din["cnt_nm"][:])

        invc = const.tile([128, BLOCKS], F32, name="invc")
        nc.vector.tensor_scalar_max(invc[:], cnt[:], 1.0)
        nc.vector.reciprocal(invc[:], invc[:])

        # ---- persistent big tensors ----
        h = big.tile([128, Epad], F16, name="h")          # edge hidden latent
        ln = big.tile([128, NPAD], F16, name="ln")        # own-node latent
        agg = big.tile([128, NPAD], F16, name="agg")      # aggregated messages
        pqtab = big.tile([128, BLOCKS, 256], F16, name="pqtab")  # local P|Q
        qtab = big.tile([128, NCORES * BLOCKS, 128], F16, name="qtab")
        dec_em = big.tile([128, ET], F32, name="dec_em")

        # DRAM staging for the split Q-table AllGathers (partition-major:
        # [128 parts, nb ranks, 128 feats] per core per part). Uneven 16/4
        # split so the step-end tail collective is small; both sizes are
        # multiples of UNIT=4 so the qtab pack stays a single strided DMA.
        UNIT = 4
        HSPLIT = [(0, HS1), (HS1, BLOCKS)]
        # step 0 ships the whole table in one collective (it is the first cc,
        # so it also absorbs the one-time comm-init barrier); steps 1-2 use
        # the HS1 split so the AllGathers fire mid-step.
        qown = [None] + [
            [dram.tile([128, (b1 - b0) * 128], F16, name=f"qown_{s}_{hf}")
             for hf, (b0, b1) in enumerate(HSPLIT)] for s in range(1, S)]
        qfull = [None] + [
            [dram.tile([NCORES, 128, (b1 - b0) * 128], F16,
                       name=f"qfull_{s}_{hf}", addr_space="Shared")
             for hf, (b0, b1) in enumerate(HSPLIT)] for s in range(1, S)]
        qown0 = dram.tile([128, BLOCKS * 128], F16, name="qown0")
        qfull0 = dram.tile([NCORES, 128, BLOCKS * 128], F16,
                           name="qfull0", addr_space="Shared")

        def half_of(b):
            for hf, (b0, b1) in enumerate(HSPLIT):
                if b0 <= b < b1:
                    return hf, b - b0
            raise AssertionError

        def mlp_rows(src_row, nrows, hidden_W0, b0, W1h, b1, dst):
            """dst[:, 512j:...] = W1h.T @ relu(W0 (x) row_j + b0) + b1."""
            for j in range(nrows):
                ps = ps_m.tile([128, 512], F32, tag="m")
                nc.tensor.matmul(ps[:], hidden_W0[:],
                                 src_row[:, 512 * j:512 * (j + 1)],
                                 start=True, stop=True)
                h0 = hring.tile([128, 512], F16, tag="h")
                nc.scalar.activation(h0[:], ps[:], AF.Relu, bias=b0[:])
                ps2 = ps_pre.tile([128, 512], F32, tag="pre")
                nc.tensor.matmul(ps2[:], W1h[:], h0[:], start=True, stop=True)
                nc.scalar.activation(dst[:, 512 * j:512 * (j + 1)], ps2[:],
                                     AF.Identity, bias=b1[:])

        def pq_block(s, b):
            """pqtab[:, b] = ln_b @ [eW0a(s)|eW0b(s)]; Q half DMAs to qown."""
            ps = ps_m.tile([128, 256], F32, tag="m")
            nc.tensor.matmul(ps[:], ln[:, 128 * b:128 * (b + 1)],
                             W[f"eW0ab_{s}"][:], start=True, stop=True)
            nc.scalar.activation(pqtab[:, b, :], ps[:], AF.Copy)
            if s == 0:
                nc.sync.dma_start(qown0[:, 128 * b:128 * (b + 1)],
                                  pqtab[:, b, 128:256])
            else:
                hf, rb = half_of(b)
                nc.sync.dma_start(qown[s][hf][:, 128 * rb:128 * (rb + 1)],
                                  pqtab[:, b, 128:256])

        def _cc(in_tile, out_tile):
            if kb_no_cc:
                nc.sync.dma_start(out_tile[0, :, :], in_tile[:])
            else:
                nc.gpsimd.collective_compute(
                    "AllGather", OP.bypass,
                    replica_groups=[list(range(NCORES))],
                    ins=[in_tile.opt()],
                    outs=[out_tile.opt()],
                )

        def qshare(s, hf):
            """AllGather one half of the Q table (qtab load happens later)."""
            _cc(qown[s][hf], qfull[s][hf])

        def qtab_load(s, hf):
            """Pack one qfull half into qtab. Issued on the GpSimd engine so
            its FIFO orders these writes after every step-(s-1) gather (Tile
            does not track dma_gather's read of qtab). Ranks are numbered so
            each half is a contiguous qtab slice -> one strided DMA."""
            b0, b1 = HSPLIT[hf]
            nb = b1 - b0
            off = NCORES * sum(e - a for a, e in HSPLIT[:hf])
            src = (qfull0[:, :, 128 * b0:128 * b1] if s == 0
                   else qfull[s][hf][:])
            nc.gpsimd.dma_start(
                qtab[:, off:off + NCORES * nb, :].rearrange(
                    "p (j rk) f -> p j (rk f)", j=NCORES),
                src.rearrange("j p x -> p j x"))

        # ---- encoders / initial tables ----
        # node encoder first so pq_pass(0) + AllGather overlap the edge encoder
        mlp_rows(xrows, NPAD // 512, W["encn_W0h"], W["encn_b0"],
                 W["encn_W1h"], W["encn_b1"], ln)
        for b in range(BLOCKS):
            pq_block(0, b)
        if kb_no_cc:
            nc.sync.dma_start(qfull0[0, :, :], qown0[:])
        else:
            nc.gpsimd.collective_compute(
                "AllGather", OP.bypass,
                replica_groups=[list(range(NCORES))],
                ins=[qown0.opt()],
                outs=[qfull0.opt()],
            )
        # bulk constants, traced after the step-0 collective so they don't
        # delay the node encoder -> pq -> AllGather critical path
        # gq_idx is consumed by dma_gather, whose input reads Tile does not
        # track; issue its load on the GpSimd engine so the SWDGE ring
        # orders it ahead of every gather.
        gq_idx = const.tile([128, Epad // 16], I16, name="gq_idx_s")
        nc.gpsimd.dma_start(gq_idx[:], din["gq_idx"][:])
        ea_em = const.tile([128, ET], F32, name="ea_em_s")
        nc.sync.dma_start(ea_em[:], din["ea_em"][:])
        dm_em = const.tile([128, ET], F16, name="dm_em_s")
        nc.sync.dma_start(dm_em[:], din["dm_em"][:])
        dmc_em = const.tile([128, ET], F16, name="dmc_em_s")
        nc.sync.dma_start(dmc_em[:], din["dmc_em"][:])
        # edge encoder: FIRST layer only -> h ; second layer folds into Wf_0.
        for j in range(NCH):
            r = rows16.tile([1, 512], F16, tag="rowin16")
            nc.sync.dma_start(r[:], din["ea_r"][j:j + 1, :])
            ps = ps_m.tile([128, 512], F32, tag="m")
            nc.tensor.matmul(ps[:], W["ence_W0h"][:], r[:], start=True, stop=True)
            nc.scalar.activation(h[:, 512 * j:512 * (j + 1)], ps[:],
                                 AF.Relu, bias=W["ence_b0"][:])

        for s in range(S):
            do_agg = s < S - 1
            Wf = W[f"Wf_{s}"]
            b0p = W[f"b0p_{s}"]
            Wscat = W[f"eW1h_{s}"]

            gq_store = {}

            def emit_gather(b, gi):
                gt0, gnt = groups[gi]
                i0 = b * EB + gt0 * 128
                ni = gnt * 128
                gq_t = gath.tile([128, 1, GT * 128], F16, tag="gq")
                if kb_no_gather:
                    nc.vector.memset(gq_t[:], 0.0)
                else:
                    nc.gpsimd.dma_gather(
                        gq_t[:, :, :ni], qtab[:],
                        gq_idx[:, i0 // 16:(i0 + ni) // 16],
                        num_idxs=ni, num_idxs_reg=ni,
                        elem_size=128, transpose=True,
                        single_packet=kb_sp,
                        sbuf_tokens_per_rank=128,
                        sbuf_free_dim_per_rank=256,
                        sbuf_byte_offset=0)
                gq_store[(b, gi)] = gq_t

            qtab_load(s, 0)
            if a_ok:
                # each block's first group only references half-A rows, so
                # two of those gathers can run before the half-B table load
                # (covering the tail AllGather + load latency)
                emit_gather(0, 0)
                emit_gather(1, 0)
                emit_gather(2, 0)
                emit_gather(3, 0)
            qtab_load(s, 1)
            for b in range(BLOCKS):
                if do_agg:
                    g_ps = ps_g.tile([128, 128], F32, tag="g")
                    sel_t = selp.tile([128, Tb, 128], F16, tag="sel")
                    nc.sync.dma_start(
                        sel_t[:],
                        din["sel"][b * EB:(b + 1) * EB, :].rearrange(
                            "(t p) s -> p t s", p=128))
                selT_t = seltp.tile([128, EB], F16, tag="selT")
                nc.sync.dma_start(selT_t[:], din["selT"][:, b * EB:(b + 1) * EB])
                for gi, (gt0, gnt) in enumerate(groups):
                    i0 = b * EB + gt0 * 128
                    ni = gnt * 128
                    if (b, gi) in gq_store:
                        gq_t = gq_store.pop((b, gi))
                    else:
                        emit_gather(b, gi)
                        gq_t = gq_store.pop((b, gi))
                    # chunks of <=512 within the group
                    co = 0
                    while co < ni:
                        cw = min(512, ni - co)
                        goff = i0 + co            # global edge-slot offset
                        lo = gt0 * 128 + co       # offset within block
                        ps = ps_pre.tile([128, 512], F32, tag="pre")
                        nc.tensor.matmul(ps[:, :cw], Wf[:],
                                         h[:, goff:goff + cw],
                                         start=True, stop=False)
                        nc.tensor.matmul(ps[:, :cw], pqtab[:, b, 0:128],
                                         selT_t[:, lo:lo + cw],
                                         start=False, stop=True)
                        pre16 = hring.tile([128, 512], F16, tag="h")
                        nc.vector.tensor_tensor(pre16[:, :cw], ps[:, :cw],
                                                gq_t[:, 0, co:co + cw],
                                                op=OP.add)
                        nc.vector.tensor_scalar(h[:, goff:goff + cw],
                                                pre16[:, :cw], b0p[:],
                                                0.0, op0=OP.add, op1=OP.max)
                        if s == S - 1:
                            # decoder fused into the last step's chunk loop
                            # (dec layer-0 absorbed le = W1(2).T h + b1)
                            dps = ps_let.tile([128, 512], F32, tag="let")
                            nc.tensor.matmul(dps[:, :cw], W["Wdecf"][:],
                                             h[:, goff:goff + cw],
                                             start=True, stop=True)
                            hd = hring.tile([128, 512], F16, tag="h")
                            nc.scalar.activation(hd[:, :cw], dps[:, :cw],
                                                 AF.Relu, bias=W["db0p"][:])
                            d_ps = ps_m.tile([128, 4], F32, tag="m")
                            for u in range(cw // 128):
                                nc.tensor.matmul(
                                    d_ps[:, u:u + 1],
                                    hd[:, 128 * u:128 * (u + 1)],
                                    W["dec_W1h"][:], start=True, stop=True,
                                    skip_group_check=True)
                            nc.vector.tensor_scalar_add(
                                dec_em[:, goff // 128:goff // 128 + cw // 128],
                                d_ps[:, :cw // 128], W["dec_b1"][:])
                        if do_agg:
                            let_ps = ps_let.tile([128, 512], F32, tag="let")
                            for u in range(cw // 128):
                                nc.tensor.matmul(
                                    let_ps[:, 128 * u:128 * (u + 1)],
                                    h[:, goff + 128 * u:goff + 128 * (u + 1)],
                                    Wscat[:], start=True, stop=True,
                                    skip_group_check=True)
                            let_sb = letring.tile([128, 512], F16, tag="let")
                            nc.scalar.activation(let_sb[:, :cw], let_ps[:, :cw],
                                                 AF.Copy)
                            for u in range(cw // 128):
                                tt = (lo // 128) + u
                                nc.tensor.matmul(
                                    g_ps[:], sel_t[:, tt, :],
                                    let_sb[:, 128 * u:128 * (u + 1)],
                                    start=(tt == 0), stop=(tt == Tb - 1),
                                    skip_group_check=True)
                        co += cw
                if do_agg:
                    # finish this block's aggregation, then immediately run
                    # its node MLP + next-step P/Q so the AllGather halves
                    # launch mid-step instead of serializing at the step end.
                    sg = sgring.tile([128, 128], F32, tag="sg")
                    nc.vector.tensor_scalar_mul(sg[:], g_ps[:], invc[:, b:b + 1])
                    ps_t = ps_m.tile([128, 128], F32, tag="m")
                    nc.tensor.transpose(ps_t[:], sg[:], W["ident"][:])
                    nc.scalar.activation(agg[:, 128 * b:128 * (b + 1)], ps_t[:],
                                         AF.Copy)
                    o = 128 * b
                    p_ps = ps_m.tile([128, 128], F32, tag="m")
                    nc.tensor.matmul(p_ps[:], W[f"nW0a_{s}"][:],
                                     ln[:, o:o + 128], start=True, stop=False)
                    nc.tensor.matmul(p_ps[:], W[f"nW0bh_{s}"][:],
                                     agg[:, o:o + 128], start=False, stop=True)
                    hn = hring.tile([128, 128], F16, tag="h")
                    nc.scalar.activation(hn[:], p_ps[:], AF.Relu,
                                         bias=W[f"nb0p_{s}"][:])
                    l_ps = ps_m.tile([128, 128], F32, tag="m")
                    nc.tensor.matmul(l_ps[:], W[f"nW1h_{s}"][:], hn[:],
                                     start=True, stop=True)
                    nc.scalar.activation(ln[:, o:o + 128], l_ps[:],
                                         AF.Identity, bias=W[f"nb1_{s}"][:])
                    pq_block(s + 1, b)
                    for hf, (b0, b1) in enumerate(HSPLIT):
                        if b == b1 - 1:
                            qshare(s + 1, hf)

        # ---- final combine (in ea_em): out = dm*0.5*sqrt(ea) + dmc*dec ----
        nc.scalar.sqrt(ea_em[:], ea_em[:])
        nc.vector.scalar_tensor_tensor(ea_em[:], dm_em[:], 0.5, ea_em[:],
                                       op0=OP.mult, op1=OP.mult)
        nc.vector.tensor_tensor(dec_em[:], dmc_em[:], dec_em[:], op=OP.mult)
        nc.vector.tensor_tensor(ea_em[:], ea_em[:], dec_em[:], op=OP.add)
        nc.sync.dma_start(out_em[:], ea_em[:])

    nc.compile()


# ----------------------------------------------------------------------------
# Entry point
# ----------------------------------------------------------------------------

def _get_program(Tb, w_shapes):
    key = (Tb, os.environ.get("KB_GSRC_HBM"), os.environ.get("KB_SP"),
           os.environ.get("KB_NQ"))
    if key not in _CACHE:
        import time
        t0 = time.time()
        nc = bacc.Bacc("TRN2", target_bir_lowering=False, debug=False,
                       num_devices=NCORES)
        _build(nc, Tb, w_shapes, a_ok)
        if os.environ.get("KERNEL_VERBOSE"):
            print(f"[kernel] build+schedule+compile: {time.time()-t0:.1f}s",
                  flush=True)
        _CACHE[key] = nc
    return _CACHE[key]


def kernel(**inputs):
    per_core, Tb, a_ok = _prep(inputs["x"], inputs["edge_attr"], inputs["edge_index"])
    w = _weights_inputs(inputs)
    w_shapes = [(k, v.shape, v.dtype.type) for k, v in w.items()]
    nc = _get_program(Tb, w_shapes, a_ok)

    in_maps = []
    for k in range(NCORES):
        m = dict(w)
        pc = per_core[k]
        for key in ("ea_r", "ea_em", "dm_em", "dmc_em", "gq_idx",
                    "sel", "selT", "x_r", "cnt_nm"):
            m[key] = pc[key]
        in_maps.append(m)

    trace = bool(int(os.environ.get("KERNEL_TRACE", "0")))
    import time as _time
    _t0 = _time.time()
    res = run_bass_kernel_spmd(
        nc, in_maps, core_ids=list(range(NCORES)), trace=trace,
        tmpdir=os.environ.get("KERNEL_TRACE_DIR") or None)
    if os.environ.get("KERNEL_VERBOSE"):
        print(f"[kernel] exec phase: {_time.time()-_t0:.1f}s", flush=True)
    if trace:
        print(f"HW exec time: {res.exec_time_ns} ns")
        if res.instructions_and_trace:
            print("trace:", res.instructions_and_trace[1])

    out = np.zeros((E, 1), dtype=np.float32)
    ET = (BLOCKS * Tb * 128) // 128
    for k in range(NCORES):
        o = res.results[k]["out_em"]           # [128, ET]
        flat = o.T.reshape(-1)                 # slot order
        orig = per_core[k]["orig"]
        valid = orig >= 0
        out[orig[valid], 0] = flat[valid]
    return out


# revision 74
# speedup vs baseline: 1.4258x; 1.0031x over previous
"""Trainium2 Bass kernel for NeuralPCG GNN message passing (8 NeuronCores).

Strategy: destination-sharded edges (core k owns all edges whose dest node is
in its 2500-node range), feature-major fp16 matmuls.

Per message-passing step, for each edge e=(r,c):
    pre_h = Wf.T @ h_prev  +  P[r]  +  Q[c]          (PSUM accumulate)
    h     = relu(pre_h + b0')                        (one DVE op)
where Wf = W1(s-1) @ eW0c(s) is host-folded (le never materialized),
P[r] comes from a one-hot sel matmul against the local P table, and
Q[c] comes from an SBUF-source dma_gather out of a packed Q table that
is AllGathered across the 8 cores each step.

The mean aggregation scatters le' = W1.T @ h edge-major via per-tile
matmuls (lhsT=h_tile, rhs=W1) then one-hot sel matmuls accumulating
per-block segment sums in PSUM.
"""
import os
import numpy as np
import ml_dtypes
from contextlib import ExitStack

import concourse.bass as bass
import concourse.tile as tile
from concourse import bacc, mybir
from concourse.bass_utils import run_bass_kernel_spmd

N = 20000
E = 320000
L = 128
S = 3
NCORES = 8
NB = 2500            # nodes per core
BLOCKS = 20          # 128-node blocks per core
NPAD = BLOCKS * 128  # 2560
GT = int(os.environ.get("KB_GT", "6"))  # tiles per gather group
HS1 = int(os.environ.get("KB_HS1", "18"))  # blocks in AllGather half A

F32 = mybir.dt.float32
F16 = mybir.dt.float16
I16 = mybir.dt.int16
AF = mybir.ActivationFunctionType
OP = mybir.AluOpType

NP16 = np.float16

_CACHE = {}


# ----------------------------------------------------------------------------
# Host-side graph preprocessing (index manipulation + sharding only)
# ----------------------------------------------------------------------------

def _wrap_idxs(idx):
    """[n] int -> [128, n//16] int16 wrapped layout for dma_gather."""
    n = idx.shape[0]
    assert n % 16 == 0
    block = idx.reshape(n // 16, 16).T.astype(np.int16)
    return np.tile(block, (8, 1))


def _prep(x, edge_attr, edge_index):
    row = np.asarray(edge_index[0]).astype(np.int64)
    col = np.asarray(edge_index[1]).astype(np.int64)
    ea = np.asarray(edge_attr).reshape(-1).astype(np.float32)
    xf = np.asarray(x).reshape(-1).astype(np.float32)

    cnt_full = np.bincount(row, minlength=N).astype(np.float32)
    core_of = row // NB

    # Bin-pack each core's nodes into its 20 slot-blocks so per-block edge
    # counts are balanced (greedy LPT): the padded tile count Tb is set by
    # the WORST block, and consecutive-id blocks leave ~13% padding.
    posg = np.empty(N, dtype=np.int64)   # node -> within-core slot position
    for k in range(NCORES):
        deg = cnt_full[k * NB:(k + 1) * NB]
        order = np.argsort(-deg, kind="stable")
        bl = np.zeros(BLOCKS)
        bn = np.zeros(BLOCKS, dtype=np.int64)
        pos = np.empty(NB, dtype=np.int64)
        for i in order:
            cand = np.nonzero(bn < 128)[0]
            b = cand[np.argmin(bl[cand])]
            pos[i] = b * 128 + bn[b]
            bn[b] += 1
            bl[b] += deg[i] - 1  # self-loop edges are not gathered
        posg[k * NB:(k + 1) * NB] = pos
    blk_of = posg // 128
    slot_of = posg % 128

    # one self-loop edge per node goes to a dedicated per-block tile (its
    # Q[c]=Q[r] comes from the local table, no gather); everything else is
    # the gathered "regular" region of TbR tiles per block.
    first_self = np.zeros(E, dtype=bool)
    diag = row == col
    seen = np.zeros(N, dtype=bool)
    for e in np.nonzero(diag)[0]:
        n = row[e]
        if not seen[n]:
            seen[n] = True
            first_self[e] = True

    cores = []
    ebc_max = 0
    a_ok = True
    for k in range(NCORES):
        keids = np.nonzero(core_of == k)[0]
        selfe = keids[first_self[keids]]
        eids = keids[~first_self[keids]]
        blk_e = blk_of[row[eids]]
        # within each block put half-A-referencing edges (col in any core's
        # blocks [0, HS1)) first, so each block's first gather group only
        # needs the big (early) AllGather half
        isA = blk_of[col[eids]] < HS1
        order = np.lexsort((~isA, blk_e))
        eids = eids[order]
        blk = blk_of[row[eids]]
        bc = np.bincount(blk, minlength=BLOCKS)
        bcA = np.bincount(blk[blk_of[col[eids]] < HS1], minlength=BLOCKS)
        if np.any(bcA < np.minimum(bc, GT * 128)):
            a_ok = False
        ebc_max = max(ebc_max, int(bc.max()))
        cores.append((eids, blk, bc, selfe))

    Tb = max(6, (ebc_max + 127) // 128)   # regular (gathered) tiles per block
    EB = (Tb + 1) * 128                   # + 1 self-loop tile per block
    Epad = BLOCKS * EB
    ET = Epad // 128  # number of 128-edge tiles
    NCH = Epad // 512 if Epad % 512 == 0 else -1
    assert Epad % 512 == 0

    # Gather-table row numbering. Ranks are laid out so each AllGather half
    # is a contiguous slice of qtab: first all cores' blocks [0, HS1), then
    # all cores' blocks [HS1, BLOCKS).
    def trow(n):
        j = n // NB
        b, sl = blk_of[n], slot_of[n]
        g = np.where(b < HS1, j * HS1 + b,
                     NCORES * HS1 + j * (BLOCKS - HS1) + (b - HS1))
        return g * 128 + sl

    per_core = []
    for k in range(NCORES):
        eids, blk, bc, selfe = cores[k]
        starts = np.zeros(BLOCKS, dtype=np.int64)
        np.cumsum(bc[:-1], out=starts[1:])
        pos_in_blk = np.arange(len(eids)) - starts[blk]
        dst_reg = blk * EB + pos_in_blk
        # self-loop edges: tile Tb of their block, position = slot
        dst_self = blk_of[row[selfe]] * EB + Tb * 128 + slot_of[row[selfe]]
        eids = np.concatenate([eids, selfe])
        dst = np.concatenate([dst_reg, dst_self])
        r = row[eids]
        c = col[eids]

        gq = np.zeros(Epad, dtype=np.int64)
        slot = np.full(Epad, -1, dtype=np.int64)
        ea_s = np.ones(Epad, dtype=np.float32)
        dm = np.zeros(Epad, dtype=np.float32)
        orig = np.full(Epad, -1, dtype=np.int64)

        gq[dst] = trow(c)
        slot[dst] = slot_of[r]
        ea_s[dst] = ea[eids]
        dm[dst] = (r == c).astype(np.float32)
        orig[dst] = eids

        # one-hot (edge-major, for the scatter) and slot-major (for P bcast)
        sel = np.zeros((Epad, 128), dtype=NP16)
        valid = slot >= 0
        vs = np.nonzero(valid)[0]
        sel[vs, slot[valid]] = 1.0
        selT = np.zeros((128, Epad), dtype=NP16)
        selT[slot[valid], vs] = 1.0

        pos = posg[k * NB:(k + 1) * NB]
        own = cnt_full[k * NB:(k + 1) * NB]
        tmp = np.zeros(BLOCKS * 128, dtype=np.float32)
        tmp[pos] = own
        cnt_nm = tmp.reshape(BLOCKS, 128).T.copy()

        x_own = np.zeros(NPAD, dtype=np.float32)
        x_own[pos] = xf[k * NB:(k + 1) * NB]

        em = lambda a: a.reshape(ET, 128).T.copy()  # edge-slot-major [128, ET]
        per_core.append(dict(
            ea_r=ea_s.reshape(NCH, 512),
            ea_em=em(ea_s),
            dm_em=em(dm).astype(NP16),
            dmc_em=em((1.0 - dm) * (slot >= 0)).astype(NP16),
            gq_idx=_wrap_idxs(gq),
            sel=sel,
            selT=selT,
            x_r=x_own.reshape(NPAD // 512, 512).astype(NP16),
            cnt_nm=cnt_nm,
            orig=orig,
        ))
    return per_core, Tb, a_ok


def _weights_inputs(inp):
    """Build the weight/bias input arrays (shared across cores).

    Host-side weight folding:
      Wf[s]   = W1(s-1) @ eW0c(s)      (le never materialized on device)
      b0p[s]  = eb0(s) + eW0c(s).T @ b1(s-1)
      nb0p[s] = nb0(s) + nW0b(s).T @ eb1(s)   (agg carries no b1 term)
      Wdecf   = eW1(2) @ dec_W0 ;  db0p = dec_b0 + dec_W0.T @ eb1(2)
    """
    g = lambda name: np.asarray(inp[name], dtype=np.float32)
    w = {}
    col = lambda a: a.reshape(128, 1).astype(np.float32)

    w["encn_W0h"] = g("encn_W0").reshape(1, L).astype(NP16)
    w["encn_b0"] = col(g("encn_b0"))
    w["encn_W1h"] = g("encn_W1").astype(NP16)
    w["encn_b1"] = col(g("encn_b1"))
    w["ence_W0"] = g("ence_W0").reshape(1, L)
    w["ence_b0"] = col(g("ence_b0"))
    eW0, eb0, eW1, eb1 = g("eW0"), g("eb0"), g("eW1"), g("eb1")
    nW0, nb0, nW1, nb1 = g("nW0"), g("nb0"), g("nW1"), g("nb1")
    ence_W1, ence_b1 = g("ence_W1"), g("ence_b1")
    for s in range(S):
        eW0c = eW0[s, 2 * L:, :]                       # [L, L]
        W1prev = ence_W1 if s == 0 else eW1[s - 1]
        b1prev = ence_b1 if s == 0 else eb1[s - 1]
        w[f"Wf_{s}"] = (W1prev @ eW0c).astype(NP16)
        w[f"b0p_{s}"] = col(eb0[s] + eW0c.T @ b1prev)
        w[f"eW0ab_{s}"] = np.concatenate(
            [eW0[s, :L, :], eW0[s, L:2 * L, :]], axis=1).astype(NP16)
        w[f"eW1h_{s}"] = eW1[s].astype(NP16)
    for s in range(S - 1):
        w[f"nW0a_{s}"] = nW0[s, :L, :].astype(NP16)
        w[f"nW0bh_{s}"] = nW0[s, L:, :].astype(NP16)
        w[f"nb0p_{s}"] = col(nb0[s] + nW0[s, L:, :].T @ eb1[s])
        w[f"nW1h_{s}"] = nW1[s].astype(NP16)
        w[f"nb1_{s}"] = col(nb1[s])
    dec_W0, dec_b0 = g("dec_W0"), g("dec_b0")
    w["Wdecf"] = (eW1[2] @ dec_W0).astype(NP16)
    w["db0p"] = col(dec_b0 + dec_W0.T @ eb1[2])
    w["dec_W1h"] = g("dec_W1").reshape(L, 1).astype(NP16)
    w["dec_b1"] = np.full((128, 1), float(np.asarray(inp["dec_b1"]).reshape(-1)[0]),
                          dtype=np.float32)
    w["ident"] = np.eye(128, dtype=np.float32)
    return w


# ----------------------------------------------------------------------------
# Device program
# ----------------------------------------------------------------------------

def _build(nc, Tb, w_shapes, a_ok):
    kb_no_gather = bool(int(os.environ.get("KB_NO_GATHER", "0")))
    kb_no_cc = bool(int(os.environ.get("KB_NO_CC", "0")))
    kb_sp = bool(int(os.environ.get("KB_SP", "1")))
    kb_hbm = bool(int(os.environ.get("KB_GSRC_HBM", "0")))
    EB = Tb * 128
    Epad = BLOCKS * EB
    ET = Epad // 128
    NCH = Epad // 512
    # gather groups (in tiles) per block
    groups = []
    t0 = 0
    while t0 < Tb:
        groups.append((t0, min(GT, Tb - t0)))
        t0 += GT

    din = {}

    def inp(name, shape, dtype):
        din[name] = nc.dram_tensor(name, shape, dtype, kind="ExternalInput")
        return din[name]

    inp("ea_r", [NCH, 512], F32)
    inp("ea_em", [128, ET], F32)
    inp("dm_em", [128, ET], F16)
    inp("dmc_em", [128, ET], F16)
    inp("gq_idx", [128, Epad // 16], I16)
    inp("sel", [Epad, 128], F16)
    inp("selT", [128, Epad], F16)
    inp("x_r", [NPAD // 512, 512], F16)
    inp("cnt_nm", [128, BLOCKS], F32)
    for name, arr_shape, np_dtype in w_shapes:
        inp(name, list(arr_shape), F16 if np_dtype == NP16 else F32)

    out_em = nc.dram_tensor("out_em", [128, ET], F32, kind="ExternalOutput")

    with tile.TileContext(nc) as tc, ExitStack() as ctx:
        P = lambda name, bufs, **kw: ctx.enter_context(
            tc.tile_pool(name=name, bufs=bufs, **kw))
        const = P("const", 1)
        big = P("big", 1)
        dram = P("dram", 1, space="DRAM")
        selp = P("selp", 3)      # edge-major one-hot per block
        seltp = P("seltp", 3)    # slot-major one-hot per block
        gath = P("gath", 3)      # gathered Q tiles
        hring = P("hring", 4)
        letring = P("letring", 3)
        sgring = P("sgring", 2)
        rows16 = P("rows16", 2)
        ps_pre = P("ps_pre", 2, space="PSUM")
        ps_let = P("ps_let", 2, space="PSUM")
        ps_g = P("ps_g", 2, space="PSUM")
        ps_m = P("ps_m", 2, space="PSUM")

        # ---- load constants / weights ----
        # node-encoder input rows first: they head the Sync DMA queue so the
        # node-enc -> pq -> AllGather(0) critical path starts immediately
        xrows = const.tile([1, NPAD], F16, name="xrows")
        for j in range(NPAD // 512):
            nc.sync.dma_start(xrows[:, 512 * j:512 * (j + 1)],
                              din["x_r"][j:j + 1, :])
        W = {}
        for name, arr_shape, np_dtype in w_shapes:
            t = const.tile(list(arr_shape), F16 if np_dtype == NP16 else F32,
                           name=f"w_{name}")
            nc.sync.dma_start(t[:], din[name][:])
            W[name] = t
        cnt = const.tile([128, BLOCKS], F32, name="cnt_s")
        nc.sync.dma_start(cnt[:], din["cnt_nm"][:])

        invc = const.tile([128, BLOCKS], F32, name="invc")
        nc.vector.tensor_scalar_max(invc[:], cnt[:], 1.0)
        nc.vector.reciprocal(invc[:], invc[:])

        # ---- persistent big tensors ----
        h = big.tile([128, Epad], F16, name="h")          # edge hidden latent
        ln = big.tile([128, NPAD], F16, name="ln")        # own-node latent
        agg = big.tile([128, NPAD], F16, name="agg")      # aggregated messages
        pqtab = big.tile([128, BLOCKS, 256], F16, name="pqtab")  # local P|Q
        qtab = big.tile([128, NCORES * BLOCKS, 128], F16, name="qtab")
        dec_em = big.tile([128, ET], F32, name="dec_em")

        # DRAM staging for the split Q-table AllGathers (partition-major:
        # [128 parts, nb ranks, 128 feats] per core per part). Uneven 16/4
        # split so the step-end tail collective is small; both sizes are
        # multiples of UNIT=4 so the qtab pack stays a single strided DMA.
        UNIT = 4
        HSPLIT = [(0, HS1), (HS1, BLOCKS)]
        # step 0 ships the whole table in one collective (it is the first cc,
        # so it also absorbs the one-time comm-init barrier); steps 1-2 use
        # the HS1 split so the AllGathers fire mid-step.
        qown = [None] + [
            [dram.tile([128, (b1 - b0) * 128], F16, name=f"qown_{s}_{hf}")
             for hf, (b0, b1) in enumerate(HSPLIT)] for s in range(1, S)]
        qfull = [None] + [
            [dram.tile([NCORES, 128, (b1 - b0) * 128], F16,
                       name=f"qfull_{s}_{hf}", addr_space="Shared")
             for hf, (b0, b1) in enumerate(HSPLIT)] for s in range(1, S)]
        qown0 = dram.tile([128, BLOCKS * 128], F16, name="qown0")
        qfull0 = dram.tile([NCORES, 128, BLOCKS * 128], F16,
                           name="qfull0", addr_space="Shared")

        def half_of(b):
            for hf, (b0, b1) in enumerate(HSPLIT):
                if b0 <= b < b1:
                    return hf, b - b0
            raise AssertionError

        def mlp_rows(src_row, nrows, hidden_W0, b0, W1h, b1, dst):
            """dst[:, 512j:...] = W1h.T @ relu(W0 (x) row_j + b0) + b1."""
            for j in range(nrows):
                ps = ps_m.tile([128, 512], F32, tag="m")
                nc.tensor.matmul(ps[:], hidden_W0[:],
                                 src_row[:, 512 * j:512 * (j + 1)],
                                 start=True, stop=True)
                h0 = hring.tile([128, 512], F16, tag="h")
                nc.scalar.activation(h0[:], ps[:], AF.Relu, bias=b0[:])
                ps2 = ps_pre.tile([128, 512], F32, tag="pre")
                nc.tensor.matmul(ps2[:], W1h[:], h0[:], start=True, stop=True)
                nc.scalar.activation(dst[:, 512 * j:512 * (j + 1)], ps2[:],
                                     AF.Identity, bias=b1[:])

        def pq_block(s, b):
            """pqtab[:, b] = ln_b @ [eW0a(s)|eW0b(s)]; Q half DMAs to qown."""
            ps = ps_m.tile([128, 256], F32, tag="m")
            nc.tensor.matmul(ps[:], ln[:, 128 * b:128 * (b + 1)],
                             W[f"eW0ab_{s}"][:], start=True, stop=True)
            nc.scalar.activation(pqtab[:, b, :], ps[:], AF.Copy)
            if s == 0:
                nc.scalar.dma_start(qown0[:, 128 * b:128 * (b + 1)],
                                    pqtab[:, b, 128:256])
            else:
                hf, rb = half_of(b)
                nc.scalar.dma_start(qown[s][hf][:, 128 * rb:128 * (rb + 1)],
                                    pqtab[:, b, 128:256])

        def _cc(in_tile, out_tile):
            if kb_no_cc:
                nc.sync.dma_start(out_tile[0, :, :], in_tile[:])
            else:
                nc.gpsimd.collective_compute(
                    "AllGather", OP.bypass,
                    replica_groups=[list(range(NCORES))],
                    ins=[in_tile.opt()],
                    outs=[out_tile.opt()],
                )

        def qshare(s, hf):
            """AllGather one half of the Q table (qtab load happens later)."""
            _cc(qown[s][hf], qfull[s][hf])

        def qtab_load(s, hf):
            """Pack one qfull half into qtab. Issued on the GpSimd engine so
            its FIFO orders these writes after every step-(s-1) gather (Tile
            does not track dma_gather's read of qtab). Ranks are numbered so
            each half is a contiguous qtab slice -> one strided DMA."""
            b0, b1 = HSPLIT[hf]
            nb = b1 - b0
            off = NCORES * sum(e - a for a, e in HSPLIT[:hf])
            src = (qfull0[:, :, 128 * b0:128 * b1] if s == 0
                   else qfull[s][hf][:])
            nc.gpsimd.dma_start(
                qtab[:, off:off + NCORES * nb, :].rearrange(
                    "p (j rk) f -> p j (rk f)", j=NCORES),
                src.rearrange("j p x -> p j x"))

        # ---- encoders / initial tables ----
        # node encoder first so pq_pass(0) + AllGather overlap the edge encoder
        mlp_rows(xrows, NPAD // 512, W["encn_W0h"], W["encn_b0"],
                 W["encn_W1h"], W["encn_b1"], ln)
        for b in range(BLOCKS):
            pq_block(0, b)
        if kb_no_cc:
            nc.sync.dma_start(qfull0[0, :, :], qown0[:])
        else:
            nc.gpsimd.collective_compute(
                "AllGather", OP.bypass,
                replica_groups=[list(range(NCORES))],
                ins=[qown0.opt()],
                outs=[qfull0.opt()],
            )
        # bulk constants, traced after the step-0 collective so they don't
        # delay the node encoder -> pq -> AllGather critical path
        # gq_idx is consumed by dma_gather, whose input reads Tile does not
        # track; issue its load on the GpSimd engine so the SWDGE ring
        # orders it ahead of every gather.
        gq_idx = const.tile([128, Epad // 16], I16, name="gq_idx_s")
        nc.gpsimd.dma_start(gq_idx[:], din["gq_idx"][:])
        ea_em = const.tile([128, ET], F32, name="ea_em_s")
        nc.sync.dma_start(ea_em[:], din["ea_em"][:])
        dm_em = const.tile([128, ET], F16, name="dm_em_s")
        nc.sync.dma_start(dm_em[:], din["dm_em"][:])
        dmc_em = const.tile([128, ET], F16, name="dmc_em_s")
        nc.sync.dma_start(dmc_em[:], din["dmc_em"][:])
        # edge encoder: FIRST layer only -> h ; second layer folds into Wf_0.
        for j in range(NCH):
            r = rows16.tile([1, 512], F16, tag="rowin16")
            nc.sync.dma_start(r[:], din["ea_r"][j:j + 1, :])
            ps = ps_m.tile([128, 512], F32, tag="m")
            nc.tensor.matmul(ps[:], W["ence_W0h"][:], r[:], start=True, stop=True)
            nc.scalar.activation(h[:, 512 * j:512 * (j + 1)], ps[:],
                                 AF.Relu, bias=W["ence_b0"][:])

        for s in range(S):
            do_agg = s < S - 1
            Wf = W[f"Wf_{s}"]
            b0p = W[f"b0p_{s}"]
            Wscat = W[f"eW1h_{s}"]

            gq_store = {}

            def emit_gather(b, gi):
                gt0, gnt = groups[gi]
                i0 = b * EB + gt0 * 128
                ni = gnt * 128
                gq_t = gath.tile([128, 1, GT * 128], F16, tag="gq")
                if kb_no_gather:
                    nc.vector.memset(gq_t[:], 0.0)
                else:
                    nc.gpsimd.dma_gather(
                        gq_t[:, :, :ni], qtab[:],
                        gq_idx[:, i0 // 16:(i0 + ni) // 16],
                        num_idxs=ni, num_idxs_reg=ni,
                        elem_size=128, transpose=True,
                        single_packet=kb_sp,
                        sbuf_tokens_per_rank=128,
                        sbuf_free_dim_per_rank=256,
                        sbuf_byte_offset=0)
                gq_store[(b, gi)] = gq_t

            qtab_load(s, 0)
            if a_ok:
                # each block's first group only references half-A rows, so
                # two of those gathers can run before the half-B table load
                # (covering the tail AllGather + load latency)
                emit_gather(0, 0)
                emit_gather(1, 0)
                emit_gather(2, 0)
                emit_gather(3, 0)
            qtab_load(s, 1)
            for b in range(BLOCKS):
                if do_agg:
                    g_ps = ps_g.tile([128, 128], F32, tag="g")
                    sel_t = selp.tile([128, Tb, 128], F16, tag="sel")
                    nc.sync.dma_start(
                        sel_t[:],
                        din["sel"][b * EB:(b + 1) * EB, :].rearrange(
                            "(t p) s -> p t s", p=128))
                selT_t = seltp.tile([128, EB], F16, tag="selT")
                nc.scalar.dma_start(selT_t[:],
                                    din["selT"][:, b * EB:(b + 1) * EB])
                for gi, (gt0, gnt) in enumerate(groups):
                    i0 = b * EB + gt0 * 128
                    ni = gnt * 128
                    if (b, gi) in gq_store:
                        gq_t = gq_store.pop((b, gi))
                    else:
                        emit_gather(b, gi)
                        gq_t = gq_store.pop((b, gi))
                    # chunks of <=512 within the group
                    co = 0
                    while co < ni:
                        cw = min(512, ni - co)
                        goff = i0 + co            # global edge-slot offset
                        lo = gt0 * 128 + co       # offset within block
                        ps = ps_pre.tile([128, 512], F32, tag="pre")
                        nc.tensor.matmul(ps[:, :cw], Wf[:],
                                         h[:, goff:goff + cw],
                                         start=True, stop=False)
                        nc.tensor.matmul(ps[:, :cw], pqtab[:, b, 0:128],
                                         selT_t[:, lo:lo + cw],
                                         start=False, stop=True)
                        pre16 = hring.tile([128, 512], F16, tag="h")
                        nc.vector.tensor_tensor(pre16[:, :cw], ps[:, :cw],
                                                gq_t[:, 0, co:co + cw],
                                                op=OP.add)
                        nc.vector.tensor_scalar(h[:, goff:goff + cw],
                                                pre16[:, :cw], b0p[:],
                                                0.0, op0=OP.add, op1=OP.max)
                        if s == S - 1:
                            # decoder fused into the last step's chunk loop
                            # (dec layer-0 absorbed le = W1(2).T h + b1)
                            dps = ps_let.tile([128, 512], F32, tag="let")
                            nc.tensor.matmul(dps[:, :cw], W["Wdecf"][:],
                                             h[:, goff:goff + cw],
                                             start=True, stop=True)
                            hd = hring.tile([128, 512], F16, tag="h")
                            nc.scalar.activation(hd[:, :cw], dps[:, :cw],
                                                 AF.Relu, bias=W["db0p"][:])
                            d_ps = ps_m.tile([128, 4], F32, tag="m")
                            for u in range(cw // 128):
                                nc.tensor.matmul(
                                    d_ps[:, u:u + 1],
                                    hd[:, 128 * u:128 * (u + 1)],
                                    W["dec_W1h"][:], start=True, stop=True,
                                    skip_group_check=True)
                            nc.vector.tensor_scalar_add(
                                dec_em[:, goff // 128:goff // 128 + cw // 128],
                                d_ps[:, :cw // 128], W["dec_b1"][:])
                        if do_agg:
                            let_ps = ps_let.tile([128, 512], F32, tag="let")
                            for u in range(cw // 128):
                                nc.tensor.matmul(
                                    let_ps[:, 128 * u:128 * (u + 1)],
                                    h[:, goff + 128 * u:goff + 128 * (u + 1)],
                                    Wscat[:], start=True, stop=True,
                                    skip_group_check=True)
                            let_sb = letring.tile([128, 512], F16, tag="let")
                            nc.scalar.activation(let_sb[:, :cw], let_ps[:, :cw],
                                                 AF.Copy)
                            for u in range(cw // 128):
                                tt = (lo // 128) + u
                                nc.tensor.matmul(
                                    g_ps[:], sel_t[:, tt, :],
                                    let_sb[:, 128 * u:128 * (u + 1)],
                                    start=(tt == 0), stop=(tt == Tb - 1),
                                    skip_group_check=True)
                        co += cw
                if do_agg:
                    # finish this block's aggregation, then immediately run
                    # its node MLP + next-step P/Q so the AllGather halves
                    # launch mid-step instead of serializing at the step end.
                    sg = sgring.tile([128, 128], F32, tag="sg")
                    nc.vector.tensor_scalar_mul(sg[:], g_ps[:], invc[:, b:b + 1])
                    ps_t = ps_m.tile([128, 128], F32, tag="m")
                    nc.tensor.transpose(ps_t[:], sg[:], W["ident"][:])
                    nc.scalar.activation(agg[:, 128 * b:128 * (b + 1)], ps_t[:],
                                         AF.Copy)
                    o = 128 * b
                    p_ps = ps_m.tile([128, 128], F32, tag="m")
                    nc.tensor.matmul(p_ps[:], W[f"nW0a_{s}"][:],
                                     ln[:, o:o + 128], start=True, stop=False)
                    nc.tensor.matmul(p_ps[:], W[f"nW0bh_{s}"][:],
                                     agg[:, o:o + 128], start=False, stop=True)
                    hn = hring.tile([128, 128], F16, tag="h")
                    nc.scalar.activation(hn[:], p_ps[:], AF.Relu,
                                         bias=W[f"nb0p_{s}"][:])
                    l_ps = ps_m.tile([128, 128], F32, tag="m")
                    nc.tensor.matmul(l_ps[:], W[f"nW1h_{s}"][:], hn[:],
                                     start=True, stop=True)
                    nc.scalar.activation(ln[:, o:o + 128], l_ps[:],
                                         AF.Identity, bias=W[f"nb1_{s}"][:])
                    pq_block(s + 1, b)
                    for hf, (b0, b1) in enumerate(HSPLIT):
                        if b == b1 - 1:
                            qshare(s + 1, hf)

        # ---- final combine (in ea_em): out = dm*0.5*sqrt(ea) + dmc*dec ----
        nc.scalar.sqrt(ea_em[:], ea_em[:])
        nc.vector.scalar_tensor_tensor(ea_em[:], dm_em[:], 0.5, ea_em[:],
                                       op0=OP.mult, op1=OP.mult)
        nc.vector.tensor_tensor(dec_em[:], dmc_em[:], dec_em[:], op=OP.mult)
        nc.vector.tensor_tensor(ea_em[:], ea_em[:], dec_em[:], op=OP.add)
        nc.sync.dma_start(out_em[:], ea_em[:])

    nc.compile()


# ----------------------------------------------------------------------------
# Entry point
# ----------------------------------------------------------------------------

def _get_program(Tb, w_shapes):
    key = (Tb, os.environ.get("KB_GSRC_HBM"), os.environ.get("KB_SP"),
           os.environ.get("KB_NQ"))
    if key not in _CACHE:
        import time
        t0 = time.time()
        nc = bacc.Bacc("TRN2", target_bir_lowering=False, debug=False,
                       num_devices=NCORES)
        _build(nc, Tb, w_shapes, a_ok)
        if os.environ.get("KERNEL_VERBOSE"):
            print(f"[kernel] build+schedule+compile: {time.time()-t0:.1f}s",
                  flush=True)
        _CACHE[key] = nc
    return _CACHE[key]


def kernel(**inputs):
    per_core, Tb, a_ok = _prep(inputs["x"], inputs["edge_attr"], inputs["edge_index"])
    w = _weights_inputs(inputs)
    w_shapes = [(k, v.shape, v.dtype.type) for k, v in w.items()]
    nc = _get_program(Tb, w_shapes, a_ok)

    in_maps = []
    for k in range(NCORES):
        m = dict(w)
        pc = per_core[k]
        for key in ("ea_r", "ea_em", "dm_em", "dmc_em", "gq_idx",
                    "sel", "selT", "x_r", "cnt_nm"):
            m[key] = pc[key]
        in_maps.append(m)

    trace = bool(int(os.environ.get("KERNEL_TRACE", "0")))
    import time as _time
    _t0 = _time.time()
    res = run_bass_kernel_spmd(
        nc, in_maps, core_ids=list(range(NCORES)), trace=trace,
        tmpdir=os.environ.get("KERNEL_TRACE_DIR") or None)
    if os.environ.get("KERNEL_VERBOSE"):
        print(f"[kernel] exec phase: {_time.time()-_t0:.1f}s", flush=True)
    if trace:
        print(f"HW exec time: {res.exec_time_ns} ns")
        if res.instructions_and_trace:
            print("trace:", res.instructions_and_trace[1])

    out = np.zeros((E, 1), dtype=np.float32)
    ET = (BLOCKS * Tb * 128) // 128
    for k in range(NCORES):
        o = res.results[k]["out_em"]           # [128, ET]
        flat = o.T.reshape(-1)                 # slot order
        orig = per_core[k]["orig"]
        valid = orig >= 0
        out[orig[valid], 0] = flat[valid]
    return out
